# revision 1
# baseline (speedup 1.0000x reference)
"""Trainium2 Bass kernel for BaselineKNNModel (cosine-sim KNN classifier).

Contract: kernel(**inputs) takes FULL inputs (x [2048,512] f32,
embeddings [100000,512] f32, labels [100000] int) and returns the FULL
output (pred [2048] labels.dtype), distributing work across 8 NeuronCores.

Strategy (database-parallel, per sharding hint):
 - Host: normalize embeddings (cosine denominator), pad N 100000->102400,
   transpose to [512, N]; shard along N across 8 cores (12800 each).
   x normalization is skipped: per-query positive scaling cannot change
   that query's top-k ranking.
 - Device (SPMD, per core): sim tile [128 q, 512 c] = xT.T @ enT chunk via
   PE accumulation over K=512; per tile, VectorE max/max_index extract the
   top-8 values + indices of each 512-candidate chunk (global top-10 of a
   row is contained in the union of its per-chunk top-8s unless >=9 of the
   top-10 fall in one 512-chunk: P ~ 1e-11).
 - Host: merge 8 cores x 25 chunks x top-8 = 1600 candidates/query, exact
   top-10 by (value desc, index asc) = jax.lax.top_k tie order, then the
   reference's mode computation.
"""
import sys

for _p in ("/opt/trn_rl_repo", "/root/.axon_site/_ro/trn_rl_repo"):
    if _p not in sys.path:
        sys.path.insert(0, _p)

import numpy as np

import concourse.bacc as bacc
import concourse.mybir as mybir
import concourse.tile as tile
from concourse import bass_utils

F32 = mybir.dt.float32
F32R = mybir.dt.float32r
F16 = mybir.dt.float16
U32 = mybir.dt.uint32
Copy = mybir.ActivationFunctionType.Copy

B = 2048            # queries
D = 512             # embedding dim
N_EMB = 100000      # database size
K_NEIGH = 10
NUM_CLASSES = 1000
EPS = 1e-8

CORES = 8
N_PAD = 102400      # padded database size (8 * 12800)
N_CORE = N_PAD // CORES     # 12800 candidates per core
CHUNK = 512                 # candidates per sim tile (one PSUM bank)
NCHUNK = N_CORE // CHUNK    # 25
QT = B // 128               # 16 query tiles
KT = D // 128               # 4 k-tiles
NOUT = NCHUNK * 8           # 200 output slots per query per core

# f16w variant: window-max + device window top-16 + host exact rescore
WWIN = 32                   # candidates per window
WPC = N_CORE // WWIN        # 400 windows per core
BIGCHUNK = 1024             # candidates per PSUM tile (2 banks)
NSEL = 16                   # windows kept per (query, core, half)
HALF_A = (7 * BIGCHUNK) // WWIN  # windows in selection half A (224)
MARGIN = 4e-3               # fp16-sim error margin on unit-normalized sims
                            # (measured max |fp16 sim err| ~6e-5, ~60x safety)

# f8w variant: same as f16w but fp8e4m3 DoubleRow matmuls (2 fp8 weights per
# PE cell, K=256 per matmul). Inputs are scaled by F8_SCALE before rounding
# to fp8, so device sims (and window maxes) are scaled by F8_SCALE^2.
F8_SCALE = 16.0
MARGIN_F8 = 2.5e-2          # fp8 margin on unit-normalized sims
                            # (measured max err 7.1e-3 on a sample, rms 1.6e-3)

MM_DTYPE = "f8e"   # "f32" | "f32r" | "f16x3" | "f16w" | "f8w" | "f8d" | "f8e"

_CACHE = {}


def _build(variant):
    """Build + compile the per-core Bass program. Same program on all cores;
    only the `ent*` input shards differ."""
    nc = bacc.Bacc("TRN2", target_bir_lowering=False, debug=False)

    if variant == "noop":  # minimal program for RPC-overhead baselining
        d_nin = nc.dram_tensor("nin", [128, 128], F32, kind="ExternalInput")
        d_nout = nc.dram_tensor("nout", [128, 128], F32, kind="ExternalOutput")
        with tile.TileContext(nc) as tc:
            with tc.tile_pool(name="np0", bufs=1) as pool:
                t = pool.tile([128, 128], F32, tag="t")
                nc.sync.dma_start(t[:, :], d_nin[:, :])
                nc.sync.dma_start(d_nout[:, :], t[:, :])
        nc.compile()
        return nc

    if variant == "f16w":
        return _build_f16w(nc)
    if variant == "f8w":
        return _build_f8w(nc)
    if variant == "f8d":
        return _build_f8d(nc)
    if variant == "f8e":
        return _build_f8e(nc)

    f16 = variant == "f16x3"
    if f16:
        d_xhi = nc.dram_tensor("xhi", [D, B], F16, kind="ExternalInput")
        d_xlo = nc.dram_tensor("xlo", [D, B], F16, kind="ExternalInput")
        d_ehi = nc.dram_tensor("ehi", [D, N_CORE], F16, kind="ExternalInput")
        d_elo = nc.dram_tensor("elo", [D, N_CORE], F16, kind="ExternalInput")
    else:
        in_dt = F32R if variant == "f32r" else F32
        d_xt = nc.dram_tensor("xt", [D, B], in_dt, kind="ExternalInput")
        d_ent = nc.dram_tensor("ent", [D, N_CORE], in_dt, kind="ExternalInput")

    d_vals = nc.dram_tensor("vals", [B, NOUT], F32, kind="ExternalOutput")
    d_idx = nc.dram_tensor("idx", [B, NOUT], U32, kind="ExternalOutput")

    with tile.TileContext(nc) as tc:
        with (
            tc.tile_pool(name="xpool", bufs=1) as xpool,
            tc.tile_pool(name="epool", bufs=3) as epool,
            tc.tile_pool(name="ps", bufs=6, space="PSUM") as ps_pool,
            tc.tile_pool(name="sim", bufs=6) as sim_pool,
            tc.tile_pool(name="acc", bufs=1) as acc_pool,
        ):
            # resident x (stationary operand), k-tiles side by side
            if f16:
                xhi_sb = xpool.tile([128, KT * B], F16, tag="xhi")
                xlo_sb = xpool.tile([128, KT * B], F16, tag="xlo")
                for k in range(KT):
                    nc.sync.dma_start(xhi_sb[:, k * B:(k + 1) * B],
                                      d_xhi[k * 128:(k + 1) * 128, :])
                    nc.sync.dma_start(xlo_sb[:, k * B:(k + 1) * B],
                                      d_xlo[k * 128:(k + 1) * 128, :])
            else:
                xt_sb = xpool.tile([128, KT * B], in_dt, tag="xt")
                for k in range(KT):
                    nc.sync.dma_start(xt_sb[:, k * B:(k + 1) * B],
                                      d_xt[k * 128:(k + 1) * 128, :])

            # result accumulators, [128, QT*NOUT], column q*NOUT + c*8 + j
            vals_sb = acc_pool.tile([128, QT * NOUT], F32, tag="vacc")
            idx_sb = acc_pool.tile([128, QT * NOUT], U32, tag="iacc")

            for c in range(NCHUNK):
                c0 = c * CHUNK
                if f16:
                    ehi_sb = epool.tile([128, KT * CHUNK], F16, tag="ehi")
                    elo_sb = epool.tile([128, KT * CHUNK], F16, tag="elo")
                    for k in range(KT):
                        nc.sync.dma_start(ehi_sb[:, k * CHUNK:(k + 1) * CHUNK],
                                          d_ehi[k * 128:(k + 1) * 128, c0:c0 + CHUNK])
                        nc.sync.dma_start(elo_sb[:, k * CHUNK:(k + 1) * CHUNK],
                                          d_elo[k * 128:(k + 1) * 128, c0:c0 + CHUNK])
                else:
                    en_sb = epool.tile([128, KT * CHUNK], in_dt, tag="en")
                    for k in range(KT):
                        nc.sync.dma_start(en_sb[:, k * CHUNK:(k + 1) * CHUNK],
                                          d_ent[k * 128:(k + 1) * 128, c0:c0 + CHUNK])

                for q in range(QT):
                    ps = ps_pool.tile([128, CHUNK], F32, tag="ps")
                    if variant == "f16x3":
                        nmm = 3 * KT
                        i = 0
                        for k in range(KT):
                            xh = xhi_sb[:, k * B + q * 128: k * B + (q + 1) * 128]
                            xl = xlo_sb[:, k * B + q * 128: k * B + (q + 1) * 128]
                            eh = ehi_sb[:, k * CHUNK:(k + 1) * CHUNK]
                            el = elo_sb[:, k * CHUNK:(k + 1) * CHUNK]
                            for (a, bb) in ((xh, eh), (xh, el), (xl, eh)):
                                nc.tensor.matmul(ps[:, :], a, bb,
                                                 start=(i == 0), stop=(i == nmm - 1))
                                i += 1
                    else:
                        for k in range(KT):
                            lhsT = xt_sb[:, k * B + q * 128: k * B + (q + 1) * 128]
                            rhs = en_sb[:, k * CHUNK:(k + 1) * CHUNK]
                            nc.tensor.matmul(ps[:, :], lhsT, rhs,
                                             start=(k == 0), stop=(k == KT - 1))

                    sim = sim_pool.tile([128, CHUNK], F32, tag="sim")
                    nc.scalar.activation(sim[:, :], ps[:, :], Copy)

                    o = q * NOUT + c * 8
                    nc.vector.max(vals_sb[:, o:o + 8], sim[:, :])
                    nc.vector.max_index(idx_sb[:, o:o + 8], vals_sb[:, o:o + 8],
                                        sim[:, :])

            for q in range(QT):
                nc.sync.dma_start(d_vals[q * 128:(q + 1) * 128, :],
                                  vals_sb[:, q * NOUT:(q + 1) * NOUT])
                nc.sync.dma_start(d_idx[q * 128:(q + 1) * 128, :],
                                  idx_sb[:, q * NOUT:(q + 1) * NOUT])

    nc.compile()
    return nc


def _build_f16w(nc):
    """fp16 single-pass matmul; per-tile 16-wide window max (DVE reduce,
    PSUM-direct); per-core-half top-16 windows per query via
    max/match_replace (first half's selection overlaps the main loop);
    host rescores the selected windows exactly."""
    Max = mybir.AluOpType.max
    X = mybir.AxisListType.X

    d_xh = nc.dram_tensor("xh", [D, B], F16, kind="ExternalInput")
    d_eh = nc.dram_tensor("eh", [D, N_CORE], F16, kind="ExternalInput")
    d_wvals = nc.dram_tensor("wvals", [B, 2 * NSEL], F32, kind="ExternalOutput")
    d_widx = nc.dram_tensor("widx", [B, 2 * NSEL], U32, kind="ExternalOutput")

    # chunk layout: 12 x 1024 + 1 x 512 = 12800
    chunks = [(i * BIGCHUNK, BIGCHUNK) for i in range(N_CORE // BIGCHUNK)]
    rem = N_CORE - (N_CORE // BIGCHUNK) * BIGCHUNK
    if rem:
        chunks.append((N_CORE - rem, rem))
    # selection halves aligned to chunk boundaries:
    # half A = chunks 0-6 (448 windows), half B = chunks 7-12 (352 windows)
    HALF_B = WPC - HALF_A

    def select(wq, vout, iout, o, width, mr_pool):
        nc.vector.max(vout[:, o:o + 8], wq)
        nc.vector.max_index(iout[:, o:o + 8], vout[:, o:o + 8], wq)
        mr = mr_pool.tile([128, width], F32, tag="mr")
        nc.vector.match_replace(mr[:, :width], vout[:, o:o + 8], wq, -1e30)
        nc.vector.max(vout[:, o + 8:o + 16], mr[:, :width])
        nc.vector.max_index(iout[:, o + 8:o + 16],
                            vout[:, o + 8:o + 16], mr[:, :width])

    with tile.TileContext(nc) as tc:
        with (
            tc.tile_pool(name="xpool", bufs=1) as xpool,
            tc.tile_pool(name="epool", bufs=3) as epool,
            tc.tile_pool(name="ps", bufs=3, space="PSUM") as ps_pool,
            tc.tile_pool(name="wacc", bufs=1) as wacc_pool,
            tc.tile_pool(name="mrp", bufs=4) as mr_pool,
            tc.tile_pool(name="outp", bufs=1) as out_pool,
        ):
            xh_sb = xpool.tile([128, KT * B], F16, tag="xh")
            for k in range(KT):
                nc.sync.dma_start(xh_sb[:, k * B:(k + 1) * B],
                                  d_xh[k * 128:(k + 1) * 128, :])

            wmax_sb = wacc_pool.tile([128, QT * WPC], F32, tag="wacc")
            vout_sb = out_pool.tile([128, QT * 2 * NSEL], F32, tag="vout")
            iout_sb = out_pool.tile([128, QT * 2 * NSEL], U32, tag="iout")

            for ci, (c0, cw) in enumerate(chunks):
                eh_sb = epool.tile([128, KT * BIGCHUNK], F16, tag="eh")
                for k in range(KT):
                    nc.sync.dma_start(eh_sb[:, k * cw:(k + 1) * cw],
                                      d_eh[k * 128:(k + 1) * 128, c0:c0 + cw])
                for q in range(QT):
                    ps = ps_pool.tile([128, BIGCHUNK], F32, tag="ps")
                    for s in range(cw // 512):
                        for k in range(KT):
                            nc.tensor.matmul(
                                ps[:, s * 512:(s + 1) * 512],
                                xh_sb[:, k * B + q * 128: k * B + (q + 1) * 128],
                                eh_sb[:, k * cw + s * 512: k * cw + s * 512 + 512],
                                start=(k == 0), stop=(k == KT - 1))
                    nwin = cw // WWIN
                    wslot = q * WPC + c0 // WWIN
                    nc.vector.tensor_reduce(
                        wmax_sb[:, wslot:wslot + nwin],
                        ps[:, :cw].rearrange("p (w i) -> p w i", i=WWIN),
                        axis=X, op=Max)
                # half A (windows [0, HALF_A)) is complete after chunk 6;
                # spread its per-q selection over chunks 6..12 (2-3 q each)
                if ci >= 6:
                    n_grp = len(chunks) - 6
                    qs = [q for q in range(QT) if q % n_grp == ci - 6]
                    for q in qs:
                        select(wmax_sb[:, q * WPC:q * WPC + HALF_A],
                               vout_sb, iout_sb, q * 2 * NSEL, HALF_A, mr_pool)

            for q in range(QT):  # half B (windows [HALF_A, WPC))
                select(wmax_sb[:, q * WPC + HALF_A:(q + 1) * WPC],
                       vout_sb, iout_sb, q * 2 * NSEL + NSEL, HALF_B, mr_pool)

            for q in range(QT):
                nc.sync.dma_start(d_wvals[q * 128:(q + 1) * 128, :],
                                  vout_sb[:, q * 2 * NSEL:(q + 1) * 2 * NSEL])
                nc.sync.dma_start(d_widx[q * 128:(q + 1) * 128, :],
                                  iout_sb[:, q * 2 * NSEL:(q + 1) * 2 * NSEL])

    nc.compile()
    return nc


def _build_f8w(nc):
    """Same structure as f16w, but fp8e4m3 DoubleRow matmuls: operands carry
    [partition, j(2), cols] APs; each matmul contracts 256 dims (2 k-groups
    of 128), so K=512 takes 2 matmuls per 512-wide output slice."""
    Max = mybir.AluOpType.max
    X = mybir.AxisListType.X
    F8 = mybir.dt.float8e4
    DR = mybir.MatmulPerfMode.DoubleRow

    d_x8 = nc.dram_tensor("x8", [D, B], F8, kind="ExternalInput")
    d_e8 = nc.dram_tensor("e8", [D, N_CORE], F8, kind="ExternalInput")
    d_wvals = nc.dram_tensor("wvals", [B, 2 * NSEL], F32, kind="ExternalOutput")
    d_widx = nc.dram_tensor("widx", [B, 2 * NSEL], U32, kind="ExternalOutput")

    chunks = [(i * BIGCHUNK, BIGCHUNK) for i in range(N_CORE // BIGCHUNK)]
    rem = N_CORE - (N_CORE // BIGCHUNK) * BIGCHUNK
    if rem:
        chunks.append((N_CORE - rem, rem))
    HALF_B = WPC - HALF_A

    def select(wq, vout, iout, o, width, mr_pool):
        nc.vector.max(vout[:, o:o + 8], wq)
        nc.vector.max_index(iout[:, o:o + 8], vout[:, o:o + 8], wq)
        mr = mr_pool.tile([128, width], F32, tag="mr")
        nc.vector.match_replace(mr[:, :width], vout[:, o:o + 8], wq, -1e30)
        nc.vector.max(vout[:, o + 8:o + 16], mr[:, :width])
        nc.vector.max_index(iout[:, o + 8:o + 16],
                            vout[:, o + 8:o + 16], mr[:, :width])

    with tile.TileContext(nc) as tc:
        with (
            tc.tile_pool(name="xpool", bufs=1) as xpool,
            tc.tile_pool(name="epool", bufs=3) as epool,
            tc.tile_pool(name="ps", bufs=3, space="PSUM") as ps_pool,
            tc.tile_pool(name="wacc", bufs=1) as wacc_pool,
            tc.tile_pool(name="mrp", bufs=4) as mr_pool,
            tc.tile_pool(name="outp", bufs=1) as out_pool,
        ):
            # [g][j][cols] layout: row-range g*256 + j*128 of the [D, *] input
            x_sb = xpool.tile([128, 4 * B], F8, tag="x8")
            for g in range(2):
                for j in range(2):
                    r0 = g * 256 + j * 128
                    nc.sync.dma_start(x_sb[:, (g * 2 + j) * B:(g * 2 + j + 1) * B],
                                      d_x8[r0:r0 + 128, :])

            wmax_sb = wacc_pool.tile([128, QT * WPC], F32, tag="wacc")
            vout_sb = out_pool.tile([128, QT * 2 * NSEL], F32, tag="vout")
            iout_sb = out_pool.tile([128, QT * 2 * NSEL], U32, tag="iout")

            for ci, (c0, cw) in enumerate(chunks):
                eh_sb = epool.tile([128, 4 * BIGCHUNK], F8, tag="e8")
                for g in range(2):
                    for j in range(2):
                        r0 = g * 256 + j * 128
                        nc.sync.dma_start(
                            eh_sb[:, (g * 2 + j) * cw:(g * 2 + j + 1) * cw],
                            d_e8[r0:r0 + 128, c0:c0 + cw])
                for q in range(QT):
                    ps = ps_pool.tile([128, BIGCHUNK], F32, tag="ps")
                    for s in range(cw // 512):
                        for g in range(2):
                            lhsT = x_sb[:, g * 2 * B:(g + 1) * 2 * B].rearrange(
                                "p (j b) -> p j b", j=2)[:, :, q * 128:(q + 1) * 128]
                            rhs = eh_sb[:, g * 2 * cw:(g + 1) * 2 * cw].rearrange(
                                "p (j n) -> p j n", j=2)[:, :, s * 512:(s + 1) * 512]
                            nc.tensor.matmul(ps[:, s * 512:(s + 1) * 512],
                                             lhsT, rhs, perf_mode=DR,
                                             start=(g == 0), stop=(g == 1))
                    nwin = cw // WWIN
                    wslot = q * WPC + c0 // WWIN
                    nc.vector.tensor_reduce(
                        wmax_sb[:, wslot:wslot + nwin],
                        ps[:, :cw].rearrange("p (w i) -> p w i", i=WWIN),
                        axis=X, op=Max)
                if ci >= 6:
                    n_grp = len(chunks) - 6
                    qs = [q for q in range(QT) if q % n_grp == ci - 6]
                    for q in qs:
                        select(wmax_sb[:, q * WPC:q * WPC + HALF_A],
                               vout_sb, iout_sb, q * 2 * NSEL, HALF_A, mr_pool)

            for q in range(QT):
                select(wmax_sb[:, q * WPC + HALF_A:(q + 1) * WPC],
                       vout_sb, iout_sb, q * 2 * NSEL + NSEL, HALF_B, mr_pool)

            for q in range(QT):
                nc.sync.dma_start(d_wvals[q * 128:(q + 1) * 128, :],
                                  vout_sb[:, q * 2 * NSEL:(q + 1) * 2 * NSEL])
                nc.sync.dma_start(d_widx[q * 128:(q + 1) * 128, :],
                                  iout_sb[:, q * 2 * NSEL:(q + 1) * 2 * NSEL])

    nc.compile()
    return nc


_F8_LUT = None


def _to_f8(a):
    """Fast float->fp8e4m3: fp16 hardware cast, then a 64K-entry LUT over the
    fp16 bit patterns (ml_dtypes' elementwise astype is ~50x slower). The
    double rounding vs a direct fp32->fp8 cast is harmless here: any
    consistent rounding is covered by the selection margin."""
    global _F8_LUT
    import ml_dtypes
    if _F8_LUT is None:
        with np.errstate(all="ignore"):
            all16 = np.arange(65536, dtype=np.uint16).view(np.float16)
            _F8_LUT = (all16.astype(np.float32)
                       .astype(ml_dtypes.float8_e4m3).view(np.uint8))
    h = a.astype(np.float16).view(np.uint16)
    return _F8_LUT[h].view(ml_dtypes.float8_e4m3)


def _build_f8d(nc):
    """f8w minus on-device window selection: the full per-window max array
    ships to the host (3.3MB/core), which does the margin selection itself.
    ScalarE stages PSUM->SBUF so the DVE reduce pays the SBUF (not PSUM)
    access bubble; DVE runs nothing but the 208 window-max reduces."""
    Max = mybir.AluOpType.max
    X = mybir.AxisListType.X
    F8 = mybir.dt.float8e4
    DR = mybir.MatmulPerfMode.DoubleRow
    Copy = mybir.ActivationFunctionType.Copy

    d_x8 = nc.dram_tensor("x8", [D, B], F8, kind="ExternalInput")
    d_e8 = nc.dram_tensor("e8", [D, N_CORE], F8, kind="ExternalInput")
    d_wmax = nc.dram_tensor("wmax", [B, WPC], F32, kind="ExternalOutput")

    chunks = [(i * BIGCHUNK, BIGCHUNK) for i in range(N_CORE // BIGCHUNK)]
    rem = N_CORE - (N_CORE // BIGCHUNK) * BIGCHUNK
    if rem:
        chunks.append((N_CORE - rem, rem))

    with tile.TileContext(nc) as tc:
        with (
            tc.tile_pool(name="xpool", bufs=1) as xpool,
            tc.tile_pool(name="epool", bufs=3) as epool,
            tc.tile_pool(name="ps", bufs=3, space="PSUM") as ps_pool,
            tc.tile_pool(name="stg", bufs=3) as stg_pool,
            tc.tile_pool(name="wacc", bufs=1) as wacc_pool,
        ):
            x_sb = xpool.tile([128, 4 * B], F8, tag="x8")
            for g in range(2):
                for j in range(2):
                    r0 = g * 256 + j * 128
                    nc.sync.dma_start(x_sb[:, (g * 2 + j) * B:(g * 2 + j + 1) * B],
                                      d_x8[r0:r0 + 128, :])

            wmax_sb = wacc_pool.tile([128, QT * WPC], F32, tag="wacc")

            for (c0, cw) in chunks:
                eh_sb = epool.tile([128, 4 * BIGCHUNK], F8, tag="e8")
                for g in range(2):
                    for j in range(2):
                        r0 = g * 256 + j * 128
                        nc.sync.dma_start(
                            eh_sb[:, (g * 2 + j) * cw:(g * 2 + j + 1) * cw],
                            d_e8[r0:r0 + 128, c0:c0 + cw])
                for q in range(QT):
                    ps = ps_pool.tile([128, BIGCHUNK], F32, tag="ps")
                    for s in range(cw // 512):
                        for g in range(2):
                            lhsT = x_sb[:, g * 2 * B:(g + 1) * 2 * B].rearrange(
                                "p (j b) -> p j b", j=2)[:, :, q * 128:(q + 1) * 128]
                            rhs = eh_sb[:, g * 2 * cw:(g + 1) * 2 * cw].rearrange(
                                "p (j n) -> p j n", j=2)[:, :, s * 512:(s + 1) * 512]
                            nc.tensor.matmul(ps[:, s * 512:(s + 1) * 512],
                                             lhsT, rhs, perf_mode=DR,
                                             start=(g == 0), stop=(g == 1))
                    stg = stg_pool.tile([128, BIGCHUNK], F32, tag="stg")
                    nc.scalar.activation(stg[:, :cw], ps[:, :cw], Copy)
                    nwin = cw // WWIN
                    wslot = q * WPC + c0 // WWIN
                    nc.vector.tensor_reduce(
                        wmax_sb[:, wslot:wslot + nwin],
                        stg[:, :cw].rearrange("p (w i) -> p w i", i=WWIN),
                        axis=X, op=Max)

            for q in range(QT):
                nc.sync.dma_start(d_wmax[q * 128:(q + 1) * 128, :],
                                  wmax_sb[:, q * WPC:(q + 1) * WPC])

    nc.compile()
    return nc


def _build_f8e(nc):
    """f8d with wider DVE reduces (two staged PSUM tiles -> one 2048-wide
    window-max, halving the per-op SBUF bubble count) and per-half early
    wmax DMA-out so the output transfer overlaps the main loop."""
    Max = mybir.AluOpType.max
    X = mybir.AxisListType.X
    F8 = mybir.dt.float8e4
    DR = mybir.MatmulPerfMode.DoubleRow
    Copy = mybir.ActivationFunctionType.Copy

    d_x8 = nc.dram_tensor("x8", [D, B], F8, kind="ExternalInput")
    d_e8 = nc.dram_tensor("e8", [D, N_CORE], F8, kind="ExternalInput")
    d_wmax = nc.dram_tensor("wmax", [B, WPC], F32, kind="ExternalOutput")

    BC = 2048  # 4 PSUM banks per tile; 6x2048 + 1x512 = 12800
    chunks = [(i * BC, BC) for i in range(N_CORE // BC)]
    rem = N_CORE - (N_CORE // BC) * BC
    if rem:
        chunks.append((N_CORE - rem, rem))
    AWIN = (4 * BC) // WWIN  # 256 windows (chunks 0-3) ship mid-loop

    with tile.TileContext(nc) as tc:
        with (
            tc.tile_pool(name="xpool", bufs=1) as xpool,
            tc.tile_pool(name="epool", bufs=3) as epool,
            tc.tile_pool(name="ps", bufs=2, space="PSUM") as ps_pool,
            tc.tile_pool(name="stg", bufs=3) as stg_pool,
            tc.tile_pool(name="wacc", bufs=1) as wacc_pool,
        ):
            x_sb = xpool.tile([128, 4 * B], F8, tag="x8")
            for g in range(2):
                for j in range(2):
                    r0 = g * 256 + j * 128
                    nc.sync.dma_start(x_sb[:, (g * 2 + j) * B:(g * 2 + j + 1) * B],
                                      d_x8[r0:r0 + 128, :])

            wmax_sb = wacc_pool.tile([128, QT * WPC], F32, tag="wacc")

            for ci, (c0, cw) in enumerate(chunks):
                eh_sb = epool.tile([128, 4 * BC], F8, tag="e8")
                for g in range(2):
                    for j in range(2):
                        r0 = g * 256 + j * 128
                        nc.sync.dma_start(
                            eh_sb[:, (g * 2 + j) * cw:(g * 2 + j + 1) * cw],
                            d_e8[r0:r0 + 128, c0:c0 + cw])
                for q in range(QT):
                    ps = ps_pool.tile([128, BC], F32, tag="ps")
                    for s in range(cw // 512):
                        for g in range(2):
                            lhsT = x_sb[:, g * 2 * B:(g + 1) * 2 * B].rearrange(
                                "p (j b) -> p j b", j=2)[:, :, q * 128:(q + 1) * 128]
                            rhs = eh_sb[:, g * 2 * cw:(g + 1) * 2 * cw].rearrange(
                                "p (j n) -> p j n", j=2)[:, :, s * 512:(s + 1) * 512]
                            nc.tensor.matmul(ps[:, s * 512:(s + 1) * 512],
                                             lhsT, rhs, perf_mode=DR,
                                             start=(g == 0), stop=(g == 1))
                    stg = stg_pool.tile([128, BC], F32, tag="stg")
                    nc.scalar.activation(stg[:, :cw], ps[:, :cw], Copy)
                    nwin = cw // WWIN
                    wslot = q * WPC + c0 // WWIN
                    nc.vector.tensor_reduce(
                        wmax_sb[:, wslot:wslot + nwin],
                        stg[:, :cw].rearrange("p (w i) -> p w i", i=WWIN),
                        axis=X, op=Max)
                    if ci == 3:  # chunks 0-3 reduced for q: ship 256 windows
                        nc.sync.dma_start(
                            d_wmax[q * 128:(q + 1) * 128, :AWIN],
                            wmax_sb[:, q * WPC:q * WPC + AWIN])

            for q in range(QT):
                nc.sync.dma_start(d_wmax[q * 128:(q + 1) * 128, AWIN:],
                                  wmax_sb[:, q * WPC + AWIN:(q + 1) * WPC])

    nc.compile()
    return nc


def _prep_f8w(xn, e, inv):
    """in_maps for the f8w variant: fp8e4m3 transposed normalized shards,
    scaled by F8_SCALE to stay clear of the fp8 subnormal range."""
    import ml_dtypes
    f8 = ml_dtypes.float8_e4m3
    x8 = _to_f8(np.ascontiguousarray(xn.T) * np.float32(F8_SCALE))
    in_maps = []
    for i in range(CORES):
        lo_r, hi_r = i * N_CORE, (i + 1) * N_CORE
        n_real = max(0, min(hi_r, N_EMB) - lo_r)
        e8 = np.zeros((D, N_CORE), dtype=f8)
        if n_real > 0:
            sl = e[lo_r:lo_r + n_real] * (inv[lo_r:lo_r + n_real]
                                          * np.float32(F8_SCALE))[:, None]
            e8[:, :n_real] = _to_f8(sl.T)
        in_maps.append({"x8": x8, "e8": e8})
    return in_maps


def _get_nc(variant=None):
    variant = variant or MM_DTYPE
    if variant not in _CACHE:
        _CACHE[variant] = _build(variant)
    return _CACHE[variant]


def _normalize(x, embeddings):
    x = np.asarray(x, dtype=np.float32)
    e = np.asarray(embeddings, dtype=np.float32)
    xn = x / np.maximum(np.linalg.norm(x, axis=1, keepdims=True), EPS)
    inv = (1.0 / np.maximum(np.linalg.norm(e, axis=1), EPS)).astype(np.float32)
    return xn, e, inv


def _prep_f16w(xn, e, inv):
    """in_maps for the f16w variant: fp16 transposed normalized shards."""
    xh = np.ascontiguousarray(xn.T).astype(np.float16)
    in_maps = []
    for i in range(CORES):
        lo_r, hi_r = i * N_CORE, (i + 1) * N_CORE
        n_real = max(0, min(hi_r, N_EMB) - lo_r)
        eh = np.zeros((D, N_CORE), dtype=np.float16)
        if n_real > 0:
            sl = e[lo_r:lo_r + n_real] * inv[lo_r:lo_r + n_real][:, None]
            eh[:, :n_real] = sl.T.astype(np.float16)
        in_maps.append({"xh": xh, "eh": eh})
    return in_maps


def _prep_inputs(x, embeddings, variant):
    """Host prep: normalize embeddings, pad, transpose, shard; returns in_maps.

    Works per-core-shard to keep intermediates cache-sized."""
    if variant == "f16w":
        xn, e, inv = _normalize(x, embeddings)
        return _prep_f16w(xn, e, inv)
    if variant in ("f8w", "f8d", "f8e"):
        xn, e, inv = _normalize(x, embeddings)
        return _prep_f8w(xn, e, inv)
    x = np.asarray(x, dtype=np.float32)
    e = np.asarray(embeddings, dtype=np.float32)
    inv = (1.0 / np.maximum(np.linalg.norm(e, axis=1), EPS)).astype(np.float32)
    xt = np.ascontiguousarray(x.T)               # [D, B]

    in_maps = []
    for i in range(CORES):
        lo_r, hi_r = i * N_CORE, (i + 1) * N_CORE
        n_real = max(0, min(hi_r, N_EMB) - lo_r)
        ent = np.zeros((D, N_CORE), dtype=np.float32)
        if n_real > 0:
            sl = e[lo_r:lo_r + n_real]
            ent[:, :n_real] = sl.T * inv[lo_r:lo_r + n_real][None, :]
        if variant == "f16x3":
            ehi = ent.astype(np.float16)
            elo = (ent - ehi).astype(np.float16)
            in_maps.append({"ehi": ehi, "elo": elo})
        else:
            in_maps.append({"ent": ent})

    if variant == "f16x3":
        xhi = xt.astype(np.float16)
        xlo = (xt - xhi).astype(np.float16)
        for m in in_maps:
            m["xhi"] = xhi
            m["xlo"] = xlo
    else:
        for m in in_maps:
            m["xt"] = xt
    return in_maps


def _merge(results, labels):
    """Host merge: exact global top-10 from per-core per-chunk top-8 pools,
    then the reference's mode computation."""
    vals = np.concatenate([r["vals"] for r in results], axis=1)   # [B, 8*NOUT]
    idx8 = np.concatenate([r["idx"] for r in results], axis=1).astype(np.int64)

    col_base = (np.arange(NOUT, dtype=np.int64) // 8) * CHUNK      # chunk offset
    core_base = np.repeat(np.arange(CORES, dtype=np.int64) * N_CORE, NOUT)
    g = idx8 + np.tile(col_base, CORES)[None, :] + core_base[None, :]

    # padding rows (g >= N_EMB) are zero embeddings: exclude
    u = vals.view(np.uint32)
    key = np.where(u & 0x80000000, ~u, u | 0x80000000).astype(np.uint64)
    combo = ((np.uint64(0xFFFFFFFF) - key) << np.uint64(17)) | g.astype(np.uint64)
    combo[g >= N_EMB] = np.uint64(0xFFFFFFFFFFFFFFFF)
    order = np.argsort(combo, axis=1, kind="stable")[:, :K_NEIGH]
    neighbors = np.take_along_axis(g, order, axis=1)               # [B, 10]

    labels = np.asarray(labels)
    nl = labels[neighbors].astype(np.int64)                        # [B, 10]
    eq = nl[:, :, None] == nl[:, None, :]
    counts = eq.sum(-1)
    mkey = counts * (NUM_CLASSES + 1) + (NUM_CLASSES - nl)
    mi = np.argmax(mkey, axis=1)
    pred = np.take_along_axis(nl, mi[:, None], axis=1)[:, 0]
    return pred.astype(labels.dtype)


class _Runner:
    """Caches the shard_map-jitted executable across calls (mirrors
    bass2jax.run_bass_via_pjrt's multi-core path, which re-traces per call)."""

    def __init__(self, variant):
        import jax
        import concourse.mybir as mb
        from concourse import bass2jax
        from jax.experimental.shard_map import shard_map
        from jax.sharding import Mesh, PartitionSpec

        bass2jax.install_neuronx_cc_hook()
        self.jax = jax
        nc = _get_nc(variant)
        partition_name = (nc.partition_id_tensor.name
                          if nc.partition_id_tensor else None)
        in_names, out_names, out_avals, zeros = [], [], [], []
        for alloc in nc.m.functions[0].allocations:
            if not isinstance(alloc, mb.MemoryLocationSet):
                continue
            name = alloc.memorylocations[0].name
            if alloc.kind == "ExternalInput":
                if name != partition_name:
                    in_names.append(name)
            elif alloc.kind == "ExternalOutput":
                shape = tuple(alloc.tensor_shape)
                dtype = mb.dt.np(alloc.dtype)
                out_avals.append(jax.core.ShapedArray(shape, dtype))
                out_names.append(name)
                zeros.append(np.zeros((CORES * shape[0],) + shape[1:], dtype))
        self.in_names = list(in_names)
        self.out_names = out_names
        self.out_avals = out_avals
        self.zeros = zeros
        n_params = len(in_names)
        all_names = in_names + out_names
        if partition_name is not None:
            all_names = all_names + [partition_name]
        donate = tuple(range(n_params, n_params + len(out_names)))

        def _body(*args):
            operands = list(args)
            if partition_name is not None:
                operands.append(bass2jax.partition_id_tensor())
            outs = bass2jax._bass_exec_p.bind(
                *operands,
                out_avals=tuple(out_avals),
                in_names=tuple(all_names),
                out_names=tuple(out_names),
                lowering_input_output_aliases=(),
                sim_require_finite=True,
                sim_require_nnan=True,
                nc=nc,
            )
            return tuple(outs)

        devices = jax.devices()[:CORES]
        self.mesh = Mesh(np.asarray(devices), ("core",))
        self.pspec = PartitionSpec("core")
        in_specs = (self.pspec,) * (n_params + len(out_names))
        out_specs = (self.pspec,) * len(out_names)
        self.sharded = jax.jit(
            shard_map(_body, mesh=self.mesh, in_specs=in_specs,
                      out_specs=out_specs, check_rep=False),
            donate_argnums=donate, keep_unused=True,
        )

    def concat_inputs(self, in_maps):
        return [
            np.concatenate([np.asarray(m[name]) for m in in_maps], axis=0)
            for name in self.in_names
        ]

    def device_put(self, concat_in):
        from jax.sharding import NamedSharding
        sh = NamedSharding(self.mesh, self.pspec)
        return [self.jax.device_put(a, sh) for a in concat_in]

    def execute(self, concat_in):
        zeros = [np.zeros_like(z) for z in self.zeros]
        out_arrs = self.sharded(*concat_in, *zeros)
        return out_arrs

    def run(self, in_maps):
        out_arrs = self.execute(self.concat_inputs(in_maps))
        return [
            {
                name: np.asarray(out_arrs[i]).reshape(
                    CORES, *self.out_avals[i].shape)[c]
                for i, name in enumerate(self.out_names)
            }
            for c in range(CORES)
        ]


_RUNNERS = {}


def _get_runner(variant=None):
    variant = variant or MM_DTYPE
    if variant not in _RUNNERS:
        _RUNNERS[variant] = _Runner(variant)
    return _RUNNERS[variant]


def _mode_pred(neighbors, labels):
    """Reference's torch.mode semantics on gathered neighbor labels."""
    labels = np.asarray(labels)
    nl = labels[neighbors].astype(np.int64)                        # [B, 10]
    eq = nl[:, :, None] == nl[:, None, :]
    counts = eq.sum(-1)
    mkey = counts * (NUM_CLASSES + 1) + (NUM_CLASSES - nl)
    mi = np.argmax(mkey, axis=1)
    pred = np.take_along_axis(nl, mi[:, None], axis=1)[:, 0]
    return pred.astype(labels.dtype)


def _merge_f16w(results, labels, xn, e, inv, margin=MARGIN):
    """Select windows >= (10th-best window max) - margin, rescore those
    candidates exactly in fp64, exact global top-10, then mode."""
    wv = np.stack([r["wvals"] for r in results], axis=1)      # [B, 8, 32]
    wi = np.stack([r["widx"] for r in results], axis=1).astype(np.int64)
    wi[:, :, NSEL:] += HALF_A   # half-B indices are relative to its slice
    gw = wi + (np.arange(CORES, dtype=np.int64) * WPC)[None, :, None]
    wv = wv.reshape(B, CORES * 2 * NSEL)
    gw = gw.reshape(B, CORES * 2 * NSEL)

    w10 = np.partition(wv, wv.shape[1] - K_NEIGH, axis=1)[:, wv.shape[1] - K_NEIGH]
    keep = wv >= (w10[:, None] - margin)
    smax = int(keep.sum(axis=1).max())

    # top-smax windows per row by value; mask out ones below the cutoff
    order = np.argsort(-wv, axis=1, kind="stable")[:, :smax]
    sel_g = np.take_along_axis(gw, order, axis=1)              # [B, smax]
    sel_keep = np.take_along_axis(keep, order, axis=1)

    # rescore grouped by window: each window's embeddings are one contiguous
    # 32-row slice, shared by every query that selected it (~6400 windows
    # total vs ~170k (row, window) pairs -> tiny gathers, BLAS-sized GEMMs)
    e = np.asarray(e, dtype=np.float32)
    xn32 = np.ascontiguousarray(xn, dtype=np.float32)
    rows_idx, slots = np.nonzero(sel_keep)
    wins = sel_g[rows_idx, slots]
    order = np.argsort(wins, kind="stable")
    rows_idx, slots, wins = rows_idx[order], slots[order], wins[order]
    uniq, starts = np.unique(wins, return_index=True)
    bounds = np.append(starts, len(wins))

    sims = np.full((B, smax, WWIN), -np.inf, dtype=np.float32)
    for ui in range(len(uniq)):
        w = int(uniq[ui])
        c0, c1 = w * WWIN, min(w * WWIN + WWIN, N_EMB)
        if c1 <= c0:
            continue
        s0, s1 = bounds[ui], bounds[ui + 1]
        en_w = e[c0:c1] * inv[c0:c1][:, None]                  # [<=32, D]
        sblk = xn32[rows_idx[s0:s1]] @ en_w.T                  # [nrows, <=32]
        sims[rows_idx[s0:s1], slots[s0:s1], :c1 - c0] = sblk

    cand = (sel_g[:, :, None] * WWIN +
            np.arange(WWIN, dtype=np.int64)[None, None, :]).reshape(B, -1)
    sims = sims.reshape(B, -1)

    # exact top-10 by (-sim, cand) via an order-preserving uint64 key
    u = sims.view(np.uint32)
    mono = np.where(u & 0x80000000, ~u, u | 0x80000000).astype(np.uint64)
    combo = ((np.uint64(0xFFFFFFFF) - mono) << np.uint64(17)) | \
        cand.astype(np.uint64)
    combo[sims == -np.inf] = np.uint64(0xFFFFFFFFFFFFFFFF)
    ordr = np.argsort(combo, axis=1, kind="stable")[:, :K_NEIGH]
    neighbors = np.take_along_axis(cand, ordr, axis=1)
    return _mode_pred(neighbors, labels)


def _merge_f8d(results, labels, xn, e, inv, margin):
    """Host-side window selection from the full per-window-max arrays, then
    the window-grouped exact rescore."""
    wv = np.concatenate([r["wmax"] for r in results], axis=1)   # [B, 8*WPC]
    nw = wv.shape[1]
    w10 = np.partition(wv, nw - K_NEIGH, axis=1)[:, nw - K_NEIGH]
    keep = wv >= (w10[:, None] - margin)                        # [B, 8*WPC]

    rows_idx, wins = np.nonzero(keep)        # wins are global window ids
    slots = (np.cumsum(keep, axis=1) - 1)[rows_idx, wins]
    smax = int(keep.sum(axis=1).max())

    e = np.asarray(e, dtype=np.float32)
    xn32 = np.ascontiguousarray(xn, dtype=np.float32)
    order = np.argsort(wins, kind="stable")
    rows_s, slots_s, wins_s = rows_idx[order], slots[order], wins[order]
    uniq, starts = np.unique(wins_s, return_index=True)
    bounds = np.append(starts, len(wins_s))

    sims = np.full((B, smax, WWIN), -np.inf, dtype=np.float32)
    wfull = np.zeros((B, smax), dtype=np.int64)
    wfull[rows_idx, slots] = wins
    for ui in range(len(uniq)):
        w = int(uniq[ui])
        c0, c1 = w * WWIN, min(w * WWIN + WWIN, N_EMB)
        if c1 <= c0:
            continue
        s0, s1 = bounds[ui], bounds[ui + 1]
        en_w = e[c0:c1] * inv[c0:c1][:, None]
        sblk = xn32[rows_s[s0:s1]] @ en_w.T
        sims[rows_s[s0:s1], slots_s[s0:s1], :c1 - c0] = sblk

    cand = (wfull[:, :, None] * WWIN +
            np.arange(WWIN, dtype=np.int64)[None, None, :]).reshape(B, -1)
    sims = sims.reshape(B, -1)
    u = sims.view(np.uint32)
    mono = np.where(u & 0x80000000, ~u, u | 0x80000000).astype(np.uint64)
    combo = ((np.uint64(0xFFFFFFFF) - mono) << np.uint64(17)) | \
        cand.astype(np.uint64)
    combo[sims == -np.inf] = np.uint64(0xFFFFFFFFFFFFFFFF)
    ordr = np.argsort(combo, axis=1, kind="stable")[:, :K_NEIGH]
    neighbors = np.take_along_axis(cand, ordr, axis=1)
    return _mode_pred(neighbors, labels)


def run_on_hw(x, embeddings, variant=None):
    runner = _get_runner(variant)
    in_maps = _prep_inputs(x, embeddings, variant or MM_DTYPE)
    return runner.run(in_maps)


def kernel(x, embeddings, labels):
    variant = MM_DTYPE
    if variant == "f16w":
        xn, e, inv = _normalize(x, embeddings)
        runner = _get_runner(variant)
        results = runner.run(_prep_f16w(xn, e, inv))
        return _merge_f16w(results, labels, xn, e, inv)
    if variant == "f8w":
        xn, e, inv = _normalize(x, embeddings)
        runner = _get_runner(variant)
        results = runner.run(_prep_f8w(xn, e, inv))
        return _merge_f16w(results, labels, xn, e, inv,
                           margin=MARGIN_F8 * F8_SCALE * F8_SCALE)
    if variant in ("f8d", "f8e"):
        xn, e, inv = _normalize(x, embeddings)
        runner = _get_runner(variant)
        results = runner.run(_prep_f8w(xn, e, inv))
        return _merge_f8d(results, labels, xn, e, inv,
                          margin=MARGIN_F8 * F8_SCALE * F8_SCALE)
    results = run_on_hw(x, embeddings)
    return _merge(results, labels)



# revision 20
# speedup vs baseline: 1.6072x; 1.6072x over previous
"""Trainium2 Bass kernel for BaselineKNNModel (cosine-sim KNN classifier).

Contract: kernel(**inputs) takes FULL inputs (x [2048,512] f32,
embeddings [100000,512] f32, labels [100000] int) and returns the FULL
output (pred [2048] labels.dtype), distributing work across 8 NeuronCores.

Strategy (database-parallel, per sharding hint):
 - Host: normalize embeddings (cosine denominator), pad N 100000->102400,
   transpose to [512, N]; shard along N across 8 cores (12800 each).
   x normalization is skipped: per-query positive scaling cannot change
   that query's top-k ranking.
 - Device (SPMD, per core): sim tile [128 q, 512 c] = xT.T @ enT chunk via
   PE accumulation over K=512; per tile, VectorE max/max_index extract the
   top-8 values + indices of each 512-candidate chunk (global top-10 of a
   row is contained in the union of its per-chunk top-8s unless >=9 of the
   top-10 fall in one 512-chunk: P ~ 1e-11).
 - Host: merge 8 cores x 25 chunks x top-8 = 1600 candidates/query, exact
   top-10 by (value desc, index asc) = jax.lax.top_k tie order, then the
   reference's mode computation.
"""
import sys

for _p in ("/opt/trn_rl_repo", "/root/.axon_site/_ro/trn_rl_repo"):
    if _p not in sys.path:
        sys.path.insert(0, _p)

import numpy as np

import concourse.bacc as bacc
import concourse.mybir as mybir
import concourse.tile as tile
from concourse import bass_utils

F32 = mybir.dt.float32
F32R = mybir.dt.float32r
F16 = mybir.dt.float16
U32 = mybir.dt.uint32
Copy = mybir.ActivationFunctionType.Copy

B = 2048            # queries
D = 512             # embedding dim
N_EMB = 100000      # database size
K_NEIGH = 10
NUM_CLASSES = 1000
EPS = 1e-8

CORES = 8
N_PAD = 102400      # padded database size (8 * 12800)
N_CORE = N_PAD // CORES     # 12800 candidates per core
CHUNK = 512                 # candidates per sim tile (one PSUM bank)
NCHUNK = N_CORE // CHUNK    # 25
QT = B // 128               # 16 query tiles
KT = D // 128               # 4 k-tiles
NOUT = NCHUNK * 8           # 200 output slots per query per core

# f16w variant: window-max + device window top-16 + host exact rescore
WWIN = 32                   # candidates per window
WPC = N_CORE // WWIN        # 400 windows per core
BIGCHUNK = 1024             # candidates per PSUM tile (2 banks)
NSEL = 16                   # windows kept per (query, core, half)
HALF_A = (7 * BIGCHUNK) // WWIN  # windows in selection half A (224)
MARGIN = 4e-3               # fp16-sim error margin on unit-normalized sims
                            # (measured max |fp16 sim err| ~6e-5, ~60x safety)

# f8w variant: same as f16w but fp8e4m3 DoubleRow matmuls (2 fp8 weights per
# PE cell, K=256 per matmul). Inputs are scaled by F8_SCALE before rounding
# to fp8, so device sims (and window maxes) are scaled by F8_SCALE^2.
F8_SCALE = 16.0
MARGIN_F8 = 2.5e-2          # fp8 margin on unit-normalized sims
                            # (measured max err 7.1e-3 on a sample, rms 1.6e-3)

MM_DTYPE = "f8t"   # "f32" | "f32r" | "f16x3" | "f16w" | "f8w" | "f8d" | "f8e" | "f8t"

_CACHE = {}


def _build(variant):
    """Build + compile the per-core Bass program. Same program on all cores;
    only the `ent*` input shards differ."""
    nc = bacc.Bacc("TRN2", target_bir_lowering=False, debug=False)

    if variant == "noop":  # minimal program for RPC-overhead baselining
        d_nin = nc.dram_tensor("nin", [128, 128], F32, kind="ExternalInput")
        d_nout = nc.dram_tensor("nout", [128, 128], F32, kind="ExternalOutput")
        with tile.TileContext(nc) as tc:
            with tc.tile_pool(name="np0", bufs=1) as pool:
                t = pool.tile([128, 128], F32, tag="t")
                nc.sync.dma_start(t[:, :], d_nin[:, :])
                nc.sync.dma_start(d_nout[:, :], t[:, :])
        nc.compile()
        return nc

    if variant == "f16w":
        return _build_f16w(nc)
    if variant == "f8w":
        return _build_f8w(nc)
    if variant == "f8d":
        return _build_f8d(nc)
    if variant == "f8e":
        return _build_f8e(nc)
    if variant == "f8t":
        return _build_f8t(nc)
    if variant == "f8v":
        return _build_f8v(nc)
    if variant == "f8m":
        return _build_f8m(nc)

    f16 = variant == "f16x3"
    if f16:
        d_xhi = nc.dram_tensor("xhi", [D, B], F16, kind="ExternalInput")
        d_xlo = nc.dram_tensor("xlo", [D, B], F16, kind="ExternalInput")
        d_ehi = nc.dram_tensor("ehi", [D, N_CORE], F16, kind="ExternalInput")
        d_elo = nc.dram_tensor("elo", [D, N_CORE], F16, kind="ExternalInput")
    else:
        in_dt = F32R if variant == "f32r" else F32
        d_xt = nc.dram_tensor("xt", [D, B], in_dt, kind="ExternalInput")
        d_ent = nc.dram_tensor("ent", [D, N_CORE], in_dt, kind="ExternalInput")

    d_vals = nc.dram_tensor("vals", [B, NOUT], F32, kind="ExternalOutput")
    d_idx = nc.dram_tensor("idx", [B, NOUT], U32, kind="ExternalOutput")

    with tile.TileContext(nc) as tc:
        with (
            tc.tile_pool(name="xpool", bufs=1) as xpool,
            tc.tile_pool(name="epool", bufs=3) as epool,
            tc.tile_pool(name="ps", bufs=6, space="PSUM") as ps_pool,
            tc.tile_pool(name="sim", bufs=6) as sim_pool,
            tc.tile_pool(name="acc", bufs=1) as acc_pool,
        ):
            # resident x (stationary operand), k-tiles side by side
            if f16:
                xhi_sb = xpool.tile([128, KT * B], F16, tag="xhi")
                xlo_sb = xpool.tile([128, KT * B], F16, tag="xlo")
                for k in range(KT):
                    nc.sync.dma_start(xhi_sb[:, k * B:(k + 1) * B],
                                      d_xhi[k * 128:(k + 1) * 128, :])
                    nc.sync.dma_start(xlo_sb[:, k * B:(k + 1) * B],
                                      d_xlo[k * 128:(k + 1) * 128, :])
            else:
                xt_sb = xpool.tile([128, KT * B], in_dt, tag="xt")
                for k in range(KT):
                    nc.sync.dma_start(xt_sb[:, k * B:(k + 1) * B],
                                      d_xt[k * 128:(k + 1) * 128, :])

            # result accumulators, [128, QT*NOUT], column q*NOUT + c*8 + j
            vals_sb = acc_pool.tile([128, QT * NOUT], F32, tag="vacc")
            idx_sb = acc_pool.tile([128, QT * NOUT], U32, tag="iacc")

            for c in range(NCHUNK):
                c0 = c * CHUNK
                if f16:
                    ehi_sb = epool.tile([128, KT * CHUNK], F16, tag="ehi")
                    elo_sb = epool.tile([128, KT * CHUNK], F16, tag="elo")
                    for k in range(KT):
                        nc.sync.dma_start(ehi_sb[:, k * CHUNK:(k + 1) * CHUNK],
                                          d_ehi[k * 128:(k + 1) * 128, c0:c0 + CHUNK])
                        nc.sync.dma_start(elo_sb[:, k * CHUNK:(k + 1) * CHUNK],
                                          d_elo[k * 128:(k + 1) * 128, c0:c0 + CHUNK])
                else:
                    en_sb = epool.tile([128, KT * CHUNK], in_dt, tag="en")
                    for k in range(KT):
                        nc.sync.dma_start(en_sb[:, k * CHUNK:(k + 1) * CHUNK],
                                          d_ent[k * 128:(k + 1) * 128, c0:c0 + CHUNK])

                for q in range(QT):
                    ps = ps_pool.tile([128, CHUNK], F32, tag="ps")
                    if variant == "f16x3":
                        nmm = 3 * KT
                        i = 0
                        for k in range(KT):
                            xh = xhi_sb[:, k * B + q * 128: k * B + (q + 1) * 128]
                            xl = xlo_sb[:, k * B + q * 128: k * B + (q + 1) * 128]
                            eh = ehi_sb[:, k * CHUNK:(k + 1) * CHUNK]
                            el = elo_sb[:, k * CHUNK:(k + 1) * CHUNK]
                            for (a, bb) in ((xh, eh), (xh, el), (xl, eh)):
                                nc.tensor.matmul(ps[:, :], a, bb,
                                                 start=(i == 0), stop=(i == nmm - 1))
                                i += 1
                    else:
                        for k in range(KT):
                            lhsT = xt_sb[:, k * B + q * 128: k * B + (q + 1) * 128]
                            rhs = en_sb[:, k * CHUNK:(k + 1) * CHUNK]
                            nc.tensor.matmul(ps[:, :], lhsT, rhs,
                                             start=(k == 0), stop=(k == KT - 1))

                    sim = sim_pool.tile([128, CHUNK], F32, tag="sim")
                    nc.scalar.activation(sim[:, :], ps[:, :], Copy)

                    o = q * NOUT + c * 8
                    nc.vector.max(vals_sb[:, o:o + 8], sim[:, :])
                    nc.vector.max_index(idx_sb[:, o:o + 8], vals_sb[:, o:o + 8],
                                        sim[:, :])

            for q in range(QT):
                nc.sync.dma_start(d_vals[q * 128:(q + 1) * 128, :],
                                  vals_sb[:, q * NOUT:(q + 1) * NOUT])
                nc.sync.dma_start(d_idx[q * 128:(q + 1) * 128, :],
                                  idx_sb[:, q * NOUT:(q + 1) * NOUT])

    nc.compile()
    return nc


def _build_f16w(nc):
    """fp16 single-pass matmul; per-tile 16-wide window max (DVE reduce,
    PSUM-direct); per-core-half top-16 windows per query via
    max/match_replace (first half's selection overlaps the main loop);
    host rescores the selected windows exactly."""
    Max = mybir.AluOpType.max
    X = mybir.AxisListType.X

    d_xh = nc.dram_tensor("xh", [D, B], F16, kind="ExternalInput")
    d_eh = nc.dram_tensor("eh", [D, N_CORE], F16, kind="ExternalInput")
    d_wvals = nc.dram_tensor("wvals", [B, 2 * NSEL], F32, kind="ExternalOutput")
    d_widx = nc.dram_tensor("widx", [B, 2 * NSEL], U32, kind="ExternalOutput")

    # chunk layout: 12 x 1024 + 1 x 512 = 12800
    chunks = [(i * BIGCHUNK, BIGCHUNK) for i in range(N_CORE // BIGCHUNK)]
    rem = N_CORE - (N_CORE // BIGCHUNK) * BIGCHUNK
    if rem:
        chunks.append((N_CORE - rem, rem))
    # selection halves aligned to chunk boundaries:
    # half A = chunks 0-6 (448 windows), half B = chunks 7-12 (352 windows)
    HALF_B = WPC - HALF_A

    def select(wq, vout, iout, o, width, mr_pool):
        nc.vector.max(vout[:, o:o + 8], wq)
        nc.vector.max_index(iout[:, o:o + 8], vout[:, o:o + 8], wq)
        mr = mr_pool.tile([128, width], F32, tag="mr")
        nc.vector.match_replace(mr[:, :width], vout[:, o:o + 8], wq, -1e30)
        nc.vector.max(vout[:, o + 8:o + 16], mr[:, :width])
        nc.vector.max_index(iout[:, o + 8:o + 16],
                            vout[:, o + 8:o + 16], mr[:, :width])

    with tile.TileContext(nc) as tc:
        with (
            tc.tile_pool(name="xpool", bufs=1) as xpool,
            tc.tile_pool(name="epool", bufs=3) as epool,
            tc.tile_pool(name="ps", bufs=3, space="PSUM") as ps_pool,
            tc.tile_pool(name="wacc", bufs=1) as wacc_pool,
            tc.tile_pool(name="mrp", bufs=4) as mr_pool,
            tc.tile_pool(name="outp", bufs=1) as out_pool,
        ):
            xh_sb = xpool.tile([128, KT * B], F16, tag="xh")
            for k in range(KT):
                nc.sync.dma_start(xh_sb[:, k * B:(k + 1) * B],
                                  d_xh[k * 128:(k + 1) * 128, :])

            wmax_sb = wacc_pool.tile([128, QT * WPC], F32, tag="wacc")
            vout_sb = out_pool.tile([128, QT * 2 * NSEL], F32, tag="vout")
            iout_sb = out_pool.tile([128, QT * 2 * NSEL], U32, tag="iout")

            for ci, (c0, cw) in enumerate(chunks):
                eh_sb = epool.tile([128, KT * BIGCHUNK], F16, tag="eh")
                for k in range(KT):
                    nc.sync.dma_start(eh_sb[:, k * cw:(k + 1) * cw],
                                      d_eh[k * 128:(k + 1) * 128, c0:c0 + cw])
                for q in range(QT):
                    ps = ps_pool.tile([128, BIGCHUNK], F32, tag="ps")
                    for s in range(cw // 512):
                        for k in range(KT):
                            nc.tensor.matmul(
                                ps[:, s * 512:(s + 1) * 512],
                                xh_sb[:, k * B + q * 128: k * B + (q + 1) * 128],
                                eh_sb[:, k * cw + s * 512: k * cw + s * 512 + 512],
                                start=(k == 0), stop=(k == KT - 1))
                    nwin = cw // WWIN
                    wslot = q * WPC + c0 // WWIN
                    nc.vector.tensor_reduce(
                        wmax_sb[:, wslot:wslot + nwin],
                        ps[:, :cw].rearrange("p (w i) -> p w i", i=WWIN),
                        axis=X, op=Max)
                # half A (windows [0, HALF_A)) is complete after chunk 6;
                # spread its per-q selection over chunks 6..12 (2-3 q each)
                if ci >= 6:
                    n_grp = len(chunks) - 6
                    qs = [q for q in range(QT) if q % n_grp == ci - 6]
                    for q in qs:
                        select(wmax_sb[:, q * WPC:q * WPC + HALF_A],
                               vout_sb, iout_sb, q * 2 * NSEL, HALF_A, mr_pool)

            for q in range(QT):  # half B (windows [HALF_A, WPC))
                select(wmax_sb[:, q * WPC + HALF_A:(q + 1) * WPC],
                       vout_sb, iout_sb, q * 2 * NSEL + NSEL, HALF_B, mr_pool)

            for q in range(QT):
                nc.sync.dma_start(d_wvals[q * 128:(q + 1) * 128, :],
                                  vout_sb[:, q * 2 * NSEL:(q + 1) * 2 * NSEL])
                nc.sync.dma_start(d_widx[q * 128:(q + 1) * 128, :],
                                  iout_sb[:, q * 2 * NSEL:(q + 1) * 2 * NSEL])

    nc.compile()
    return nc


def _build_f8w(nc):
    """Same structure as f16w, but fp8e4m3 DoubleRow matmuls: operands carry
    [partition, j(2), cols] APs; each matmul contracts 256 dims (2 k-groups
    of 128), so K=512 takes 2 matmuls per 512-wide output slice."""
    Max = mybir.AluOpType.max
    X = mybir.AxisListType.X
    F8 = mybir.dt.float8e4
    DR = mybir.MatmulPerfMode.DoubleRow

    d_x8 = nc.dram_tensor("x8", [D, B], F8, kind="ExternalInput")
    d_e8 = nc.dram_tensor("e8", [D, N_CORE], F8, kind="ExternalInput")
    d_wvals = nc.dram_tensor("wvals", [B, 2 * NSEL], F32, kind="ExternalOutput")
    d_widx = nc.dram_tensor("widx", [B, 2 * NSEL], U32, kind="ExternalOutput")

    chunks = [(i * BIGCHUNK, BIGCHUNK) for i in range(N_CORE // BIGCHUNK)]
    rem = N_CORE - (N_CORE // BIGCHUNK) * BIGCHUNK
    if rem:
        chunks.append((N_CORE - rem, rem))
    HALF_B = WPC - HALF_A

    def select(wq, vout, iout, o, width, mr_pool):
        nc.vector.max(vout[:, o:o + 8], wq)
        nc.vector.max_index(iout[:, o:o + 8], vout[:, o:o + 8], wq)
        mr = mr_pool.tile([128, width], F32, tag="mr")
        nc.vector.match_replace(mr[:, :width], vout[:, o:o + 8], wq, -1e30)
        nc.vector.max(vout[:, o + 8:o + 16], mr[:, :width])
        nc.vector.max_index(iout[:, o + 8:o + 16],
                            vout[:, o + 8:o + 16], mr[:, :width])

    with tile.TileContext(nc) as tc:
        with (
            tc.tile_pool(name="xpool", bufs=1) as xpool,
            tc.tile_pool(name="epool", bufs=3) as epool,
            tc.tile_pool(name="ps", bufs=3, space="PSUM") as ps_pool,
            tc.tile_pool(name="wacc", bufs=1) as wacc_pool,
            tc.tile_pool(name="mrp", bufs=4) as mr_pool,
            tc.tile_pool(name="outp", bufs=1) as out_pool,
        ):
            # [g][j][cols] layout: row-range g*256 + j*128 of the [D, *] input
            x_sb = xpool.tile([128, 4 * B], F8, tag="x8")
            for g in range(2):
                for j in range(2):
                    r0 = g * 256 + j * 128
                    nc.sync.dma_start(x_sb[:, (g * 2 + j) * B:(g * 2 + j + 1) * B],
                                      d_x8[r0:r0 + 128, :])

            wmax_sb = wacc_pool.tile([128, QT * WPC], F32, tag="wacc")
            vout_sb = out_pool.tile([128, QT * 2 * NSEL], F32, tag="vout")
            iout_sb = out_pool.tile([128, QT * 2 * NSEL], U32, tag="iout")

            for ci, (c0, cw) in enumerate(chunks):
                eh_sb = epool.tile([128, 4 * BIGCHUNK], F8, tag="e8")
                for g in range(2):
                    for j in range(2):
                        r0 = g * 256 + j * 128
                        nc.sync.dma_start(
                            eh_sb[:, (g * 2 + j) * cw:(g * 2 + j + 1) * cw],
                            d_e8[r0:r0 + 128, c0:c0 + cw])
                for q in range(QT):
                    ps = ps_pool.tile([128, BIGCHUNK], F32, tag="ps")
                    for s in range(cw // 512):
                        for g in range(2):
                            lhsT = x_sb[:, g * 2 * B:(g + 1) * 2 * B].rearrange(
                                "p (j b) -> p j b", j=2)[:, :, q * 128:(q + 1) * 128]
                            rhs = eh_sb[:, g * 2 * cw:(g + 1) * 2 * cw].rearrange(
                                "p (j n) -> p j n", j=2)[:, :, s * 512:(s + 1) * 512]
                            nc.tensor.matmul(ps[:, s * 512:(s + 1) * 512],
                                             lhsT, rhs, perf_mode=DR,
                                             start=(g == 0), stop=(g == 1))
                    nwin = cw // WWIN
                    wslot = q * WPC + c0 // WWIN
                    nc.vector.tensor_reduce(
                        wmax_sb[:, wslot:wslot + nwin],
                        ps[:, :cw].rearrange("p (w i) -> p w i", i=WWIN),
                        axis=X, op=Max)
                if ci >= 6:
                    n_grp = len(chunks) - 6
                    qs = [q for q in range(QT) if q % n_grp == ci - 6]
                    for q in qs:
                        select(wmax_sb[:, q * WPC:q * WPC + HALF_A],
                               vout_sb, iout_sb, q * 2 * NSEL, HALF_A, mr_pool)

            for q in range(QT):
                select(wmax_sb[:, q * WPC + HALF_A:(q + 1) * WPC],
                       vout_sb, iout_sb, q * 2 * NSEL + NSEL, HALF_B, mr_pool)

            for q in range(QT):
                nc.sync.dma_start(d_wvals[q * 128:(q + 1) * 128, :],
                                  vout_sb[:, q * 2 * NSEL:(q + 1) * 2 * NSEL])
                nc.sync.dma_start(d_widx[q * 128:(q + 1) * 128, :],
                                  iout_sb[:, q * 2 * NSEL:(q + 1) * 2 * NSEL])

    nc.compile()
    return nc


_F8_LUT = None


def _to_f8(a):
    """Fast float->fp8e4m3: fp16 hardware cast, then a 64K-entry LUT over the
    fp16 bit patterns (ml_dtypes' elementwise astype is ~50x slower). The
    double rounding vs a direct fp32->fp8 cast is harmless here: any
    consistent rounding is covered by the selection margin."""
    global _F8_LUT
    import ml_dtypes
    if _F8_LUT is None:
        with np.errstate(all="ignore"):
            all16 = np.arange(65536, dtype=np.uint16).view(np.float16)
            _F8_LUT = (all16.astype(np.float32)
                       .astype(ml_dtypes.float8_e4m3).view(np.uint8))
    h = a.astype(np.float16).view(np.uint16)
    return _F8_LUT[h].view(ml_dtypes.float8_e4m3)


def _build_f8d(nc):
    """f8w minus on-device window selection: the full per-window max array
    ships to the host (3.3MB/core), which does the margin selection itself.
    ScalarE stages PSUM->SBUF so the DVE reduce pays the SBUF (not PSUM)
    access bubble; DVE runs nothing but the 208 window-max reduces."""
    Max = mybir.AluOpType.max
    X = mybir.AxisListType.X
    F8 = mybir.dt.float8e4
    DR = mybir.MatmulPerfMode.DoubleRow
    Copy = mybir.ActivationFunctionType.Copy

    d_x8 = nc.dram_tensor("x8", [D, B], F8, kind="ExternalInput")
    d_e8 = nc.dram_tensor("e8", [D, N_CORE], F8, kind="ExternalInput")
    d_wmax = nc.dram_tensor("wmax", [B, WPC], F32, kind="ExternalOutput")

    chunks = [(i * BIGCHUNK, BIGCHUNK) for i in range(N_CORE // BIGCHUNK)]
    rem = N_CORE - (N_CORE // BIGCHUNK) * BIGCHUNK
    if rem:
        chunks.append((N_CORE - rem, rem))

    with tile.TileContext(nc) as tc:
        with (
            tc.tile_pool(name="xpool", bufs=1) as xpool,
            tc.tile_pool(name="epool", bufs=3) as epool,
            tc.tile_pool(name="ps", bufs=3, space="PSUM") as ps_pool,
            tc.tile_pool(name="stg", bufs=3) as stg_pool,
            tc.tile_pool(name="wacc", bufs=1) as wacc_pool,
        ):
            x_sb = xpool.tile([128, 4 * B], F8, tag="x8")
            for g in range(2):
                for j in range(2):
                    r0 = g * 256 + j * 128
                    nc.sync.dma_start(x_sb[:, (g * 2 + j) * B:(g * 2 + j + 1) * B],
                                      d_x8[r0:r0 + 128, :])

            wmax_sb = wacc_pool.tile([128, QT * WPC], F32, tag="wacc")

            for (c0, cw) in chunks:
                eh_sb = epool.tile([128, 4 * BIGCHUNK], F8, tag="e8")
                for g in range(2):
                    for j in range(2):
                        r0 = g * 256 + j * 128
                        nc.sync.dma_start(
                            eh_sb[:, (g * 2 + j) * cw:(g * 2 + j + 1) * cw],
                            d_e8[r0:r0 + 128, c0:c0 + cw])
                for q in range(QT):
                    ps = ps_pool.tile([128, BIGCHUNK], F32, tag="ps")
                    for s in range(cw // 512):
                        for g in range(2):
                            lhsT = x_sb[:, g * 2 * B:(g + 1) * 2 * B].rearrange(
                                "p (j b) -> p j b", j=2)[:, :, q * 128:(q + 1) * 128]
                            rhs = eh_sb[:, g * 2 * cw:(g + 1) * 2 * cw].rearrange(
                                "p (j n) -> p j n", j=2)[:, :, s * 512:(s + 1) * 512]
                            nc.tensor.matmul(ps[:, s * 512:(s + 1) * 512],
                                             lhsT, rhs, perf_mode=DR,
                                             start=(g == 0), stop=(g == 1))
                    stg = stg_pool.tile([128, BIGCHUNK], F32, tag="stg")
                    nc.scalar.activation(stg[:, :cw], ps[:, :cw], Copy)
                    nwin = cw // WWIN
                    wslot = q * WPC + c0 // WWIN
                    nc.vector.tensor_reduce(
                        wmax_sb[:, wslot:wslot + nwin],
                        stg[:, :cw].rearrange("p (w i) -> p w i", i=WWIN),
                        axis=X, op=Max)

            for q in range(QT):
                nc.sync.dma_start(d_wmax[q * 128:(q + 1) * 128, :],
                                  wmax_sb[:, q * WPC:(q + 1) * WPC])

    nc.compile()
    return nc


def _build_f8e(nc):
    """f8d with wider DVE reduces (two staged PSUM tiles -> one 2048-wide
    window-max, halving the per-op SBUF bubble count) and per-half early
    wmax DMA-out so the output transfer overlaps the main loop."""
    Max = mybir.AluOpType.max
    X = mybir.AxisListType.X
    F8 = mybir.dt.float8e4
    DR = mybir.MatmulPerfMode.DoubleRow
    Copy = mybir.ActivationFunctionType.Copy

    d_x8 = nc.dram_tensor("x8", [D, B], F8, kind="ExternalInput")
    d_e8 = nc.dram_tensor("e8", [D, N_CORE], F8, kind="ExternalInput")
    d_wmax = nc.dram_tensor("wmax", [B, WPC], F32, kind="ExternalOutput")

    BC = 2048  # 4 PSUM banks per tile; 6x2048 + 1x512 = 12800
    chunks = [(i * BC, BC) for i in range(N_CORE // BC)]
    rem = N_CORE - (N_CORE // BC) * BC
    if rem:
        chunks.append((N_CORE - rem, rem))
    AWIN = (4 * BC) // WWIN  # 256 windows (chunks 0-3) ship mid-loop

    with tile.TileContext(nc) as tc:
        with (
            tc.tile_pool(name="xpool", bufs=1) as xpool,
            tc.tile_pool(name="epool", bufs=3) as epool,
            tc.tile_pool(name="ps", bufs=2, space="PSUM") as ps_pool,
            tc.tile_pool(name="stg", bufs=3) as stg_pool,
            tc.tile_pool(name="wacc", bufs=1) as wacc_pool,
        ):
            x_sb = xpool.tile([128, 4 * B], F8, tag="x8")
            for g in range(2):
                for j in range(2):
                    r0 = g * 256 + j * 128
                    nc.sync.dma_start(x_sb[:, (g * 2 + j) * B:(g * 2 + j + 1) * B],
                                      d_x8[r0:r0 + 128, :])

            wmax_sb = wacc_pool.tile([128, QT * WPC], F32, tag="wacc")

            for ci, (c0, cw) in enumerate(chunks):
                eh_sb = epool.tile([128, 4 * BC], F8, tag="e8")
                for g in range(2):
                    for j in range(2):
                        r0 = g * 256 + j * 128
                        nc.sync.dma_start(
                            eh_sb[:, (g * 2 + j) * cw:(g * 2 + j + 1) * cw],
                            d_e8[r0:r0 + 128, c0:c0 + cw])
                for q in range(QT):
                    ps = ps_pool.tile([128, BC], F32, tag="ps")
                    for s in range(cw // 512):
                        for g in range(2):
                            lhsT = x_sb[:, g * 2 * B:(g + 1) * 2 * B].rearrange(
                                "p (j b) -> p j b", j=2)[:, :, q * 128:(q + 1) * 128]
                            rhs = eh_sb[:, g * 2 * cw:(g + 1) * 2 * cw].rearrange(
                                "p (j n) -> p j n", j=2)[:, :, s * 512:(s + 1) * 512]
                            nc.tensor.matmul(ps[:, s * 512:(s + 1) * 512],
                                             lhsT, rhs, perf_mode=DR,
                                             start=(g == 0), stop=(g == 1))
                    stg = stg_pool.tile([128, BC], F32, tag="stg")
                    nc.scalar.activation(stg[:, :cw], ps[:, :cw], Copy)
                    nwin = cw // WWIN
                    wslot = q * WPC + c0 // WWIN
                    nc.vector.tensor_reduce(
                        wmax_sb[:, wslot:wslot + nwin],
                        stg[:, :cw].rearrange("p (w i) -> p w i", i=WWIN),
                        axis=X, op=Max)
                    if ci == 3:  # chunks 0-3 reduced for q: ship 256 windows
                        nc.sync.dma_start(
                            d_wmax[q * 128:(q + 1) * 128, :AWIN],
                            wmax_sb[:, q * WPC:q * WPC + AWIN])

            for q in range(QT):
                nc.sync.dma_start(d_wmax[q * 128:(q + 1) * 128, AWIN:],
                                  wmax_sb[:, q * WPC + AWIN:(q + 1) * WPC])

    nc.compile()
    return nc


# f8t variant: pairwise-max fold tree over the sim tile, balanced across
# DVE + Pool + ScalarE so the fp8 matmuls (PE) are the bottleneck. Window
# width 4 (chunk cols {j, j+cw/4, j+cw/2, j+3cw/4}); full per-window max
# array ships to host for margin selection + exact rescore.
FT_BC = 2048                 # main chunk width (4 PSUM banks)
FT_WD = 124                  # fold1 pairs on DVE (direct fp32 PSUM)
FT_WP = 520                  # fold1 pairs on Pool (direct fp32 PSUM)
FT_WS = FT_BC // 2 - FT_WD - FT_WP   # fold1 pairs via ScalarE fp16 staging
FT_RD = 66                   # remainder-chunk fold1 pairs on DVE
FT_NW = (N_CORE // FT_BC) * (FT_BC // 4) + (N_CORE % FT_BC) // 4  # 3200


def _build_f8t(nc):
    Max = mybir.AluOpType.max
    F8 = mybir.dt.float8e4
    DR = mybir.MatmulPerfMode.DoubleRow
    Copy = mybir.ActivationFunctionType.Copy

    # [p, slot, cols] inputs: slot k holds rows 128k..128k+127 of the
    # transposed operand; slot order == (g, j) DoubleRow order.
    d_x8 = nc.dram_tensor("x8", [128, 4, B], F8, kind="ExternalInput")
    d_e8 = nc.dram_tensor("e8", [128, 4, N_CORE], F8, kind="ExternalInput")
    # [p, q, w]: query q*128+p, window w (host transposes back)
    d_wmax = nc.dram_tensor("wmax", [128, QT, FT_NW], F16, kind="ExternalOutput")

    chunks = [(i * FT_BC, FT_BC) for i in range(N_CORE // FT_BC)]
    rem = N_CORE - (N_CORE // FT_BC) * FT_BC
    if rem:
        chunks.append((N_CORE - rem, rem))

    with tile.TileContext(nc) as tc:
        with (
            tc.tile_pool(name="xpool", bufs=1) as xpool,
            tc.tile_pool(name="epool", bufs=3) as epool,
            tc.tile_pool(name="ps", bufs=2, space="PSUM") as ps_pool,
            tc.tile_pool(name="stg", bufs=3) as stg_pool,
            tc.tile_pool(name="s1p", bufs=3) as s1_pool,
            tc.tile_pool(name="wm", bufs=2) as wm_pool,
        ):
            x_sb = xpool.tile([128, 4 * B], F8, tag="x8")
            nc.sync.dma_start(
                x_sb[:, :].rearrange("p (k b) -> p k b", k=4), d_x8[:, :, :])

            wout = 0
            for ci, (c0, cw) in enumerate(chunks):
                half = cw // 2
                quar = cw // 4
                e_sb = epool.tile([128, 4 * cw], F8, tag="e8")
                nc.sync.dma_start(
                    e_sb[:, :].rearrange("p (k n) -> p k n", k=4),
                    d_e8[:, :, c0:c0 + cw])
                wm_sb = wm_pool.tile([128, QT * quar], F16, tag="wm")
                for q in range(QT):
                    ps = ps_pool.tile([128, cw], F32, tag="ps")
                    for s in range(cw // 512):
                        for g in range(2):
                            lhsT = x_sb[:, g * 2 * B:(g + 1) * 2 * B].rearrange(
                                "p (j b) -> p j b", j=2)[:, :, q * 128:(q + 1) * 128]
                            rhs = e_sb[:, g * 2 * cw:(g + 1) * 2 * cw].rearrange(
                                "p (j n) -> p j n", j=2)[:, :, s * 512:(s + 1) * 512]
                            nc.tensor.matmul(ps[:, s * 512:(s + 1) * 512],
                                             lhsT, rhs, perf_mode=DR,
                                             start=(g == 0), stop=(g == 1))
                    # fold1: s1[j] = max(ps[j], ps[j+half]), j in [0, half)
                    s1 = s1_pool.tile([128, half], F16, tag="s1")
                    if cw == FT_BC:
                        wd, wp, ws = FT_WD, FT_WP, FT_WS
                    else:
                        wd, wp, ws = FT_RD, half - FT_RD, 0
                    nc.vector.tensor_tensor(
                        s1[:, :wd], ps[:, :wd], ps[:, half:half + wd], op=Max)
                    nc.gpsimd.tensor_tensor(
                        s1[:, wd:wd + wp], ps[:, wd:wd + wp],
                        ps[:, half + wd:half + wd + wp], op=Max)
                    if ws:
                        stg = stg_pool.tile([128, 2 * ws], F16, tag="stg")
                        psv = ps[:, :].rearrange("p (h j) -> p h j", h=2)
                        nc.scalar.activation(
                            stg[:, :].rearrange("p (h j) -> p h j", h=2),
                            psv[:, :, wd + wp:half], Copy)
                        nc.vector.tensor_tensor(
                            s1[:, wd + wp:], stg[:, :ws], stg[:, ws:], op=Max)
                    # fold2: wm[w] = max(s1[w], s1[w+quar]) (fp16, 2x mode)
                    nc.vector.tensor_tensor(
                        wm_sb[:, q * quar:(q + 1) * quar],
                        s1[:, :quar], s1[:, quar:], op=Max)
                nc.sync.dma_start(
                    d_wmax[:, :, wout:wout + quar],
                    wm_sb[:, :].rearrange("p (q w) -> p q w", q=QT))
                wout += quar

    nc.compile()
    return nc


# f8v variant: per-iteration candidate range split into two half-chunks with
# SEPARATE PSUM tiles so each has exactly one reader engine (the tile sync
# compiler has one sem-wait slot per instruction; multiple reader engines on
# one tile get chained serially and stall the PE).
#  - DVE half: fold1 fp32->fp16 pairwise max + fp16 fold2 -> width-4 windows
#  - Pool half: single fp32 pairwise max -> width-2 windows (shipped as-is)
FV_HALF = 1024               # cols per half-chunk (2 PSUM banks)
FV_NWC = FV_HALF // 4 + FV_HALF // 2   # 768 window cols per full chunk
FV_NW = 6 * FV_NWC + (FV_NWC // 4)     # 4800 per core (incl 512-remainder)


def _build_f8v(nc):
    Max = mybir.AluOpType.max
    F8 = mybir.dt.float8e4
    DR = mybir.MatmulPerfMode.DoubleRow

    d_x8 = nc.dram_tensor("x8", [128, 4, B], F8, kind="ExternalInput")
    d_e8 = nc.dram_tensor("e8", [128, 4, N_CORE], F8, kind="ExternalInput")
    d_wmax = nc.dram_tensor("wmax", [128, QT, FV_NW], F16, kind="ExternalOutput")

    chunks = [(i * FT_BC, FT_BC) for i in range(N_CORE // FT_BC)]
    rem = N_CORE - (N_CORE // FT_BC) * FT_BC
    if rem:
        chunks.append((N_CORE - rem, rem))

    def mm(ps, x_sb, e_sb, cw, q, lo, hi):
        """fp8 DR matmuls for chunk cols [lo, hi) into ps[:, 0:hi-lo]."""
        for s0 in range(lo, hi, 512):
            sw = min(512, hi - s0)
            for g in range(2):
                lhsT = x_sb[:, g * 2 * B:(g + 1) * 2 * B].rearrange(
                    "p (j b) -> p j b", j=2)[:, :, q * 128:(q + 1) * 128]
                rhs = e_sb[:, g * 2 * cw:(g + 1) * 2 * cw].rearrange(
                    "p (j n) -> p j n", j=2)[:, :, s0:s0 + sw]
                nc.tensor.matmul(ps[:, s0 - lo:s0 - lo + sw], lhsT, rhs,
                                 perf_mode=DR, start=(g == 0), stop=(g == 1))

    with tile.TileContext(nc) as tc:
        with (
            tc.tile_pool(name="xpool", bufs=1) as xpool,
            tc.tile_pool(name="epool", bufs=3) as epool,
            tc.tile_pool(name="psd", bufs=2, space="PSUM") as psd_pool,
            tc.tile_pool(name="psp", bufs=2, space="PSUM") as psp_pool,
            tc.tile_pool(name="s1p", bufs=3) as s1_pool,
            tc.tile_pool(name="wm", bufs=2) as wm_pool,
        ):
            x_sb = xpool.tile([128, 4 * B], F8, tag="x8")
            nc.sync.dma_start(
                x_sb[:, :].rearrange("p (k b) -> p k b", k=4), d_x8[:, :, :])

            wout = 0
            for ci, (c0, cw) in enumerate(chunks):
                half = cw // 2          # cols per engine region
                hq = half // 2          # fold1 pair count per region
                w4 = half // 4          # width-4 window count (DVE region)
                nwc = w4 + half // 2    # window cols this chunk
                e_sb = epool.tile([128, 4 * cw], F8, tag="e8")
                nc.sync.dma_start(
                    e_sb[:, :].rearrange("p (k n) -> p k n", k=4),
                    d_e8[:, :, c0:c0 + cw])
                wm_sb = wm_pool.tile([128, QT * nwc], F16, tag="wm")
                for q in range(QT):
                    # DVE region: chunk cols [0, half)
                    ps_d = psd_pool.tile([128, half], F32, tag="psd")
                    mm(ps_d, x_sb, e_sb, cw, q, 0, half)
                    s1 = s1_pool.tile([128, hq], F16, tag="s1")
                    nc.vector.tensor_tensor(
                        s1[:, :], ps_d[:, :hq], ps_d[:, hq:], op=Max)
                    o = q * nwc
                    nc.vector.tensor_tensor(
                        wm_sb[:, o:o + w4], s1[:, :w4], s1[:, w4:], op=Max)
                    # Pool region: chunk cols [half, cw) -> width-2 windows
                    ps_p = psp_pool.tile([128, half], F32, tag="psp")
                    mm(ps_p, x_sb, e_sb, cw, q, half, cw)
                    nc.gpsimd.tensor_tensor(
                        wm_sb[:, o + w4:o + nwc], ps_p[:, :hq], ps_p[:, hq:],
                        op=Max)
                nc.sync.dma_start(
                    d_wmax[:, :, wout:wout + nwc],
                    wm_sb[:, :].rearrange("p (q w) -> p q w", q=QT))
                wout += nwc

    nc.compile()
    return nc


# f8m variant: hardware-legal consumption of the sim matrix. Real TRN2
# constraints (walrus verifier): an instruction reads at most ONE operand
# from PSUM; gpsimd (Pool) cannot run TensorTensor at all; DVE pool_max is
# rejected. Legal fast path: Act copies half the candidates PSUM->fp16 SBUF
# (0.83 ns/elem) while DVE retires a pair per cycle via
# max(psum_half, staged_half) (one PSUM operand). 1-bank PSUM tiles at
# depth 4 hide the mm->act->t_t chain; stg/wm are per-chunk tiles so WAR
# deps land on ancient instructions.
FM_NW = 6 * 1024 + 256       # width-2 window cols per core (6400)


def _build_f8m(nc):
    Max = mybir.AluOpType.max
    F8 = mybir.dt.float8e4
    DR = mybir.MatmulPerfMode.DoubleRow
    Copy = mybir.ActivationFunctionType.Copy

    d_x8 = nc.dram_tensor("x8", [128, 4, B], F8, kind="ExternalInput")
    d_e8 = nc.dram_tensor("e8", [128, 4, N_CORE], F8, kind="ExternalInput")
    d_wmax = nc.dram_tensor("wmax", [128, QT, FM_NW], F16, kind="ExternalOutput")

    chunks = [(i * FT_BC, FT_BC) for i in range(N_CORE // FT_BC)]
    rem = N_CORE - (N_CORE // FT_BC) * FT_BC
    if rem:
        chunks.append((N_CORE - rem, rem))

    def mm(ps, x_sb, e_sb, cw, q, lo, hi):
        for s0 in range(lo, hi, 512):
            sw = min(512, hi - s0)
            for g in range(2):
                lhsT = x_sb[:, g * 2 * B:(g + 1) * 2 * B].rearrange(
                    "p (j b) -> p j b", j=2)[:, :, q * 128:(q + 1) * 128]
                rhs = e_sb[:, g * 2 * cw:(g + 1) * 2 * cw].rearrange(
                    "p (j n) -> p j n", j=2)[:, :, s0:s0 + sw]
                nc.tensor.matmul(ps[:, s0 - lo:s0 - lo + sw], lhsT, rhs,
                                 perf_mode=DR, start=(g == 0), stop=(g == 1))

    with tile.TileContext(nc) as tc:
        with (
            tc.tile_pool(name="xpool", bufs=1) as xpool,
            tc.tile_pool(name="epool", bufs=3) as epool,
            tc.tile_pool(name="psd", bufs=4, space="PSUM") as psd_pool,
            tc.tile_pool(name="psa", bufs=4, space="PSUM") as psa_pool,
            tc.tile_pool(name="stg", bufs=2) as stg_pool,
            tc.tile_pool(name="wm", bufs=2) as wm_pool,
        ):
            x_sb = xpool.tile([128, 4 * B], F8, tag="x8")
            x_view = x_sb[:, :].rearrange("p (k b) -> p k b", k=4)

            wout = 0
            for ci, (c0, cw) in enumerate(chunks):
                nhalf = cw // 1024 if cw >= 1024 else 1
                hw_ = cw // nhalf        # cols per iteration (1024 or 512)
                pw = hw_ // 2            # pair count per iteration
                nwc = QT * nhalf * pw    # wm cols this chunk
                e_sb = epool.tile([128, 4 * cw], F8, tag="e8")
                e_view = e_sb[:, :].rearrange("p (k n) -> p k n", k=4)
                # chunk 0: piece-wise loads (first e8 piece, then x8 by
                # q-group) so the first matmuls start ~3us earlier
                npiece = 4 if ci == 0 else 1
                pcw = cw // npiece
                nc.sync.dma_start(e_view[:, :, 0:pcw], d_e8[:, :, c0:c0 + pcw])
                if ci == 0:
                    for xi in range(4):
                        nc.sync.dma_start(
                            x_view[:, :, xi * 512:(xi + 1) * 512],
                            d_x8[:, :, xi * 512:(xi + 1) * 512])
                for pi in range(1, npiece):
                    nc.sync.dma_start(
                        e_view[:, :, pi * pcw:(pi + 1) * pcw],
                        d_e8[:, :, c0 + pi * pcw:c0 + (pi + 1) * pcw])
                wm_sb = wm_pool.tile([128, nwc], F16, tag="wm")
                stg_sb = stg_pool.tile([128, QT * nhalf * pw], F16, tag="stg")
                ng = 4 if ci >= len(chunks) - 2 else 2
                qg = QT // ng
                it = 0
                for q in range(QT):
                    for h in range(nhalf):
                        lo = h * hw_
                        ps_d = psd_pool.tile([128, pw], F32, tag="psd")
                        mm(ps_d, x_sb, e_sb, cw, q, lo, lo + pw)
                        ps_a = psa_pool.tile([128, pw], F32, tag="psa")
                        mm(ps_a, x_sb, e_sb, cw, q, lo + pw, lo + hw_)
                        st = stg_sb[:, it * pw:(it + 1) * pw]
                        nc.scalar.activation(st, ps_a[:, :], Copy)
                        nc.vector.tensor_tensor(
                            wm_sb[:, it * pw:(it + 1) * pw],
                            ps_d[:, :], st, op=Max)
                        it += 1
                        # ship finished q-groups of wm mid-chunk so the
                        # output transfer overlaps compute; finer groups on
                        # the last chunks to shrink the drain tail
                        if it % (qg * nhalf) == 0 and it < QT * nhalf:
                            g0 = it // (qg * nhalf) - 1
                            nc.sync.dma_start(
                                d_wmax[:, g0 * qg:(g0 + 1) * qg,
                                       wout:wout + nhalf * pw],
                                wm_sb[:, g0 * (nwc // ng):(g0 + 1) * (nwc // ng)]
                                .rearrange("p (q w) -> p q w", q=qg))
                nc.sync.dma_start(
                    d_wmax[:, QT - qg:, wout:wout + nhalf * pw],
                    wm_sb[:, nwc - nwc // ng:].rearrange(
                        "p (q w) -> p q w", q=qg))
                wout += nhalf * pw

    nc.compile()
    return nc


def _fm_members():
    """[FM_NW, 4] member map for f8m: all windows width-2 {c, c+pw}."""
    M = np.full((FM_NW, 4), N_EMB, np.int64)
    w0 = 0
    c0 = 0
    while c0 < N_CORE:
        cw = min(FT_BC, N_CORE - c0)
        nhalf = cw // 1024 if cw >= 1024 else 1
        pw = cw // nhalf // 2
        for h in range(nhalf):
            j = np.arange(pw)[:, None]
            M[w0:w0 + pw, :2] = c0 + h * 2 * pw + j + np.arange(2)[None, :] * pw
            w0 += pw
        c0 += cw
    assert w0 == FM_NW
    return M


def _prep_f8w(xn, e, inv):
    """in_maps for the f8w variant: fp8e4m3 transposed normalized shards,
    scaled by F8_SCALE to stay clear of the fp8 subnormal range."""
    import ml_dtypes
    f8 = ml_dtypes.float8_e4m3
    x8 = _to_f8(np.ascontiguousarray(xn.T) * np.float32(F8_SCALE))
    in_maps = []
    for i in range(CORES):
        lo_r, hi_r = i * N_CORE, (i + 1) * N_CORE
        n_real = max(0, min(hi_r, N_EMB) - lo_r)
        e8 = np.zeros((D, N_CORE), dtype=f8)
        if n_real > 0:
            sl = e[lo_r:lo_r + n_real] * (inv[lo_r:lo_r + n_real]
                                          * np.float32(F8_SCALE))[:, None]
            e8[:, :n_real] = _to_f8(sl.T)
        in_maps.append({"x8": x8, "e8": e8})
    return in_maps


def _prep_f8t(xn, e, inv):
    """f8t in_maps: fp8 shards in [p, slot, cols] layout (slot k = rows
    128k..128k+127 of the [D, *] transposed operand)."""
    import ml_dtypes
    f8 = ml_dtypes.float8_e4m3
    x8 = _to_f8(np.ascontiguousarray(xn.T) * np.float32(F8_SCALE))
    x8 = np.ascontiguousarray(x8.reshape(4, 128, B).transpose(1, 0, 2))
    in_maps = []
    for i in range(CORES):
        lo_r, hi_r = i * N_CORE, (i + 1) * N_CORE
        n_real = max(0, min(hi_r, N_EMB) - lo_r)
        e8 = np.zeros((D, N_CORE), dtype=f8)
        if n_real > 0:
            sl = e[lo_r:lo_r + n_real] * (inv[lo_r:lo_r + n_real]
                                          * np.float32(F8_SCALE))[:, None]
            e8[:, :n_real] = _to_f8(sl.T)
        e8 = np.ascontiguousarray(e8.reshape(4, 128, N_CORE).transpose(1, 0, 2))
        in_maps.append({"x8": x8, "e8": e8})
    return in_maps


def _f8t_members():
    """[FT_NW, 4] member map: core-relative candidate ids of each window."""
    M = np.empty((FT_NW, 4), np.int64)
    w0 = 0
    c0 = 0
    while c0 < N_CORE:
        cw = min(FT_BC, N_CORE - c0)
        quar = cw // 4
        M[w0:w0 + quar] = (c0 + np.arange(quar)[:, None]
                           + np.arange(4)[None, :] * quar)
        w0 += quar
        c0 += cw
    assert w0 == FT_NW
    return M


def _fv_members():
    """[FV_NW, 4] member map for f8v; width-2 windows pad with N_EMB (which
    the merge masks to -inf)."""
    M = np.full((FV_NW, 4), N_EMB, np.int64)
    w0 = 0
    c0 = 0
    while c0 < N_CORE:
        cw = min(FT_BC, N_CORE - c0)
        half, hq, w4 = cw // 2, cw // 4, cw // 8
        j = np.arange(w4)[:, None]
        M[w0:w0 + w4] = c0 + j + np.arange(4)[None, :] * w4
        c = np.arange(hq)[:, None]
        M[w0 + w4:w0 + w4 + hq, :2] = c0 + half + c + np.arange(2)[None, :] * hq
        w0 += w4 + hq
        c0 += cw
    assert w0 == FV_NW
    return M


def _merge_fv(results, labels, xn, e, inv, margin, nw, members):
    """Margin-select windows from per-window maxes, exact rescore of each
    kept window's members, exact top-10 + mode. Works for any window->member
    map `members` [nw, 4] (pad slots with ids >= N_EMB)."""
    wv = np.concatenate(
        [np.asarray(r["wmax"]).transpose(1, 0, 2).reshape(B, nw)
         for r in results], axis=1).astype(np.float32)       # [B, 8*nw]
    tot = wv.shape[1]
    w10 = np.partition(wv, tot - K_NEIGH, axis=1)[:, tot - K_NEIGH]
    keep = wv >= (w10[:, None] - margin)                     # [B, 8*nw]

    rows_idx, wins = np.nonzero(keep)        # wins: global window ids
    slots = (np.cumsum(keep, axis=1) - 1)[rows_idx, wins]
    smax = int(keep.sum(axis=1).max())

    e = np.asarray(e, dtype=np.float32)
    xn32 = np.ascontiguousarray(xn, dtype=np.float32)
    order = np.argsort(wins, kind="stable")
    rows_s, slots_s, wins_s = rows_idx[order], slots[order], wins[order]
    uniq, starts = np.unique(wins_s, return_index=True)
    bounds = np.append(starts, len(wins_s))

    sims = np.full((B, smax, 4), -np.inf, dtype=np.float32)
    wfull = np.zeros((B, smax), dtype=np.int64)
    wfull[rows_idx, slots] = wins
    for ui in range(len(uniq)):
        w = int(uniq[ui])
        core, wloc = divmod(w, nw)
        mem = core * N_CORE + members[wloc]                  # [4] global ids
        valid = mem < N_EMB
        if not valid.any():
            continue
        mv = mem[valid]
        s0, s1 = bounds[ui], bounds[ui + 1]
        en_w = e[mv] * inv[mv][:, None]
        sblk = xn32[rows_s[s0:s1]] @ en_w.T                  # [nrows, <=4]
        sims[rows_s[s0:s1][:, None], slots_s[s0:s1][:, None],
             np.nonzero(valid)[0][None, :]] = sblk

    cores_f = wfull // nw
    cand = np.minimum(cores_f[:, :, None] * N_CORE + members[wfull % nw],
                      N_EMB).reshape(B, -1)
    sims = sims.reshape(B, -1)
    u = sims.view(np.uint32)
    mono = np.where(u & 0x80000000, ~u, u | 0x80000000).astype(np.uint64)
    combo = ((np.uint64(0xFFFFFFFF) - mono) << np.uint64(17)) | \
        cand.astype(np.uint64)
    combo[sims == -np.inf] = np.uint64(0xFFFFFFFFFFFFFFFF)
    ordr = np.argsort(combo, axis=1, kind="stable")[:, :K_NEIGH]
    neighbors = np.take_along_axis(cand, ordr, axis=1)
    return _mode_pred(neighbors, labels)


def _merge_f8t(results, labels, xn, e, inv, margin):
    """Margin-select windows from per-window maxes, exact rescore of the
    4 members of each kept window, exact top-10 + mode."""
    # device output [p, q, w] -> rows q*128+p
    wv = np.concatenate(
        [np.asarray(r["wmax"]).transpose(1, 0, 2).reshape(B, FT_NW)
         for r in results], axis=1).astype(np.float32)       # [B, 8*FT_NW]
    nw = wv.shape[1]
    w10 = np.partition(wv, nw - K_NEIGH, axis=1)[:, nw - K_NEIGH]
    keep = wv >= (w10[:, None] - margin)                     # [B, 8*FT_NW]

    rows_idx, wins = np.nonzero(keep)        # wins: global window ids
    slots = (np.cumsum(keep, axis=1) - 1)[rows_idx, wins]
    smax = int(keep.sum(axis=1).max())

    M = _f8t_members()                       # [FT_NW, 4]
    e = np.asarray(e, dtype=np.float32)
    xn32 = np.ascontiguousarray(xn, dtype=np.float32)
    order = np.argsort(wins, kind="stable")
    rows_s, slots_s, wins_s = rows_idx[order], slots[order], wins[order]
    uniq, starts = np.unique(wins_s, return_index=True)
    bounds = np.append(starts, len(wins_s))

    sims = np.full((B, smax, 4), -np.inf, dtype=np.float32)
    wfull = np.zeros((B, smax), dtype=np.int64)
    wfull[rows_idx, slots] = wins
    for ui in range(len(uniq)):
        w = int(uniq[ui])
        core, wloc = divmod(w, FT_NW)
        mem = core * N_CORE + M[wloc]                        # [4] global ids
        valid = mem < N_EMB
        if not valid.any():
            continue
        mv = mem[valid]
        s0, s1 = bounds[ui], bounds[ui + 1]
        en_w = e[mv] * inv[mv][:, None]
        sblk = xn32[rows_s[s0:s1]] @ en_w.T                  # [nrows, <=4]
        sims[rows_s[s0:s1][:, None], slots_s[s0:s1][:, None],
             np.nonzero(valid)[0][None, :]] = sblk

    cores_f = wfull // FT_NW
    cand = (cores_f[:, :, None] * N_CORE + M[wfull % FT_NW]).reshape(B, -1)
    sims = sims.reshape(B, -1)
    u = sims.view(np.uint32)
    mono = np.where(u & 0x80000000, ~u, u | 0x80000000).astype(np.uint64)
    combo = ((np.uint64(0xFFFFFFFF) - mono) << np.uint64(17)) | \
        cand.astype(np.uint64)
    combo[sims == -np.inf] = np.uint64(0xFFFFFFFFFFFFFFFF)
    ordr = np.argsort(combo, axis=1, kind="stable")[:, :K_NEIGH]
    neighbors = np.take_along_axis(cand, ordr, axis=1)
    return _mode_pred(neighbors, labels)


def _get_nc(variant=None):
    variant = variant or MM_DTYPE
    if variant not in _CACHE:
        _CACHE[variant] = _build(variant)
    return _CACHE[variant]


def _normalize(x, embeddings):
    x = np.asarray(x, dtype=np.float32)
    e = np.asarray(embeddings, dtype=np.float32)
    xn = x / np.maximum(np.linalg.norm(x, axis=1, keepdims=True), EPS)
    inv = (1.0 / np.maximum(np.linalg.norm(e, axis=1), EPS)).astype(np.float32)
    return xn, e, inv


def _prep_f16w(xn, e, inv):
    """in_maps for the f16w variant: fp16 transposed normalized shards."""
    xh = np.ascontiguousarray(xn.T).astype(np.float16)
    in_maps = []
    for i in range(CORES):
        lo_r, hi_r = i * N_CORE, (i + 1) * N_CORE
        n_real = max(0, min(hi_r, N_EMB) - lo_r)
        eh = np.zeros((D, N_CORE), dtype=np.float16)
        if n_real > 0:
            sl = e[lo_r:lo_r + n_real] * inv[lo_r:lo_r + n_real][:, None]
            eh[:, :n_real] = sl.T.astype(np.float16)
        in_maps.append({"xh": xh, "eh": eh})
    return in_maps


def _prep_inputs(x, embeddings, variant):
    """Host prep: normalize embeddings, pad, transpose, shard; returns in_maps.

    Works per-core-shard to keep intermediates cache-sized."""
    if variant == "f16w":
        xn, e, inv = _normalize(x, embeddings)
        return _prep_f16w(xn, e, inv)
    if variant in ("f8t", "f8v", "f8m"):
        xn, e, inv = _normalize(x, embeddings)
        return _prep_f8t(xn, e, inv)
    if variant in ("f8w", "f8d", "f8e"):
        xn, e, inv = _normalize(x, embeddings)
        return _prep_f8w(xn, e, inv)
    x = np.asarray(x, dtype=np.float32)
    e = np.asarray(embeddings, dtype=np.float32)
    inv = (1.0 / np.maximum(np.linalg.norm(e, axis=1), EPS)).astype(np.float32)
    xt = np.ascontiguousarray(x.T)               # [D, B]

    in_maps = []
    for i in range(CORES):
        lo_r, hi_r = i * N_CORE, (i + 1) * N_CORE
        n_real = max(0, min(hi_r, N_EMB) - lo_r)
        ent = np.zeros((D, N_CORE), dtype=np.float32)
        if n_real > 0:
            sl = e[lo_r:lo_r + n_real]
            ent[:, :n_real] = sl.T * inv[lo_r:lo_r + n_real][None, :]
        if variant == "f16x3":
            ehi = ent.astype(np.float16)
            elo = (ent - ehi).astype(np.float16)
            in_maps.append({"ehi": ehi, "elo": elo})
        else:
            in_maps.append({"ent": ent})

    if variant == "f16x3":
        xhi = xt.astype(np.float16)
        xlo = (xt - xhi).astype(np.float16)
        for m in in_maps:
            m["xhi"] = xhi
            m["xlo"] = xlo
    else:
        for m in in_maps:
            m["xt"] = xt
    return in_maps


def _merge(results, labels):
    """Host merge: exact global top-10 from per-core per-chunk top-8 pools,
    then the reference's mode computation."""
    vals = np.concatenate([r["vals"] for r in results], axis=1)   # [B, 8*NOUT]
    idx8 = np.concatenate([r["idx"] for r in results], axis=1).astype(np.int64)

    col_base = (np.arange(NOUT, dtype=np.int64) // 8) * CHUNK      # chunk offset
    core_base = np.repeat(np.arange(CORES, dtype=np.int64) * N_CORE, NOUT)
    g = idx8 + np.tile(col_base, CORES)[None, :] + core_base[None, :]

    # padding rows (g >= N_EMB) are zero embeddings: exclude
    u = vals.view(np.uint32)
    key = np.where(u & 0x80000000, ~u, u | 0x80000000).astype(np.uint64)
    combo = ((np.uint64(0xFFFFFFFF) - key) << np.uint64(17)) | g.astype(np.uint64)
    combo[g >= N_EMB] = np.uint64(0xFFFFFFFFFFFFFFFF)
    order = np.argsort(combo, axis=1, kind="stable")[:, :K_NEIGH]
    neighbors = np.take_along_axis(g, order, axis=1)               # [B, 10]

    labels = np.asarray(labels)
    nl = labels[neighbors].astype(np.int64)                        # [B, 10]
    eq = nl[:, :, None] == nl[:, None, :]
    counts = eq.sum(-1)
    mkey = counts * (NUM_CLASSES + 1) + (NUM_CLASSES - nl)
    mi = np.argmax(mkey, axis=1)
    pred = np.take_along_axis(nl, mi[:, None], axis=1)[:, 0]
    return pred.astype(labels.dtype)


class _Runner:
    """Caches the shard_map-jitted executable across calls (mirrors
    bass2jax.run_bass_via_pjrt's multi-core path, which re-traces per call)."""

    def __init__(self, variant):
        import jax
        import concourse.mybir as mb
        from concourse import bass2jax
        from jax.experimental.shard_map import shard_map
        from jax.sharding import Mesh, PartitionSpec

        bass2jax.install_neuronx_cc_hook()
        self.jax = jax
        nc = _get_nc(variant)
        partition_name = (nc.partition_id_tensor.name
                          if nc.partition_id_tensor else None)
        in_names, out_names, out_avals, zeros = [], [], [], []
        for alloc in nc.m.functions[0].allocations:
            if not isinstance(alloc, mb.MemoryLocationSet):
                continue
            name = alloc.memorylocations[0].name
            if alloc.kind == "ExternalInput":
                if name != partition_name:
                    in_names.append(name)
            elif alloc.kind == "ExternalOutput":
                shape = tuple(alloc.tensor_shape)
                dtype = mb.dt.np(alloc.dtype)
                out_avals.append(jax.core.ShapedArray(shape, dtype))
                out_names.append(name)
                zeros.append(np.zeros((CORES * shape[0],) + shape[1:], dtype))
        self.in_names = list(in_names)
        self.out_names = out_names
        self.out_avals = out_avals
        self.zeros = zeros
        n_params = len(in_names)
        all_names = in_names + out_names
        if partition_name is not None:
            all_names = all_names + [partition_name]
        donate = tuple(range(n_params, n_params + len(out_names)))

        def _body(*args):
            operands = list(args)
            if partition_name is not None:
                operands.append(bass2jax.partition_id_tensor())
            outs = bass2jax._bass_exec_p.bind(
                *operands,
                out_avals=tuple(out_avals),
                in_names=tuple(all_names),
                out_names=tuple(out_names),
                lowering_input_output_aliases=(),
                sim_require_finite=True,
                sim_require_nnan=True,
                nc=nc,
            )
            return tuple(outs)

        devices = jax.devices()[:CORES]
        self.mesh = Mesh(np.asarray(devices), ("core",))
        self.pspec = PartitionSpec("core")
        in_specs = (self.pspec,) * (n_params + len(out_names))
        out_specs = (self.pspec,) * len(out_names)
        self.sharded = jax.jit(
            shard_map(_body, mesh=self.mesh, in_specs=in_specs,
                      out_specs=out_specs, check_rep=False),
            donate_argnums=donate, keep_unused=True,
        )

    def concat_inputs(self, in_maps):
        return [
            np.concatenate([np.asarray(m[name]) for m in in_maps], axis=0)
            for name in self.in_names
        ]

    def device_put(self, concat_in):
        from jax.sharding import NamedSharding
        sh = NamedSharding(self.mesh, self.pspec)
        return [self.jax.device_put(a, sh) for a in concat_in]

    def execute(self, concat_in):
        zeros = [np.zeros_like(z) for z in self.zeros]
        out_arrs = self.sharded(*concat_in, *zeros)
        return out_arrs

    def run(self, in_maps):
        out_arrs = self.execute(self.concat_inputs(in_maps))
        return [
            {
                name: np.asarray(out_arrs[i]).reshape(
                    CORES, *self.out_avals[i].shape)[c]
                for i, name in enumerate(self.out_names)
            }
            for c in range(CORES)
        ]


_RUNNERS = {}


def _get_runner(variant=None):
    variant = variant or MM_DTYPE
    if variant not in _RUNNERS:
        _RUNNERS[variant] = _Runner(variant)
    return _RUNNERS[variant]


def _mode_pred(neighbors, labels):
    """Reference's torch.mode semantics on gathered neighbor labels."""
    labels = np.asarray(labels)
    nl = labels[neighbors].astype(np.int64)                        # [B, 10]
    eq = nl[:, :, None] == nl[:, None, :]
    counts = eq.sum(-1)
    mkey = counts * (NUM_CLASSES + 1) + (NUM_CLASSES - nl)
    mi = np.argmax(mkey, axis=1)
    pred = np.take_along_axis(nl, mi[:, None], axis=1)[:, 0]
    return pred.astype(labels.dtype)


def _merge_f16w(results, labels, xn, e, inv, margin=MARGIN):
    """Select windows >= (10th-best window max) - margin, rescore those
    candidates exactly in fp64, exact global top-10, then mode."""
    wv = np.stack([r["wvals"] for r in results], axis=1)      # [B, 8, 32]
    wi = np.stack([r["widx"] for r in results], axis=1).astype(np.int64)
    wi[:, :, NSEL:] += HALF_A   # half-B indices are relative to its slice
    gw = wi + (np.arange(CORES, dtype=np.int64) * WPC)[None, :, None]
    wv = wv.reshape(B, CORES * 2 * NSEL)
    gw = gw.reshape(B, CORES * 2 * NSEL)

    w10 = np.partition(wv, wv.shape[1] - K_NEIGH, axis=1)[:, wv.shape[1] - K_NEIGH]
    keep = wv >= (w10[:, None] - margin)
    smax = int(keep.sum(axis=1).max())

    # top-smax windows per row by value; mask out ones below the cutoff
    order = np.argsort(-wv, axis=1, kind="stable")[:, :smax]
    sel_g = np.take_along_axis(gw, order, axis=1)              # [B, smax]
    sel_keep = np.take_along_axis(keep, order, axis=1)

    # rescore grouped by window: each window's embeddings are one contiguous
    # 32-row slice, shared by every query that selected it (~6400 windows
    # total vs ~170k (row, window) pairs -> tiny gathers, BLAS-sized GEMMs)
    e = np.asarray(e, dtype=np.float32)
    xn32 = np.ascontiguousarray(xn, dtype=np.float32)
    rows_idx, slots = np.nonzero(sel_keep)
    wins = sel_g[rows_idx, slots]
    order = np.argsort(wins, kind="stable")
    rows_idx, slots, wins = rows_idx[order], slots[order], wins[order]
    uniq, starts = np.unique(wins, return_index=True)
    bounds = np.append(starts, len(wins))

    sims = np.full((B, smax, WWIN), -np.inf, dtype=np.float32)
    for ui in range(len(uniq)):
        w = int(uniq[ui])
        c0, c1 = w * WWIN, min(w * WWIN + WWIN, N_EMB)
        if c1 <= c0:
            continue
        s0, s1 = bounds[ui], bounds[ui + 1]
        en_w = e[c0:c1] * inv[c0:c1][:, None]                  # [<=32, D]
        sblk = xn32[rows_idx[s0:s1]] @ en_w.T                  # [nrows, <=32]
        sims[rows_idx[s0:s1], slots[s0:s1], :c1 - c0] = sblk

    cand = (sel_g[:, :, None] * WWIN +
            np.arange(WWIN, dtype=np.int64)[None, None, :]).reshape(B, -1)
    sims = sims.reshape(B, -1)

    # exact top-10 by (-sim, cand) via an order-preserving uint64 key
    u = sims.view(np.uint32)
    mono = np.where(u & 0x80000000, ~u, u | 0x80000000).astype(np.uint64)
    combo = ((np.uint64(0xFFFFFFFF) - mono) << np.uint64(17)) | \
        cand.astype(np.uint64)
    combo[sims == -np.inf] = np.uint64(0xFFFFFFFFFFFFFFFF)
    ordr = np.argsort(combo, axis=1, kind="stable")[:, :K_NEIGH]
    neighbors = np.take_along_axis(cand, ordr, axis=1)
    return _mode_pred(neighbors, labels)


def _merge_f8d(results, labels, xn, e, inv, margin):
    """Host-side window selection from the full per-window-max arrays, then
    the window-grouped exact rescore."""
    wv = np.concatenate([r["wmax"] for r in results], axis=1)   # [B, 8*WPC]
    nw = wv.shape[1]
    w10 = np.partition(wv, nw - K_NEIGH, axis=1)[:, nw - K_NEIGH]
    keep = wv >= (w10[:, None] - margin)                        # [B, 8*WPC]

    rows_idx, wins = np.nonzero(keep)        # wins are global window ids
    slots = (np.cumsum(keep, axis=1) - 1)[rows_idx, wins]
    smax = int(keep.sum(axis=1).max())

    e = np.asarray(e, dtype=np.float32)
    xn32 = np.ascontiguousarray(xn, dtype=np.float32)
    order = np.argsort(wins, kind="stable")
    rows_s, slots_s, wins_s = rows_idx[order], slots[order], wins[order]
    uniq, starts = np.unique(wins_s, return_index=True)
    bounds = np.append(starts, len(wins_s))

    sims = np.full((B, smax, WWIN), -np.inf, dtype=np.float32)
    wfull = np.zeros((B, smax), dtype=np.int64)
    wfull[rows_idx, slots] = wins
    for ui in range(len(uniq)):
        w = int(uniq[ui])
        c0, c1 = w * WWIN, min(w * WWIN + WWIN, N_EMB)
        if c1 <= c0:
            continue
        s0, s1 = bounds[ui], bounds[ui + 1]
        en_w = e[c0:c1] * inv[c0:c1][:, None]
        sblk = xn32[rows_s[s0:s1]] @ en_w.T
        sims[rows_s[s0:s1], slots_s[s0:s1], :c1 - c0] = sblk

    cand = (wfull[:, :, None] * WWIN +
            np.arange(WWIN, dtype=np.int64)[None, None, :]).reshape(B, -1)
    sims = sims.reshape(B, -1)
    u = sims.view(np.uint32)
    mono = np.where(u & 0x80000000, ~u, u | 0x80000000).astype(np.uint64)
    combo = ((np.uint64(0xFFFFFFFF) - mono) << np.uint64(17)) | \
        cand.astype(np.uint64)
    combo[sims == -np.inf] = np.uint64(0xFFFFFFFFFFFFFFFF)
    ordr = np.argsort(combo, axis=1, kind="stable")[:, :K_NEIGH]
    neighbors = np.take_along_axis(cand, ordr, axis=1)
    return _mode_pred(neighbors, labels)


def run_on_hw(x, embeddings, variant=None):
    runner = _get_runner(variant)
    in_maps = _prep_inputs(x, embeddings, variant or MM_DTYPE)
    return runner.run(in_maps)


def kernel(x, embeddings, labels):
    variant = MM_DTYPE
    if variant == "f16w":
        xn, e, inv = _normalize(x, embeddings)
        runner = _get_runner(variant)
        results = runner.run(_prep_f16w(xn, e, inv))
        return _merge_f16w(results, labels, xn, e, inv)
    if variant == "f8w":
        xn, e, inv = _normalize(x, embeddings)
        runner = _get_runner(variant)
        results = runner.run(_prep_f8w(xn, e, inv))
        return _merge_f16w(results, labels, xn, e, inv,
                           margin=MARGIN_F8 * F8_SCALE * F8_SCALE)
    if variant == "f8t":
        xn, e, inv = _normalize(x, embeddings)
        runner = _get_runner(variant)
        results = runner.run(_prep_f8t(xn, e, inv))
        # margin: fp8 sim error (scaled) + 2x fp16 rounding of the maxes
        return _merge_f8t(results, labels, xn, e, inv,
                          margin=MARGIN_F8 * F8_SCALE * F8_SCALE + 0.5)
    if variant == "f8v":
        xn, e, inv = _normalize(x, embeddings)
        runner = _get_runner(variant)
        results = runner.run(_prep_f8t(xn, e, inv))
        return _merge_fv(results, labels, xn, e, inv,
                         MARGIN_F8 * F8_SCALE * F8_SCALE + 0.5,
                         FV_NW, _fv_members())
    if variant == "f8m":
        xn, e, inv = _normalize(x, embeddings)
        runner = _get_runner(variant)
        results = runner.run(_prep_f8t(xn, e, inv))
        return _merge_fv(results, labels, xn, e, inv,
                         MARGIN_F8 * F8_SCALE * F8_SCALE + 0.5,
                         FM_NW, _fm_members())
    if variant in ("f8d", "f8e"):
        xn, e, inv = _normalize(x, embeddings)
        runner = _get_runner(variant)
        results = runner.run(_prep_f8w(xn, e, inv))
        return _merge_f8d(results, labels, xn, e, inv,
                          margin=MARGIN_F8 * F8_SCALE * F8_SCALE)
    results = run_on_hw(x, embeddings)
    return _merge(results, labels)



# revision 26
# speedup vs baseline: 1.6370x; 1.0186x over previous
"""Trainium2 Bass kernel for BaselineKNNModel (cosine-sim KNN classifier).

Contract: kernel(**inputs) takes FULL inputs (x [2048,512] f32,
embeddings [100000,512] f32, labels [100000] int) and returns the FULL
output (pred [2048] labels.dtype), distributing work across 8 NeuronCores.

Strategy (database-parallel, per sharding hint):
 - Host: normalize embeddings (cosine denominator), pad N 100000->102400,
   transpose to [512, N]; shard along N across 8 cores (12800 each).
   x normalization is skipped: per-query positive scaling cannot change
   that query's top-k ranking.
 - Device (SPMD, per core): sim tile [128 q, 512 c] = xT.T @ enT chunk via
   PE accumulation over K=512; per tile, VectorE max/max_index extract the
   top-8 values + indices of each 512-candidate chunk (global top-10 of a
   row is contained in the union of its per-chunk top-8s unless >=9 of the
   top-10 fall in one 512-chunk: P ~ 1e-11).
 - Host: merge 8 cores x 25 chunks x top-8 = 1600 candidates/query, exact
   top-10 by (value desc, index asc) = jax.lax.top_k tie order, then the
   reference's mode computation.
"""
import sys

for _p in ("/opt/trn_rl_repo", "/root/.axon_site/_ro/trn_rl_repo"):
    if _p not in sys.path:
        sys.path.insert(0, _p)

import numpy as np

import concourse.bacc as bacc
import concourse.mybir as mybir
import concourse.tile as tile
from concourse import bass_utils

F32 = mybir.dt.float32
F32R = mybir.dt.float32r
F16 = mybir.dt.float16
U32 = mybir.dt.uint32
Copy = mybir.ActivationFunctionType.Copy

B = 2048            # queries
D = 512             # embedding dim
N_EMB = 100000      # database size
K_NEIGH = 10
NUM_CLASSES = 1000
EPS = 1e-8

CORES = 8
N_PAD = 102400      # padded database size (8 * 12800)
N_CORE = N_PAD // CORES     # 12800 candidates per core
CHUNK = 512                 # candidates per sim tile (one PSUM bank)
NCHUNK = N_CORE // CHUNK    # 25
QT = B // 128               # 16 query tiles
KT = D // 128               # 4 k-tiles
NOUT = NCHUNK * 8           # 200 output slots per query per core

# f16w variant: window-max + device window top-16 + host exact rescore
WWIN = 32                   # candidates per window
WPC = N_CORE // WWIN        # 400 windows per core
BIGCHUNK = 1024             # candidates per PSUM tile (2 banks)
NSEL = 16                   # windows kept per (query, core, half)
HALF_A = (7 * BIGCHUNK) // WWIN  # windows in selection half A (224)
MARGIN = 4e-3               # fp16-sim error margin on unit-normalized sims
                            # (measured max |fp16 sim err| ~6e-5, ~60x safety)

# f8w variant: same as f16w but fp8e4m3 DoubleRow matmuls (2 fp8 weights per
# PE cell, K=256 per matmul). Inputs are scaled by F8_SCALE before rounding
# to fp8, so device sims (and window maxes) are scaled by F8_SCALE^2.
F8_SCALE = 16.0
MARGIN_F8 = 2.5e-2          # fp8 margin on unit-normalized sims
                            # (measured max err 7.1e-3 on a sample, rms 1.6e-3)

MM_DTYPE = "f8t"   # "f32" | "f32r" | "f16x3" | "f16w" | "f8w" | "f8d" | "f8e" | "f8t"

_CACHE = {}


def _build(variant):
    """Build + compile the per-core Bass program. Same program on all cores;
    only the `ent*` input shards differ."""
    nc = bacc.Bacc("TRN2", target_bir_lowering=False, debug=False)

    if variant == "noop":  # minimal program for RPC-overhead baselining
        d_nin = nc.dram_tensor("nin", [128, 128], F32, kind="ExternalInput")
        d_nout = nc.dram_tensor("nout", [128, 128], F32, kind="ExternalOutput")
        with tile.TileContext(nc) as tc:
            with tc.tile_pool(name="np0", bufs=1) as pool:
                t = pool.tile([128, 128], F32, tag="t")
                nc.sync.dma_start(t[:, :], d_nin[:, :])
                nc.sync.dma_start(d_nout[:, :], t[:, :])
        nc.compile()
        return nc

    if variant == "f16w":
        return _build_f16w(nc)
    if variant == "f8w":
        return _build_f8w(nc)
    if variant == "f8d":
        return _build_f8d(nc)
    if variant == "f8e":
        return _build_f8e(nc)
    if variant == "f8t":
        return _build_f8t(nc)
    if variant == "f8v":
        return _build_f8v(nc)
    if variant == "f8m":
        return _build_f8m(nc)

    f16 = variant == "f16x3"
    if f16:
        d_xhi = nc.dram_tensor("xhi", [D, B], F16, kind="ExternalInput")
        d_xlo = nc.dram_tensor("xlo", [D, B], F16, kind="ExternalInput")
        d_ehi = nc.dram_tensor("ehi", [D, N_CORE], F16, kind="ExternalInput")
        d_elo = nc.dram_tensor("elo", [D, N_CORE], F16, kind="ExternalInput")
    else:
        in_dt = F32R if variant == "f32r" else F32
        d_xt = nc.dram_tensor("xt", [D, B], in_dt, kind="ExternalInput")
        d_ent = nc.dram_tensor("ent", [D, N_CORE], in_dt, kind="ExternalInput")

    d_vals = nc.dram_tensor("vals", [B, NOUT], F32, kind="ExternalOutput")
    d_idx = nc.dram_tensor("idx", [B, NOUT], U32, kind="ExternalOutput")

    with tile.TileContext(nc) as tc:
        with (
            tc.tile_pool(name="xpool", bufs=1) as xpool,
            tc.tile_pool(name="epool", bufs=3) as epool,
            tc.tile_pool(name="ps", bufs=6, space="PSUM") as ps_pool,
            tc.tile_pool(name="sim", bufs=6) as sim_pool,
            tc.tile_pool(name="acc", bufs=1) as acc_pool,
        ):
            # resident x (stationary operand), k-tiles side by side
            if f16:
                xhi_sb = xpool.tile([128, KT * B], F16, tag="xhi")
                xlo_sb = xpool.tile([128, KT * B], F16, tag="xlo")
                for k in range(KT):
                    nc.sync.dma_start(xhi_sb[:, k * B:(k + 1) * B],
                                      d_xhi[k * 128:(k + 1) * 128, :])
                    nc.sync.dma_start(xlo_sb[:, k * B:(k + 1) * B],
                                      d_xlo[k * 128:(k + 1) * 128, :])
            else:
                xt_sb = xpool.tile([128, KT * B], in_dt, tag="xt")
                for k in range(KT):
                    nc.sync.dma_start(xt_sb[:, k * B:(k + 1) * B],
                                      d_xt[k * 128:(k + 1) * 128, :])

            # result accumulators, [128, QT*NOUT], column q*NOUT + c*8 + j
            vals_sb = acc_pool.tile([128, QT * NOUT], F32, tag="vacc")
            idx_sb = acc_pool.tile([128, QT * NOUT], U32, tag="iacc")

            for c in range(NCHUNK):
                c0 = c * CHUNK
                if f16:
                    ehi_sb = epool.tile([128, KT * CHUNK], F16, tag="ehi")
                    elo_sb = epool.tile([128, KT * CHUNK], F16, tag="elo")
                    for k in range(KT):
                        nc.sync.dma_start(ehi_sb[:, k * CHUNK:(k + 1) * CHUNK],
                                          d_ehi[k * 128:(k + 1) * 128, c0:c0 + CHUNK])
                        nc.sync.dma_start(elo_sb[:, k * CHUNK:(k + 1) * CHUNK],
                                          d_elo[k * 128:(k + 1) * 128, c0:c0 + CHUNK])
                else:
                    en_sb = epool.tile([128, KT * CHUNK], in_dt, tag="en")
                    for k in range(KT):
                        nc.sync.dma_start(en_sb[:, k * CHUNK:(k + 1) * CHUNK],
                                          d_ent[k * 128:(k + 1) * 128, c0:c0 + CHUNK])

                for q in range(QT):
                    ps = ps_pool.tile([128, CHUNK], F32, tag="ps")
                    if variant == "f16x3":
                        nmm = 3 * KT
                        i = 0
                        for k in range(KT):
                            xh = xhi_sb[:, k * B + q * 128: k * B + (q + 1) * 128]
                            xl = xlo_sb[:, k * B + q * 128: k * B + (q + 1) * 128]
                            eh = ehi_sb[:, k * CHUNK:(k + 1) * CHUNK]
                            el = elo_sb[:, k * CHUNK:(k + 1) * CHUNK]
                            for (a, bb) in ((xh, eh), (xh, el), (xl, eh)):
                                nc.tensor.matmul(ps[:, :], a, bb,
                                                 start=(i == 0), stop=(i == nmm - 1))
                                i += 1
                    else:
                        for k in range(KT):
                            lhsT = xt_sb[:, k * B + q * 128: k * B + (q + 1) * 128]
                            rhs = en_sb[:, k * CHUNK:(k + 1) * CHUNK]
                            nc.tensor.matmul(ps[:, :], lhsT, rhs,
                                             start=(k == 0), stop=(k == KT - 1))

                    sim = sim_pool.tile([128, CHUNK], F32, tag="sim")
                    nc.scalar.activation(sim[:, :], ps[:, :], Copy)

                    o = q * NOUT + c * 8
                    nc.vector.max(vals_sb[:, o:o + 8], sim[:, :])
                    nc.vector.max_index(idx_sb[:, o:o + 8], vals_sb[:, o:o + 8],
                                        sim[:, :])

            for q in range(QT):
                nc.sync.dma_start(d_vals[q * 128:(q + 1) * 128, :],
                                  vals_sb[:, q * NOUT:(q + 1) * NOUT])
                nc.sync.dma_start(d_idx[q * 128:(q + 1) * 128, :],
                                  idx_sb[:, q * NOUT:(q + 1) * NOUT])

    nc.compile()
    return nc


def _build_f16w(nc):
    """fp16 single-pass matmul; per-tile 16-wide window max (DVE reduce,
    PSUM-direct); per-core-half top-16 windows per query via
    max/match_replace (first half's selection overlaps the main loop);
    host rescores the selected windows exactly."""
    Max = mybir.AluOpType.max
    X = mybir.AxisListType.X

    d_xh = nc.dram_tensor("xh", [D, B], F16, kind="ExternalInput")
    d_eh = nc.dram_tensor("eh", [D, N_CORE], F16, kind="ExternalInput")
    d_wvals = nc.dram_tensor("wvals", [B, 2 * NSEL], F32, kind="ExternalOutput")
    d_widx = nc.dram_tensor("widx", [B, 2 * NSEL], U32, kind="ExternalOutput")

    # chunk layout: 12 x 1024 + 1 x 512 = 12800
    chunks = [(i * BIGCHUNK, BIGCHUNK) for i in range(N_CORE // BIGCHUNK)]
    rem = N_CORE - (N_CORE // BIGCHUNK) * BIGCHUNK
    if rem:
        chunks.append((N_CORE - rem, rem))
    # selection halves aligned to chunk boundaries:
    # half A = chunks 0-6 (448 windows), half B = chunks 7-12 (352 windows)
    HALF_B = WPC - HALF_A

    def select(wq, vout, iout, o, width, mr_pool):
        nc.vector.max(vout[:, o:o + 8], wq)
        nc.vector.max_index(iout[:, o:o + 8], vout[:, o:o + 8], wq)
        mr = mr_pool.tile([128, width], F32, tag="mr")
        nc.vector.match_replace(mr[:, :width], vout[:, o:o + 8], wq, -1e30)
        nc.vector.max(vout[:, o + 8:o + 16], mr[:, :width])
        nc.vector.max_index(iout[:, o + 8:o + 16],
                            vout[:, o + 8:o + 16], mr[:, :width])

    with tile.TileContext(nc) as tc:
        with (
            tc.tile_pool(name="xpool", bufs=1) as xpool,
            tc.tile_pool(name="epool", bufs=3) as epool,
            tc.tile_pool(name="ps", bufs=3, space="PSUM") as ps_pool,
            tc.tile_pool(name="wacc", bufs=1) as wacc_pool,
            tc.tile_pool(name="mrp", bufs=4) as mr_pool,
            tc.tile_pool(name="outp", bufs=1) as out_pool,
        ):
            xh_sb = xpool.tile([128, KT * B], F16, tag="xh")
            for k in range(KT):
                nc.sync.dma_start(xh_sb[:, k * B:(k + 1) * B],
                                  d_xh[k * 128:(k + 1) * 128, :])

            wmax_sb = wacc_pool.tile([128, QT * WPC], F32, tag="wacc")
            vout_sb = out_pool.tile([128, QT * 2 * NSEL], F32, tag="vout")
            iout_sb = out_pool.tile([128, QT * 2 * NSEL], U32, tag="iout")

            for ci, (c0, cw) in enumerate(chunks):
                eh_sb = epool.tile([128, KT * BIGCHUNK], F16, tag="eh")
                for k in range(KT):
                    nc.sync.dma_start(eh_sb[:, k * cw:(k + 1) * cw],
                                      d_eh[k * 128:(k + 1) * 128, c0:c0 + cw])
                for q in range(QT):
                    ps = ps_pool.tile([128, BIGCHUNK], F32, tag="ps")
                    for s in range(cw // 512):
                        for k in range(KT):
                            nc.tensor.matmul(
                                ps[:, s * 512:(s + 1) * 512],
                                xh_sb[:, k * B + q * 128: k * B + (q + 1) * 128],
                                eh_sb[:, k * cw + s * 512: k * cw + s * 512 + 512],
                                start=(k == 0), stop=(k == KT - 1))
                    nwin = cw // WWIN
                    wslot = q * WPC + c0 // WWIN
                    nc.vector.tensor_reduce(
                        wmax_sb[:, wslot:wslot + nwin],
                        ps[:, :cw].rearrange("p (w i) -> p w i", i=WWIN),
                        axis=X, op=Max)
                # half A (windows [0, HALF_A)) is complete after chunk 6;
                # spread its per-q selection over chunks 6..12 (2-3 q each)
                if ci >= 6:
                    n_grp = len(chunks) - 6
                    qs = [q for q in range(QT) if q % n_grp == ci - 6]
                    for q in qs:
                        select(wmax_sb[:, q * WPC:q * WPC + HALF_A],
                               vout_sb, iout_sb, q * 2 * NSEL, HALF_A, mr_pool)

            for q in range(QT):  # half B (windows [HALF_A, WPC))
                select(wmax_sb[:, q * WPC + HALF_A:(q + 1) * WPC],
                       vout_sb, iout_sb, q * 2 * NSEL + NSEL, HALF_B, mr_pool)

            for q in range(QT):
                nc.sync.dma_start(d_wvals[q * 128:(q + 1) * 128, :],
                                  vout_sb[:, q * 2 * NSEL:(q + 1) * 2 * NSEL])
                nc.sync.dma_start(d_widx[q * 128:(q + 1) * 128, :],
                                  iout_sb[:, q * 2 * NSEL:(q + 1) * 2 * NSEL])

    nc.compile()
    return nc


def _build_f8w(nc):
    """Same structure as f16w, but fp8e4m3 DoubleRow matmuls: operands carry
    [partition, j(2), cols] APs; each matmul contracts 256 dims (2 k-groups
    of 128), so K=512 takes 2 matmuls per 512-wide output slice."""
    Max = mybir.AluOpType.max
    X = mybir.AxisListType.X
    F8 = mybir.dt.float8e4
    DR = mybir.MatmulPerfMode.DoubleRow

    d_x8 = nc.dram_tensor("x8", [D, B], F8, kind="ExternalInput")
    d_e8 = nc.dram_tensor("e8", [D, N_CORE], F8, kind="ExternalInput")
    d_wvals = nc.dram_tensor("wvals", [B, 2 * NSEL], F32, kind="ExternalOutput")
    d_widx = nc.dram_tensor("widx", [B, 2 * NSEL], U32, kind="ExternalOutput")

    chunks = [(i * BIGCHUNK, BIGCHUNK) for i in range(N_CORE // BIGCHUNK)]
    rem = N_CORE - (N_CORE // BIGCHUNK) * BIGCHUNK
    if rem:
        chunks.append((N_CORE - rem, rem))
    HALF_B = WPC - HALF_A

    def select(wq, vout, iout, o, width, mr_pool):
        nc.vector.max(vout[:, o:o + 8], wq)
        nc.vector.max_index(iout[:, o:o + 8], vout[:, o:o + 8], wq)
        mr = mr_pool.tile([128, width], F32, tag="mr")
        nc.vector.match_replace(mr[:, :width], vout[:, o:o + 8], wq, -1e30)
        nc.vector.max(vout[:, o + 8:o + 16], mr[:, :width])
        nc.vector.max_index(iout[:, o + 8:o + 16],
                            vout[:, o + 8:o + 16], mr[:, :width])

    with tile.TileContext(nc) as tc:
        with (
            tc.tile_pool(name="xpool", bufs=1) as xpool,
            tc.tile_pool(name="epool", bufs=3) as epool,
            tc.tile_pool(name="ps", bufs=3, space="PSUM") as ps_pool,
            tc.tile_pool(name="wacc", bufs=1) as wacc_pool,
            tc.tile_pool(name="mrp", bufs=4) as mr_pool,
            tc.tile_pool(name="outp", bufs=1) as out_pool,
        ):
            # [g][j][cols] layout: row-range g*256 + j*128 of the [D, *] input
            x_sb = xpool.tile([128, 4 * B], F8, tag="x8")
            for g in range(2):
                for j in range(2):
                    r0 = g * 256 + j * 128
                    nc.sync.dma_start(x_sb[:, (g * 2 + j) * B:(g * 2 + j + 1) * B],
                                      d_x8[r0:r0 + 128, :])

            wmax_sb = wacc_pool.tile([128, QT * WPC], F32, tag="wacc")
            vout_sb = out_pool.tile([128, QT * 2 * NSEL], F32, tag="vout")
            iout_sb = out_pool.tile([128, QT * 2 * NSEL], U32, tag="iout")

            for ci, (c0, cw) in enumerate(chunks):
                eh_sb = epool.tile([128, 4 * BIGCHUNK], F8, tag="e8")
                for g in range(2):
                    for j in range(2):
                        r0 = g * 256 + j * 128
                        nc.sync.dma_start(
                            eh_sb[:, (g * 2 + j) * cw:(g * 2 + j + 1) * cw],
                            d_e8[r0:r0 + 128, c0:c0 + cw])
                for q in range(QT):
                    ps = ps_pool.tile([128, BIGCHUNK], F32, tag="ps")
                    for s in range(cw // 512):
                        for g in range(2):
                            lhsT = x_sb[:, g * 2 * B:(g + 1) * 2 * B].rearrange(
                                "p (j b) -> p j b", j=2)[:, :, q * 128:(q + 1) * 128]
                            rhs = eh_sb[:, g * 2 * cw:(g + 1) * 2 * cw].rearrange(
                                "p (j n) -> p j n", j=2)[:, :, s * 512:(s + 1) * 512]
                            nc.tensor.matmul(ps[:, s * 512:(s + 1) * 512],
                                             lhsT, rhs, perf_mode=DR,
                                             start=(g == 0), stop=(g == 1))
                    nwin = cw // WWIN
                    wslot = q * WPC + c0 // WWIN
                    nc.vector.tensor_reduce(
                        wmax_sb[:, wslot:wslot + nwin],
                        ps[:, :cw].rearrange("p (w i) -> p w i", i=WWIN),
                        axis=X, op=Max)
                if ci >= 6:
                    n_grp = len(chunks) - 6
                    qs = [q for q in range(QT) if q % n_grp == ci - 6]
                    for q in qs:
                        select(wmax_sb[:, q * WPC:q * WPC + HALF_A],
                               vout_sb, iout_sb, q * 2 * NSEL, HALF_A, mr_pool)

            for q in range(QT):
                select(wmax_sb[:, q * WPC + HALF_A:(q + 1) * WPC],
                       vout_sb, iout_sb, q * 2 * NSEL + NSEL, HALF_B, mr_pool)

            for q in range(QT):
                nc.sync.dma_start(d_wvals[q * 128:(q + 1) * 128, :],
                                  vout_sb[:, q * 2 * NSEL:(q + 1) * 2 * NSEL])
                nc.sync.dma_start(d_widx[q * 128:(q + 1) * 128, :],
                                  iout_sb[:, q * 2 * NSEL:(q + 1) * 2 * NSEL])

    nc.compile()
    return nc


_F8_LUT = None


def _to_f8(a):
    """Fast float->fp8e4m3: fp16 hardware cast, then a 64K-entry LUT over the
    fp16 bit patterns (ml_dtypes' elementwise astype is ~50x slower). The
    double rounding vs a direct fp32->fp8 cast is harmless here: any
    consistent rounding is covered by the selection margin."""
    global _F8_LUT
    import ml_dtypes
    if _F8_LUT is None:
        with np.errstate(all="ignore"):
            all16 = np.arange(65536, dtype=np.uint16).view(np.float16)
            _F8_LUT = (all16.astype(np.float32)
                       .astype(ml_dtypes.float8_e4m3).view(np.uint8))
    h = a.astype(np.float16).view(np.uint16)
    return _F8_LUT[h].view(ml_dtypes.float8_e4m3)


def _build_f8d(nc):
    """f8w minus on-device window selection: the full per-window max array
    ships to the host (3.3MB/core), which does the margin selection itself.
    ScalarE stages PSUM->SBUF so the DVE reduce pays the SBUF (not PSUM)
    access bubble; DVE runs nothing but the 208 window-max reduces."""
    Max = mybir.AluOpType.max
    X = mybir.AxisListType.X
    F8 = mybir.dt.float8e4
    DR = mybir.MatmulPerfMode.DoubleRow
    Copy = mybir.ActivationFunctionType.Copy

    d_x8 = nc.dram_tensor("x8", [D, B], F8, kind="ExternalInput")
    d_e8 = nc.dram_tensor("e8", [D, N_CORE], F8, kind="ExternalInput")
    d_wmax = nc.dram_tensor("wmax", [B, WPC], F32, kind="ExternalOutput")

    chunks = [(i * BIGCHUNK, BIGCHUNK) for i in range(N_CORE // BIGCHUNK)]
    rem = N_CORE - (N_CORE // BIGCHUNK) * BIGCHUNK
    if rem:
        chunks.append((N_CORE - rem, rem))

    with tile.TileContext(nc) as tc:
        with (
            tc.tile_pool(name="xpool", bufs=1) as xpool,
            tc.tile_pool(name="epool", bufs=3) as epool,
            tc.tile_pool(name="ps", bufs=3, space="PSUM") as ps_pool,
            tc.tile_pool(name="stg", bufs=3) as stg_pool,
            tc.tile_pool(name="wacc", bufs=1) as wacc_pool,
        ):
            x_sb = xpool.tile([128, 4 * B], F8, tag="x8")
            for g in range(2):
                for j in range(2):
                    r0 = g * 256 + j * 128
                    nc.sync.dma_start(x_sb[:, (g * 2 + j) * B:(g * 2 + j + 1) * B],
                                      d_x8[r0:r0 + 128, :])

            wmax_sb = wacc_pool.tile([128, QT * WPC], F32, tag="wacc")

            for (c0, cw) in chunks:
                eh_sb = epool.tile([128, 4 * BIGCHUNK], F8, tag="e8")
                for g in range(2):
                    for j in range(2):
                        r0 = g * 256 + j * 128
                        nc.sync.dma_start(
                            eh_sb[:, (g * 2 + j) * cw:(g * 2 + j + 1) * cw],
                            d_e8[r0:r0 + 128, c0:c0 + cw])
                for q in range(QT):
                    ps = ps_pool.tile([128, BIGCHUNK], F32, tag="ps")
                    for s in range(cw // 512):
                        for g in range(2):
                            lhsT = x_sb[:, g * 2 * B:(g + 1) * 2 * B].rearrange(
                                "p (j b) -> p j b", j=2)[:, :, q * 128:(q + 1) * 128]
                            rhs = eh_sb[:, g * 2 * cw:(g + 1) * 2 * cw].rearrange(
                                "p (j n) -> p j n", j=2)[:, :, s * 512:(s + 1) * 512]
                            nc.tensor.matmul(ps[:, s * 512:(s + 1) * 512],
                                             lhsT, rhs, perf_mode=DR,
                                             start=(g == 0), stop=(g == 1))
                    stg = stg_pool.tile([128, BIGCHUNK], F32, tag="stg")
                    nc.scalar.activation(stg[:, :cw], ps[:, :cw], Copy)
                    nwin = cw // WWIN
                    wslot = q * WPC + c0 // WWIN
                    nc.vector.tensor_reduce(
                        wmax_sb[:, wslot:wslot + nwin],
                        stg[:, :cw].rearrange("p (w i) -> p w i", i=WWIN),
                        axis=X, op=Max)

            for q in range(QT):
                nc.sync.dma_start(d_wmax[q * 128:(q + 1) * 128, :],
                                  wmax_sb[:, q * WPC:(q + 1) * WPC])

    nc.compile()
    return nc


def _build_f8e(nc):
    """f8d with wider DVE reduces (two staged PSUM tiles -> one 2048-wide
    window-max, halving the per-op SBUF bubble count) and per-half early
    wmax DMA-out so the output transfer overlaps the main loop."""
    Max = mybir.AluOpType.max
    X = mybir.AxisListType.X
    F8 = mybir.dt.float8e4
    DR = mybir.MatmulPerfMode.DoubleRow
    Copy = mybir.ActivationFunctionType.Copy

    d_x8 = nc.dram_tensor("x8", [D, B], F8, kind="ExternalInput")
    d_e8 = nc.dram_tensor("e8", [D, N_CORE], F8, kind="ExternalInput")
    d_wmax = nc.dram_tensor("wmax", [B, WPC], F32, kind="ExternalOutput")

    BC = 2048  # 4 PSUM banks per tile; 6x2048 + 1x512 = 12800
    chunks = [(i * BC, BC) for i in range(N_CORE // BC)]
    rem = N_CORE - (N_CORE // BC) * BC
    if rem:
        chunks.append((N_CORE - rem, rem))
    AWIN = (4 * BC) // WWIN  # 256 windows (chunks 0-3) ship mid-loop

    with tile.TileContext(nc) as tc:
        with (
            tc.tile_pool(name="xpool", bufs=1) as xpool,
            tc.tile_pool(name="epool", bufs=3) as epool,
            tc.tile_pool(name="ps", bufs=2, space="PSUM") as ps_pool,
            tc.tile_pool(name="stg", bufs=3) as stg_pool,
            tc.tile_pool(name="wacc", bufs=1) as wacc_pool,
        ):
            x_sb = xpool.tile([128, 4 * B], F8, tag="x8")
            for g in range(2):
                for j in range(2):
                    r0 = g * 256 + j * 128
                    nc.sync.dma_start(x_sb[:, (g * 2 + j) * B:(g * 2 + j + 1) * B],
                                      d_x8[r0:r0 + 128, :])

            wmax_sb = wacc_pool.tile([128, QT * WPC], F32, tag="wacc")

            for ci, (c0, cw) in enumerate(chunks):
                eh_sb = epool.tile([128, 4 * BC], F8, tag="e8")
                for g in range(2):
                    for j in range(2):
                        r0 = g * 256 + j * 128
                        nc.sync.dma_start(
                            eh_sb[:, (g * 2 + j) * cw:(g * 2 + j + 1) * cw],
                            d_e8[r0:r0 + 128, c0:c0 + cw])
                for q in range(QT):
                    ps = ps_pool.tile([128, BC], F32, tag="ps")
                    for s in range(cw // 512):
                        for g in range(2):
                            lhsT = x_sb[:, g * 2 * B:(g + 1) * 2 * B].rearrange(
                                "p (j b) -> p j b", j=2)[:, :, q * 128:(q + 1) * 128]
                            rhs = eh_sb[:, g * 2 * cw:(g + 1) * 2 * cw].rearrange(
                                "p (j n) -> p j n", j=2)[:, :, s * 512:(s + 1) * 512]
                            nc.tensor.matmul(ps[:, s * 512:(s + 1) * 512],
                                             lhsT, rhs, perf_mode=DR,
                                             start=(g == 0), stop=(g == 1))
                    stg = stg_pool.tile([128, BC], F32, tag="stg")
                    nc.scalar.activation(stg[:, :cw], ps[:, :cw], Copy)
                    nwin = cw // WWIN
                    wslot = q * WPC + c0 // WWIN
                    nc.vector.tensor_reduce(
                        wmax_sb[:, wslot:wslot + nwin],
                        stg[:, :cw].rearrange("p (w i) -> p w i", i=WWIN),
                        axis=X, op=Max)
                    if ci == 3:  # chunks 0-3 reduced for q: ship 256 windows
                        nc.sync.dma_start(
                            d_wmax[q * 128:(q + 1) * 128, :AWIN],
                            wmax_sb[:, q * WPC:q * WPC + AWIN])

            for q in range(QT):
                nc.sync.dma_start(d_wmax[q * 128:(q + 1) * 128, AWIN:],
                                  wmax_sb[:, q * WPC + AWIN:(q + 1) * WPC])

    nc.compile()
    return nc


# f8t variant: pairwise-max fold tree over the sim tile, balanced across
# DVE + Pool + ScalarE so the fp8 matmuls (PE) are the bottleneck. Window
# width 4 (chunk cols {j, j+cw/4, j+cw/2, j+3cw/4}); full per-window max
# array ships to host for margin selection + exact rescore.
FT_BC = 2048                 # main chunk width (4 PSUM banks)
FT_WD = 124                  # fold1 pairs on DVE (direct fp32 PSUM)
FT_WP = 520                  # fold1 pairs on Pool (direct fp32 PSUM)
FT_WS = FT_BC // 2 - FT_WD - FT_WP   # fold1 pairs via ScalarE fp16 staging
FT_RD = 66                   # remainder-chunk fold1 pairs on DVE
FT_NW = (N_CORE // FT_BC) * (FT_BC // 4) + (N_CORE % FT_BC) // 4  # 3200


def _build_f8t(nc):
    Max = mybir.AluOpType.max
    F8 = mybir.dt.float8e4
    DR = mybir.MatmulPerfMode.DoubleRow
    Copy = mybir.ActivationFunctionType.Copy

    # [p, slot, cols] inputs: slot k holds rows 128k..128k+127 of the
    # transposed operand; slot order == (g, j) DoubleRow order.
    d_x8 = nc.dram_tensor("x8", [128, 4, B], F8, kind="ExternalInput")
    d_e8 = nc.dram_tensor("e8", [128, 4, N_CORE], F8, kind="ExternalInput")
    # [p, q, w]: query q*128+p, window w (host transposes back)
    d_wmax = nc.dram_tensor("wmax", [128, QT, FT_NW], F16, kind="ExternalOutput")

    chunks = [(i * FT_BC, FT_BC) for i in range(N_CORE // FT_BC)]
    rem = N_CORE - (N_CORE // FT_BC) * FT_BC
    if rem:
        chunks.append((N_CORE - rem, rem))

    with tile.TileContext(nc) as tc:
        with (
            tc.tile_pool(name="xpool", bufs=1) as xpool,
            tc.tile_pool(name="epool", bufs=3) as epool,
            tc.tile_pool(name="ps", bufs=2, space="PSUM") as ps_pool,
            tc.tile_pool(name="stg", bufs=3) as stg_pool,
            tc.tile_pool(name="s1p", bufs=3) as s1_pool,
            tc.tile_pool(name="wm", bufs=2) as wm_pool,
        ):
            x_sb = xpool.tile([128, 4 * B], F8, tag="x8")
            nc.sync.dma_start(
                x_sb[:, :].rearrange("p (k b) -> p k b", k=4), d_x8[:, :, :])

            wout = 0
            for ci, (c0, cw) in enumerate(chunks):
                half = cw // 2
                quar = cw // 4
                e_sb = epool.tile([128, 4 * cw], F8, tag="e8")
                nc.sync.dma_start(
                    e_sb[:, :].rearrange("p (k n) -> p k n", k=4),
                    d_e8[:, :, c0:c0 + cw])
                wm_sb = wm_pool.tile([128, QT * quar], F16, tag="wm")
                for q in range(QT):
                    ps = ps_pool.tile([128, cw], F32, tag="ps")
                    for s in range(cw // 512):
                        for g in range(2):
                            lhsT = x_sb[:, g * 2 * B:(g + 1) * 2 * B].rearrange(
                                "p (j b) -> p j b", j=2)[:, :, q * 128:(q + 1) * 128]
                            rhs = e_sb[:, g * 2 * cw:(g + 1) * 2 * cw].rearrange(
                                "p (j n) -> p j n", j=2)[:, :, s * 512:(s + 1) * 512]
                            nc.tensor.matmul(ps[:, s * 512:(s + 1) * 512],
                                             lhsT, rhs, perf_mode=DR,
                                             start=(g == 0), stop=(g == 1))
                    # fold1: s1[j] = max(ps[j], ps[j+half]), j in [0, half)
                    s1 = s1_pool.tile([128, half], F16, tag="s1")
                    if cw == FT_BC:
                        wd, wp, ws = FT_WD, FT_WP, FT_WS
                    else:
                        wd, wp, ws = FT_RD, half - FT_RD, 0
                    nc.vector.tensor_tensor(
                        s1[:, :wd], ps[:, :wd], ps[:, half:half + wd], op=Max)
                    nc.gpsimd.tensor_tensor(
                        s1[:, wd:wd + wp], ps[:, wd:wd + wp],
                        ps[:, half + wd:half + wd + wp], op=Max)
                    if ws:
                        stg = stg_pool.tile([128, 2 * ws], F16, tag="stg")
                        psv = ps[:, :].rearrange("p (h j) -> p h j", h=2)
                        nc.scalar.activation(
                            stg[:, :].rearrange("p (h j) -> p h j", h=2),
                            psv[:, :, wd + wp:half], Copy)
                        nc.vector.tensor_tensor(
                            s1[:, wd + wp:], stg[:, :ws], stg[:, ws:], op=Max)
                    # fold2: wm[w] = max(s1[w], s1[w+quar]) (fp16, 2x mode)
                    nc.vector.tensor_tensor(
                        wm_sb[:, q * quar:(q + 1) * quar],
                        s1[:, :quar], s1[:, quar:], op=Max)
                nc.sync.dma_start(
                    d_wmax[:, :, wout:wout + quar],
                    wm_sb[:, :].rearrange("p (q w) -> p q w", q=QT))
                wout += quar

    nc.compile()
    return nc


# f8v variant: per-iteration candidate range split into two half-chunks with
# SEPARATE PSUM tiles so each has exactly one reader engine (the tile sync
# compiler has one sem-wait slot per instruction; multiple reader engines on
# one tile get chained serially and stall the PE).
#  - DVE half: fold1 fp32->fp16 pairwise max + fp16 fold2 -> width-4 windows
#  - Pool half: single fp32 pairwise max -> width-2 windows (shipped as-is)
FV_HALF = 1024               # cols per half-chunk (2 PSUM banks)
FV_NWC = FV_HALF // 4 + FV_HALF // 2   # 768 window cols per full chunk
FV_NW = 6 * FV_NWC + (FV_NWC // 4)     # 4800 per core (incl 512-remainder)


def _build_f8v(nc):
    Max = mybir.AluOpType.max
    F8 = mybir.dt.float8e4
    DR = mybir.MatmulPerfMode.DoubleRow

    d_x8 = nc.dram_tensor("x8", [128, 4, B], F8, kind="ExternalInput")
    d_e8 = nc.dram_tensor("e8", [128, 4, N_CORE], F8, kind="ExternalInput")
    d_wmax = nc.dram_tensor("wmax", [128, QT, FV_NW], F16, kind="ExternalOutput")

    chunks = [(i * FT_BC, FT_BC) for i in range(N_CORE // FT_BC)]
    rem = N_CORE - (N_CORE // FT_BC) * FT_BC
    if rem:
        chunks.append((N_CORE - rem, rem))

    def mm(ps, x_sb, e_sb, cw, q, lo, hi):
        """fp8 DR matmuls for chunk cols [lo, hi) into ps[:, 0:hi-lo]."""
        for s0 in range(lo, hi, 512):
            sw = min(512, hi - s0)
            for g in range(2):
                lhsT = x_sb[:, g * 2 * B:(g + 1) * 2 * B].rearrange(
                    "p (j b) -> p j b", j=2)[:, :, q * 128:(q + 1) * 128]
                rhs = e_sb[:, g * 2 * cw:(g + 1) * 2 * cw].rearrange(
                    "p (j n) -> p j n", j=2)[:, :, s0:s0 + sw]
                nc.tensor.matmul(ps[:, s0 - lo:s0 - lo + sw], lhsT, rhs,
                                 perf_mode=DR, start=(g == 0), stop=(g == 1))

    with tile.TileContext(nc) as tc:
        with (
            tc.tile_pool(name="xpool", bufs=1) as xpool,
            tc.tile_pool(name="epool", bufs=3) as epool,
            tc.tile_pool(name="psd", bufs=2, space="PSUM") as psd_pool,
            tc.tile_pool(name="psp", bufs=2, space="PSUM") as psp_pool,
            tc.tile_pool(name="s1p", bufs=3) as s1_pool,
            tc.tile_pool(name="wm", bufs=2) as wm_pool,
        ):
            x_sb = xpool.tile([128, 4 * B], F8, tag="x8")
            nc.sync.dma_start(
                x_sb[:, :].rearrange("p (k b) -> p k b", k=4), d_x8[:, :, :])

            wout = 0
            for ci, (c0, cw) in enumerate(chunks):
                half = cw // 2          # cols per engine region
                hq = half // 2          # fold1 pair count per region
                w4 = half // 4          # width-4 window count (DVE region)
                nwc = w4 + half // 2    # window cols this chunk
                e_sb = epool.tile([128, 4 * cw], F8, tag="e8")
                nc.sync.dma_start(
                    e_sb[:, :].rearrange("p (k n) -> p k n", k=4),
                    d_e8[:, :, c0:c0 + cw])
                wm_sb = wm_pool.tile([128, QT * nwc], F16, tag="wm")
                for q in range(QT):
                    # DVE region: chunk cols [0, half)
                    ps_d = psd_pool.tile([128, half], F32, tag="psd")
                    mm(ps_d, x_sb, e_sb, cw, q, 0, half)
                    s1 = s1_pool.tile([128, hq], F16, tag="s1")
                    nc.vector.tensor_tensor(
                        s1[:, :], ps_d[:, :hq], ps_d[:, hq:], op=Max)
                    o = q * nwc
                    nc.vector.tensor_tensor(
                        wm_sb[:, o:o + w4], s1[:, :w4], s1[:, w4:], op=Max)
                    # Pool region: chunk cols [half, cw) -> width-2 windows
                    ps_p = psp_pool.tile([128, half], F32, tag="psp")
                    mm(ps_p, x_sb, e_sb, cw, q, half, cw)
                    nc.gpsimd.tensor_tensor(
                        wm_sb[:, o + w4:o + nwc], ps_p[:, :hq], ps_p[:, hq:],
                        op=Max)
                nc.sync.dma_start(
                    d_wmax[:, :, wout:wout + nwc],
                    wm_sb[:, :].rearrange("p (q w) -> p q w", q=QT))
                wout += nwc

    nc.compile()
    return nc


# f8m variant: hardware-legal consumption of the sim matrix. Real TRN2
# constraints (walrus verifier): an instruction reads at most ONE operand
# from PSUM; gpsimd (Pool) cannot run TensorTensor at all; DVE pool_max is
# rejected. Legal fast path: Act copies half the candidates PSUM->fp16 SBUF
# (0.83 ns/elem) while DVE retires a pair per cycle via
# max(psum_half, staged_half) (one PSUM operand). 1-bank PSUM tiles at
# depth 4 hide the mm->act->t_t chain; stg/wm are per-chunk tiles so WAR
# deps land on ancient instructions.
FM_NW = 6 * 1024 + 256       # width-2 window cols per core (6400)


def _build_f8m(nc):
    Max = mybir.AluOpType.max
    F8 = mybir.dt.float8e4
    DR = mybir.MatmulPerfMode.DoubleRow
    Copy = mybir.ActivationFunctionType.Copy

    d_x8 = nc.dram_tensor("x8", [128, 4, B], F8, kind="ExternalInput")
    d_e8 = nc.dram_tensor("e8", [128, 4, N_CORE], F8, kind="ExternalInput")
    d_wmax = nc.dram_tensor("wmax", [128, QT, FM_NW], F16, kind="ExternalOutput")

    chunks = [(i * FT_BC, FT_BC) for i in range(N_CORE // FT_BC)]
    rem = N_CORE - (N_CORE // FT_BC) * FT_BC
    if rem:
        chunks.append((N_CORE - rem, rem))

    def mm(ps, x_sb, e_sb, cw, q, lo, hi):
        for s0 in range(lo, hi, 512):
            sw = min(512, hi - s0)
            for g in range(2):
                lhsT = x_sb[:, g * 2 * B:(g + 1) * 2 * B].rearrange(
                    "p (j b) -> p j b", j=2)[:, :, q * 128:(q + 1) * 128]
                rhs = e_sb[:, g * 2 * cw:(g + 1) * 2 * cw].rearrange(
                    "p (j n) -> p j n", j=2)[:, :, s0:s0 + sw]
                nc.tensor.matmul(ps[:, s0 - lo:s0 - lo + sw], lhsT, rhs,
                                 perf_mode=DR, start=(g == 0), stop=(g == 1))

    with tile.TileContext(nc) as tc:
        with (
            tc.tile_pool(name="xpool", bufs=1) as xpool,
            tc.tile_pool(name="epool", bufs=3) as epool,
            tc.tile_pool(name="psd", bufs=4, space="PSUM") as psd_pool,
            tc.tile_pool(name="psa", bufs=4, space="PSUM") as psa_pool,
            tc.tile_pool(name="stg", bufs=2) as stg_pool,
            tc.tile_pool(name="wm", bufs=2) as wm_pool,
        ):
            x_sb = xpool.tile([128, 4 * B], F8, tag="x8")
            x_view = x_sb[:, :].rearrange("p (k b) -> p k b", k=4)

            wout = 0
            for ci, (c0, cw) in enumerate(chunks):
                nhalf = cw // 1024 if cw >= 1024 else 1
                hw_ = cw // nhalf        # cols per iteration (1024 or 512)
                pw = hw_ // 2            # pair count per iteration
                nwc = QT * nhalf * pw    # wm cols this chunk
                e_sb = epool.tile([128, 4 * cw], F8, tag="e8")
                e_view = e_sb[:, :].rearrange("p (k n) -> p k n", k=4)
                # chunk 0: piece-wise loads (first e8 piece, then x8 by
                # q-group) so the first matmuls start ~3us earlier
                if ci == 0:
                    # load order: x q-group 0 + e8 lower half (unblocks all
                    # of half 0 for q0-3 at once, keeping PE in-order), then
                    # the rest; e8 upper half last (needed ~16 iters later)
                    nc.sync.dma_start(x_view[:, :, 0:512], d_x8[:, :, 0:512])
                    nc.sync.dma_start(e_view[:, :, 0:hw_],
                                      d_e8[:, :, c0:c0 + hw_])
                    for xi in range(1, 4):
                        nc.sync.dma_start(
                            x_view[:, :, xi * 512:(xi + 1) * 512],
                            d_x8[:, :, xi * 512:(xi + 1) * 512])
                    nc.sync.dma_start(e_view[:, :, hw_:cw],
                                      d_e8[:, :, c0 + hw_:c0 + cw])
                else:
                    nc.sync.dma_start(e_view[:, :, :], d_e8[:, :, c0:c0 + cw])
                wm_sb = wm_pool.tile([128, nwc], F16, tag="wm")
                stg_sb = stg_pool.tile([128, QT * nhalf * pw], F16, tag="stg")
                ng = 4 if ci >= len(chunks) - 2 else 2
                qg = QT // ng
                # (h, q) order: all 16 queries on half 0 before half 1, so
                # chunk 0's later e8 pieces aren't needed until ~iteration 16
                for h in range(nhalf):
                    lo = h * hw_
                    hb = h * QT * pw     # this half's stg/wm column base
                    for q in range(QT):
                        ps_d = psd_pool.tile([128, pw], F32, tag="psd")
                        mm(ps_d, x_sb, e_sb, cw, q, lo, lo + pw)
                        ps_a = psa_pool.tile([128, pw], F32, tag="psa")
                        mm(ps_a, x_sb, e_sb, cw, q, lo + pw, lo + hw_)
                        st = stg_sb[:, hb + q * pw:hb + (q + 1) * pw]
                        nc.scalar.activation(st, ps_a[:, :], Copy)
                        nc.vector.tensor_tensor(
                            wm_sb[:, hb + q * pw:hb + (q + 1) * pw],
                            ps_d[:, :], st, op=Max)
                        # ship finished q-groups mid-chunk so the output
                        # transfer overlaps compute; finer groups on the
                        # last chunks to shrink the drain tail
                        if (q + 1) % qg == 0:
                            g0 = (q + 1) // qg - 1
                            nc.sync.dma_start(
                                d_wmax[:, g0 * qg:(g0 + 1) * qg,
                                       wout + h * pw:wout + (h + 1) * pw],
                                wm_sb[:, hb + g0 * qg * pw:hb + (g0 + 1) * qg * pw]
                                .rearrange("p (q w) -> p q w", q=qg))
                wout += nhalf * pw

    nc.compile()
    return nc


def _fm_members():
    """[FM_NW, 4] member map for f8m: all windows width-2 {c, c+pw}."""
    M = np.full((FM_NW, 4), N_EMB, np.int64)
    w0 = 0
    c0 = 0
    while c0 < N_CORE:
        cw = min(FT_BC, N_CORE - c0)
        nhalf = cw // 1024 if cw >= 1024 else 1
        pw = cw // nhalf // 2
        for h in range(nhalf):
            j = np.arange(pw)[:, None]
            M[w0:w0 + pw, :2] = c0 + h * 2 * pw + j + np.arange(2)[None, :] * pw
            w0 += pw
        c0 += cw
    assert w0 == FM_NW
    return M


def _prep_f8w(xn, e, inv):
    """in_maps for the f8w variant: fp8e4m3 transposed normalized shards,
    scaled by F8_SCALE to stay clear of the fp8 subnormal range."""
    import ml_dtypes
    f8 = ml_dtypes.float8_e4m3
    x8 = _to_f8(np.ascontiguousarray(xn.T) * np.float32(F8_SCALE))
    in_maps = []
    for i in range(CORES):
        lo_r, hi_r = i * N_CORE, (i + 1) * N_CORE
        n_real = max(0, min(hi_r, N_EMB) - lo_r)
        e8 = np.zeros((D, N_CORE), dtype=f8)
        if n_real > 0:
            sl = e[lo_r:lo_r + n_real] * (inv[lo_r:lo_r + n_real]
                                          * np.float32(F8_SCALE))[:, None]
            e8[:, :n_real] = _to_f8(sl.T)
        in_maps.append({"x8": x8, "e8": e8})
    return in_maps


def _prep_f8t(xn, e, inv):
    """f8t in_maps: fp8 shards in [p, slot, cols] layout (slot k = rows
    128k..128k+127 of the [D, *] transposed operand)."""
    import ml_dtypes
    f8 = ml_dtypes.float8_e4m3
    x8 = _to_f8(np.ascontiguousarray(xn.T) * np.float32(F8_SCALE))
    x8 = np.ascontiguousarray(x8.reshape(4, 128, B).transpose(1, 0, 2))
    in_maps = []
    for i in range(CORES):
        lo_r, hi_r = i * N_CORE, (i + 1) * N_CORE
        n_real = max(0, min(hi_r, N_EMB) - lo_r)
        e8 = np.zeros((D, N_CORE), dtype=f8)
        if n_real > 0:
            sl = e[lo_r:lo_r + n_real] * (inv[lo_r:lo_r + n_real]
                                          * np.float32(F8_SCALE))[:, None]
            e8[:, :n_real] = _to_f8(sl.T)
        e8 = np.ascontiguousarray(e8.reshape(4, 128, N_CORE).transpose(1, 0, 2))
        in_maps.append({"x8": x8, "e8": e8})
    return in_maps


def _f8t_members():
    """[FT_NW, 4] member map: core-relative candidate ids of each window."""
    M = np.empty((FT_NW, 4), np.int64)
    w0 = 0
    c0 = 0
    while c0 < N_CORE:
        cw = min(FT_BC, N_CORE - c0)
        quar = cw // 4
        M[w0:w0 + quar] = (c0 + np.arange(quar)[:, None]
                           + np.arange(4)[None, :] * quar)
        w0 += quar
        c0 += cw
    assert w0 == FT_NW
    return M


def _fv_members():
    """[FV_NW, 4] member map for f8v; width-2 windows pad with N_EMB (which
    the merge masks to -inf)."""
    M = np.full((FV_NW, 4), N_EMB, np.int64)
    w0 = 0
    c0 = 0
    while c0 < N_CORE:
        cw = min(FT_BC, N_CORE - c0)
        half, hq, w4 = cw // 2, cw // 4, cw // 8
        j = np.arange(w4)[:, None]
        M[w0:w0 + w4] = c0 + j + np.arange(4)[None, :] * w4
        c = np.arange(hq)[:, None]
        M[w0 + w4:w0 + w4 + hq, :2] = c0 + half + c + np.arange(2)[None, :] * hq
        w0 += w4 + hq
        c0 += cw
    assert w0 == FV_NW
    return M


def _merge_fv(results, labels, xn, e, inv, margin, nw, members):
    """Margin-select windows from per-window maxes, exact rescore of each
    kept window's members, exact top-10 + mode. Works for any window->member
    map `members` [nw, 4] (pad slots with ids >= N_EMB)."""
    wv = np.concatenate(
        [np.asarray(r["wmax"]).transpose(1, 0, 2).reshape(B, nw)
         for r in results], axis=1).astype(np.float32)       # [B, 8*nw]
    tot = wv.shape[1]
    w10 = np.partition(wv, tot - K_NEIGH, axis=1)[:, tot - K_NEIGH]
    keep = wv >= (w10[:, None] - margin)                     # [B, 8*nw]

    rows_idx, wins = np.nonzero(keep)        # wins: global window ids
    slots = (np.cumsum(keep, axis=1) - 1)[rows_idx, wins]
    smax = int(keep.sum(axis=1).max())

    e = np.asarray(e, dtype=np.float32)
    xn32 = np.ascontiguousarray(xn, dtype=np.float32)

    # exact rescore, vectorized over (kept window, member) pairs
    mem4 = (wins // nw)[:, None] * N_CORE + members[wins % nw]   # [K, 4]
    valid = mem4 < N_EMB
    pr = np.broadcast_to(rows_idx[:, None], mem4.shape)[valid]
    ps_ = np.broadcast_to(slots[:, None], mem4.shape)[valid]
    pk = np.broadcast_to(np.arange(4)[None, :], mem4.shape)[valid]
    pc = mem4[valid]

    sims = np.full((B, smax, 4), -np.inf, dtype=np.float32)
    wfull = np.zeros((B, smax), dtype=np.int64)
    wfull[rows_idx, slots] = wins
    CH = 1 << 19
    for o in range(0, len(pc), CH):
        r, c = pr[o:o + CH], pc[o:o + CH]
        s = np.einsum("ij,ij->i", xn32[r], e[c], optimize=True) * inv[c]
        sims[r, ps_[o:o + CH], pk[o:o + CH]] = s

    cores_f = wfull // nw
    cand = np.minimum(cores_f[:, :, None] * N_CORE + members[wfull % nw],
                      N_EMB).reshape(B, -1)
    sims = sims.reshape(B, -1)
    u = sims.view(np.uint32)
    mono = np.where(u & 0x80000000, ~u, u | 0x80000000).astype(np.uint64)
    combo = ((np.uint64(0xFFFFFFFF) - mono) << np.uint64(17)) | \
        cand.astype(np.uint64)
    combo[sims == -np.inf] = np.uint64(0xFFFFFFFFFFFFFFFF)
    ordr = np.argsort(combo, axis=1, kind="stable")[:, :K_NEIGH]
    neighbors = np.take_along_axis(cand, ordr, axis=1)
    return _mode_pred(neighbors, labels)


def _merge_f8t(results, labels, xn, e, inv, margin):
    """Margin-select windows from per-window maxes, exact rescore of the
    4 members of each kept window, exact top-10 + mode."""
    # device output [p, q, w] -> rows q*128+p
    wv = np.concatenate(
        [np.asarray(r["wmax"]).transpose(1, 0, 2).reshape(B, FT_NW)
         for r in results], axis=1).astype(np.float32)       # [B, 8*FT_NW]
    nw = wv.shape[1]
    w10 = np.partition(wv, nw - K_NEIGH, axis=1)[:, nw - K_NEIGH]
    keep = wv >= (w10[:, None] - margin)                     # [B, 8*FT_NW]

    rows_idx, wins = np.nonzero(keep)        # wins: global window ids
    slots = (np.cumsum(keep, axis=1) - 1)[rows_idx, wins]
    smax = int(keep.sum(axis=1).max())

    M = _f8t_members()                       # [FT_NW, 4]
    e = np.asarray(e, dtype=np.float32)
    xn32 = np.ascontiguousarray(xn, dtype=np.float32)
    order = np.argsort(wins, kind="stable")
    rows_s, slots_s, wins_s = rows_idx[order], slots[order], wins[order]
    uniq, starts = np.unique(wins_s, return_index=True)
    bounds = np.append(starts, len(wins_s))

    sims = np.full((B, smax, 4), -np.inf, dtype=np.float32)
    wfull = np.zeros((B, smax), dtype=np.int64)
    wfull[rows_idx, slots] = wins
    for ui in range(len(uniq)):
        w = int(uniq[ui])
        core, wloc = divmod(w, FT_NW)
        mem = core * N_CORE + M[wloc]                        # [4] global ids
        valid = mem < N_EMB
        if not valid.any():
            continue
        mv = mem[valid]
        s0, s1 = bounds[ui], bounds[ui + 1]
        en_w = e[mv] * inv[mv][:, None]
        sblk = xn32[rows_s[s0:s1]] @ en_w.T                  # [nrows, <=4]
        sims[rows_s[s0:s1][:, None], slots_s[s0:s1][:, None],
             np.nonzero(valid)[0][None, :]] = sblk

    cores_f = wfull // FT_NW
    cand = (cores_f[:, :, None] * N_CORE + M[wfull % FT_NW]).reshape(B, -1)
    sims = sims.reshape(B, -1)
    u = sims.view(np.uint32)
    mono = np.where(u & 0x80000000, ~u, u | 0x80000000).astype(np.uint64)
    combo = ((np.uint64(0xFFFFFFFF) - mono) << np.uint64(17)) | \
        cand.astype(np.uint64)
    combo[sims == -np.inf] = np.uint64(0xFFFFFFFFFFFFFFFF)
    ordr = np.argsort(combo, axis=1, kind="stable")[:, :K_NEIGH]
    neighbors = np.take_along_axis(cand, ordr, axis=1)
    return _mode_pred(neighbors, labels)


def _get_nc(variant=None):
    variant = variant or MM_DTYPE
    if variant not in _CACHE:
        _CACHE[variant] = _build(variant)
    return _CACHE[variant]


def _normalize(x, embeddings):
    x = np.asarray(x, dtype=np.float32)
    e = np.asarray(embeddings, dtype=np.float32)
    xn = x / np.maximum(np.linalg.norm(x, axis=1, keepdims=True), EPS)
    inv = (1.0 / np.maximum(np.linalg.norm(e, axis=1), EPS)).astype(np.float32)
    return xn, e, inv


def _prep_f16w(xn, e, inv):
    """in_maps for the f16w variant: fp16 transposed normalized shards."""
    xh = np.ascontiguousarray(xn.T).astype(np.float16)
    in_maps = []
    for i in range(CORES):
        lo_r, hi_r = i * N_CORE, (i + 1) * N_CORE
        n_real = max(0, min(hi_r, N_EMB) - lo_r)
        eh = np.zeros((D, N_CORE), dtype=np.float16)
        if n_real > 0:
            sl = e[lo_r:lo_r + n_real] * inv[lo_r:lo_r + n_real][:, None]
            eh[:, :n_real] = sl.T.astype(np.float16)
        in_maps.append({"xh": xh, "eh": eh})
    return in_maps


def _prep_inputs(x, embeddings, variant):
    """Host prep: normalize embeddings, pad, transpose, shard; returns in_maps.

    Works per-core-shard to keep intermediates cache-sized."""
    if variant == "f16w":
        xn, e, inv = _normalize(x, embeddings)
        return _prep_f16w(xn, e, inv)
    if variant in ("f8t", "f8v", "f8m"):
        xn, e, inv = _normalize(x, embeddings)
        return _prep_f8t(xn, e, inv)
    if variant in ("f8w", "f8d", "f8e"):
        xn, e, inv = _normalize(x, embeddings)
        return _prep_f8w(xn, e, inv)
    x = np.asarray(x, dtype=np.float32)
    e = np.asarray(embeddings, dtype=np.float32)
    inv = (1.0 / np.maximum(np.linalg.norm(e, axis=1), EPS)).astype(np.float32)
    xt = np.ascontiguousarray(x.T)               # [D, B]

    in_maps = []
    for i in range(CORES):
        lo_r, hi_r = i * N_CORE, (i + 1) * N_CORE
        n_real = max(0, min(hi_r, N_EMB) - lo_r)
        ent = np.zeros((D, N_CORE), dtype=np.float32)
        if n_real > 0:
            sl = e[lo_r:lo_r + n_real]
            ent[:, :n_real] = sl.T * inv[lo_r:lo_r + n_real][None, :]
        if variant == "f16x3":
            ehi = ent.astype(np.float16)
            elo = (ent - ehi).astype(np.float16)
            in_maps.append({"ehi": ehi, "elo": elo})
        else:
            in_maps.append({"ent": ent})

    if variant == "f16x3":
        xhi = xt.astype(np.float16)
        xlo = (xt - xhi).astype(np.float16)
        for m in in_maps:
            m["xhi"] = xhi
            m["xlo"] = xlo
    else:
        for m in in_maps:
            m["xt"] = xt
    return in_maps


def _merge(results, labels):
    """Host merge: exact global top-10 from per-core per-chunk top-8 pools,
    then the reference's mode computation."""
    vals = np.concatenate([r["vals"] for r in results], axis=1)   # [B, 8*NOUT]
    idx8 = np.concatenate([r["idx"] for r in results], axis=1).astype(np.int64)

    col_base = (np.arange(NOUT, dtype=np.int64) // 8) * CHUNK      # chunk offset
    core_base = np.repeat(np.arange(CORES, dtype=np.int64) * N_CORE, NOUT)
    g = idx8 + np.tile(col_base, CORES)[None, :] + core_base[None, :]

    # padding rows (g >= N_EMB) are zero embeddings: exclude
    u = vals.view(np.uint32)
    key = np.where(u & 0x80000000, ~u, u | 0x80000000).astype(np.uint64)
    combo = ((np.uint64(0xFFFFFFFF) - key) << np.uint64(17)) | g.astype(np.uint64)
    combo[g >= N_EMB] = np.uint64(0xFFFFFFFFFFFFFFFF)
    order = np.argsort(combo, axis=1, kind="stable")[:, :K_NEIGH]
    neighbors = np.take_along_axis(g, order, axis=1)               # [B, 10]

    labels = np.asarray(labels)
    nl = labels[neighbors].astype(np.int64)                        # [B, 10]
    eq = nl[:, :, None] == nl[:, None, :]
    counts = eq.sum(-1)
    mkey = counts * (NUM_CLASSES + 1) + (NUM_CLASSES - nl)
    mi = np.argmax(mkey, axis=1)
    pred = np.take_along_axis(nl, mi[:, None], axis=1)[:, 0]
    return pred.astype(labels.dtype)


class _Runner:
    """Caches the shard_map-jitted executable across calls (mirrors
    bass2jax.run_bass_via_pjrt's multi-core path, which re-traces per call)."""

    def __init__(self, variant):
        import jax
        import concourse.mybir as mb
        from concourse import bass2jax
        from jax.experimental.shard_map import shard_map
        from jax.sharding import Mesh, PartitionSpec

        bass2jax.install_neuronx_cc_hook()
        self.jax = jax
        nc = _get_nc(variant)
        partition_name = (nc.partition_id_tensor.name
                          if nc.partition_id_tensor else None)
        in_names, out_names, out_avals, zeros = [], [], [], []
        for alloc in nc.m.functions[0].allocations:
            if not isinstance(alloc, mb.MemoryLocationSet):
                continue
            name = alloc.memorylocations[0].name
            if alloc.kind == "ExternalInput":
                if name != partition_name:
                    in_names.append(name)
            elif alloc.kind == "ExternalOutput":
                shape = tuple(alloc.tensor_shape)
                dtype = mb.dt.np(alloc.dtype)
                out_avals.append(jax.core.ShapedArray(shape, dtype))
                out_names.append(name)
                zeros.append(np.zeros((CORES * shape[0],) + shape[1:], dtype))
        self.in_names = list(in_names)
        self.out_names = out_names
        self.out_avals = out_avals
        self.zeros = zeros
        n_params = len(in_names)
        all_names = in_names + out_names
        if partition_name is not None:
            all_names = all_names + [partition_name]
        donate = tuple(range(n_params, n_params + len(out_names)))

        def _body(*args):
            operands = list(args)
            if partition_name is not None:
                operands.append(bass2jax.partition_id_tensor())
            outs = bass2jax._bass_exec_p.bind(
                *operands,
                out_avals=tuple(out_avals),
                in_names=tuple(all_names),
                out_names=tuple(out_names),
                lowering_input_output_aliases=(),
                sim_require_finite=True,
                sim_require_nnan=True,
                nc=nc,
            )
            return tuple(outs)

        devices = jax.devices()[:CORES]
        self.mesh = Mesh(np.asarray(devices), ("core",))
        self.pspec = PartitionSpec("core")
        in_specs = (self.pspec,) * (n_params + len(out_names))
        out_specs = (self.pspec,) * len(out_names)
        self.sharded = jax.jit(
            shard_map(_body, mesh=self.mesh, in_specs=in_specs,
                      out_specs=out_specs, check_rep=False),
            donate_argnums=donate, keep_unused=True,
        )

    def concat_inputs(self, in_maps):
        return [
            np.concatenate([np.asarray(m[name]) for m in in_maps], axis=0)
            for name in self.in_names
        ]

    def device_put(self, concat_in):
        from jax.sharding import NamedSharding
        sh = NamedSharding(self.mesh, self.pspec)
        return [self.jax.device_put(a, sh) for a in concat_in]

    def execute(self, concat_in):
        zeros = [np.zeros_like(z) for z in self.zeros]
        out_arrs = self.sharded(*concat_in, *zeros)
        return out_arrs

    def run(self, in_maps):
        out_arrs = self.execute(self.concat_inputs(in_maps))
        return [
            {
                name: np.asarray(out_arrs[i]).reshape(
                    CORES, *self.out_avals[i].shape)[c]
                for i, name in enumerate(self.out_names)
            }
            for c in range(CORES)
        ]


_RUNNERS = {}


def _get_runner(variant=None):
    variant = variant or MM_DTYPE
    if variant not in _RUNNERS:
        _RUNNERS[variant] = _Runner(variant)
    return _RUNNERS[variant]


def _mode_pred(neighbors, labels):
    """Reference's torch.mode semantics on gathered neighbor labels."""
    labels = np.asarray(labels)
    nl = labels[neighbors].astype(np.int64)                        # [B, 10]
    eq = nl[:, :, None] == nl[:, None, :]
    counts = eq.sum(-1)
    mkey = counts * (NUM_CLASSES + 1) + (NUM_CLASSES - nl)
    mi = np.argmax(mkey, axis=1)
    pred = np.take_along_axis(nl, mi[:, None], axis=1)[:, 0]
    return pred.astype(labels.dtype)


def _merge_f16w(results, labels, xn, e, inv, margin=MARGIN):
    """Select windows >= (10th-best window max) - margin, rescore those
    candidates exactly in fp64, exact global top-10, then mode."""
    wv = np.stack([r["wvals"] for r in results], axis=1)      # [B, 8, 32]
    wi = np.stack([r["widx"] for r in results], axis=1).astype(np.int64)
    wi[:, :, NSEL:] += HALF_A   # half-B indices are relative to its slice
    gw = wi + (np.arange(CORES, dtype=np.int64) * WPC)[None, :, None]
    wv = wv.reshape(B, CORES * 2 * NSEL)
    gw = gw.reshape(B, CORES * 2 * NSEL)

    w10 = np.partition(wv, wv.shape[1] - K_NEIGH, axis=1)[:, wv.shape[1] - K_NEIGH]
    keep = wv >= (w10[:, None] - margin)
    smax = int(keep.sum(axis=1).max())

    # top-smax windows per row by value; mask out ones below the cutoff
    order = np.argsort(-wv, axis=1, kind="stable")[:, :smax]
    sel_g = np.take_along_axis(gw, order, axis=1)              # [B, smax]
    sel_keep = np.take_along_axis(keep, order, axis=1)

    # rescore grouped by window: each window's embeddings are one contiguous
    # 32-row slice, shared by every query that selected it (~6400 windows
    # total vs ~170k (row, window) pairs -> tiny gathers, BLAS-sized GEMMs)
    e = np.asarray(e, dtype=np.float32)
    xn32 = np.ascontiguousarray(xn, dtype=np.float32)
    rows_idx, slots = np.nonzero(sel_keep)
    wins = sel_g[rows_idx, slots]
    order = np.argsort(wins, kind="stable")
    rows_idx, slots, wins = rows_idx[order], slots[order], wins[order]
    uniq, starts = np.unique(wins, return_index=True)
    bounds = np.append(starts, len(wins))

    sims = np.full((B, smax, WWIN), -np.inf, dtype=np.float32)
    for ui in range(len(uniq)):
        w = int(uniq[ui])
        c0, c1 = w * WWIN, min(w * WWIN + WWIN, N_EMB)
        if c1 <= c0:
            continue
        s0, s1 = bounds[ui], bounds[ui + 1]
        en_w = e[c0:c1] * inv[c0:c1][:, None]                  # [<=32, D]
        sblk = xn32[rows_idx[s0:s1]] @ en_w.T                  # [nrows, <=32]
        sims[rows_idx[s0:s1], slots[s0:s1], :c1 - c0] = sblk

    cand = (sel_g[:, :, None] * WWIN +
            np.arange(WWIN, dtype=np.int64)[None, None, :]).reshape(B, -1)
    sims = sims.reshape(B, -1)

    # exact top-10 by (-sim, cand) via an order-preserving uint64 key
    u = sims.view(np.uint32)
    mono = np.where(u & 0x80000000, ~u, u | 0x80000000).astype(np.uint64)
    combo = ((np.uint64(0xFFFFFFFF) - mono) << np.uint64(17)) | \
        cand.astype(np.uint64)
    combo[sims == -np.inf] = np.uint64(0xFFFFFFFFFFFFFFFF)
    ordr = np.argsort(combo, axis=1, kind="stable")[:, :K_NEIGH]
    neighbors = np.take_along_axis(cand, ordr, axis=1)
    return _mode_pred(neighbors, labels)


def _merge_f8d(results, labels, xn, e, inv, margin):
    """Host-side window selection from the full per-window-max arrays, then
    the window-grouped exact rescore."""
    wv = np.concatenate([r["wmax"] for r in results], axis=1)   # [B, 8*WPC]
    nw = wv.shape[1]
    w10 = np.partition(wv, nw - K_NEIGH, axis=1)[:, nw - K_NEIGH]
    keep = wv >= (w10[:, None] - margin)                        # [B, 8*WPC]

    rows_idx, wins = np.nonzero(keep)        # wins are global window ids
    slots = (np.cumsum(keep, axis=1) - 1)[rows_idx, wins]
    smax = int(keep.sum(axis=1).max())

    e = np.asarray(e, dtype=np.float32)
    xn32 = np.ascontiguousarray(xn, dtype=np.float32)
    order = np.argsort(wins, kind="stable")
    rows_s, slots_s, wins_s = rows_idx[order], slots[order], wins[order]
    uniq, starts = np.unique(wins_s, return_index=True)
    bounds = np.append(starts, len(wins_s))

    sims = np.full((B, smax, WWIN), -np.inf, dtype=np.float32)
    wfull = np.zeros((B, smax), dtype=np.int64)
    wfull[rows_idx, slots] = wins
    for ui in range(len(uniq)):
        w = int(uniq[ui])
        c0, c1 = w * WWIN, min(w * WWIN + WWIN, N_EMB)
        if c1 <= c0:
            continue
        s0, s1 = bounds[ui], bounds[ui + 1]
        en_w = e[c0:c1] * inv[c0:c1][:, None]
        sblk = xn32[rows_s[s0:s1]] @ en_w.T
        sims[rows_s[s0:s1], slots_s[s0:s1], :c1 - c0] = sblk

    cand = (wfull[:, :, None] * WWIN +
            np.arange(WWIN, dtype=np.int64)[None, None, :]).reshape(B, -1)
    sims = sims.reshape(B, -1)
    u = sims.view(np.uint32)
    mono = np.where(u & 0x80000000, ~u, u | 0x80000000).astype(np.uint64)
    combo = ((np.uint64(0xFFFFFFFF) - mono) << np.uint64(17)) | \
        cand.astype(np.uint64)
    combo[sims == -np.inf] = np.uint64(0xFFFFFFFFFFFFFFFF)
    ordr = np.argsort(combo, axis=1, kind="stable")[:, :K_NEIGH]
    neighbors = np.take_along_axis(cand, ordr, axis=1)
    return _mode_pred(neighbors, labels)


def run_on_hw(x, embeddings, variant=None):
    runner = _get_runner(variant)
    in_maps = _prep_inputs(x, embeddings, variant or MM_DTYPE)
    return runner.run(in_maps)


def kernel(x, embeddings, labels):
    variant = MM_DTYPE
    if variant == "f16w":
        xn, e, inv = _normalize(x, embeddings)
        runner = _get_runner(variant)
        results = runner.run(_prep_f16w(xn, e, inv))
        return _merge_f16w(results, labels, xn, e, inv)
    if variant == "f8w":
        xn, e, inv = _normalize(x, embeddings)
        runner = _get_runner(variant)
        results = runner.run(_prep_f8w(xn, e, inv))
        return _merge_f16w(results, labels, xn, e, inv,
                           margin=MARGIN_F8 * F8_SCALE * F8_SCALE)
    if variant == "f8t":
        xn, e, inv = _normalize(x, embeddings)
        runner = _get_runner(variant)
        results = runner.run(_prep_f8t(xn, e, inv))
        # margin: fp8 sim error (scaled) + 2x fp16 rounding of the maxes
        return _merge_f8t(results, labels, xn, e, inv,
                          margin=MARGIN_F8 * F8_SCALE * F8_SCALE + 0.5)
    if variant == "f8v":
        xn, e, inv = _normalize(x, embeddings)
        runner = _get_runner(variant)
        results = runner.run(_prep_f8t(xn, e, inv))
        return _merge_fv(results, labels, xn, e, inv,
                         MARGIN_F8 * F8_SCALE * F8_SCALE + 0.5,
                         FV_NW, _fv_members())
    if variant == "f8m":
        xn, e, inv = _normalize(x, embeddings)
        runner = _get_runner(variant)
        results = runner.run(_prep_f8t(xn, e, inv))
        return _merge_fv(results, labels, xn, e, inv,
                         MARGIN_F8 * F8_SCALE * F8_SCALE + 0.5,
                         FM_NW, _fm_members())
    if variant in ("f8d", "f8e"):
        xn, e, inv = _normalize(x, embeddings)
        runner = _get_runner(variant)
        results = runner.run(_prep_f8w(xn, e, inv))
        return _merge_f8d(results, labels, xn, e, inv,
                          margin=MARGIN_F8 * F8_SCALE * F8_SCALE)
    results = run_on_hw(x, embeddings)
    return _merge(results, labels)



# revision 29
# speedup vs baseline: 1.6469x; 1.0060x over previous
"""Trainium2 Bass kernel for BaselineKNNModel (cosine-sim KNN classifier).

Contract: kernel(**inputs) takes FULL inputs (x [2048,512] f32,
embeddings [100000,512] f32, labels [100000] int) and returns the FULL
output (pred [2048] labels.dtype), distributing work across 8 NeuronCores.

Strategy (database-parallel, per sharding hint; active variant "f8m"):
 - Host: normalize embeddings (cosine denominator), pad N 100000->102400,
   shard along N across 8 cores (12800 each), quantize to fp8e4m3 (scaled
   by 16 to clear the subnormal range). x normalization is skipped:
   per-query positive scaling cannot change that query's top-k ranking.
 - Device (SPMD, per core): fp8 DoubleRow matmuls (K=256 per pass) produce
   sim tiles in PSUM. TRN2 allows at most one PSUM operand per instruction
   and gpsimd has no PSUM access, so the sim matrix is consumed by the two
   engines that can touch it: ScalarE copies half the candidates
   PSUM->fp16 SBUF while DVE retires a width-2 window per cycle via
   max(psum_half, staged_half). 1-bank PSUM tiles at depth 4 hide the
   mm->act->t_t dependency chain (the tile sync compiler has a single
   sem-wait slot per instruction, so chained deps otherwise stall PE).
   Per-window fp16 maxes (102400/2 per query per core) DMA out in
   q-groups overlapped with compute.
 - Host: margin selection (keep windows within fp8+fp16 error margin of
   the 10th-best window max), exact fp32 rescore of kept windows' members,
   exact top-10 by (value desc, index asc) = jax.lax.top_k tie order, then
   the reference's mode computation.
"""
import sys

for _p in ("/opt/trn_rl_repo", "/root/.axon_site/_ro/trn_rl_repo"):
    if _p not in sys.path:
        sys.path.insert(0, _p)

import numpy as np

import concourse.bacc as bacc
import concourse.mybir as mybir
import concourse.tile as tile
from concourse import bass_utils

F32 = mybir.dt.float32
F32R = mybir.dt.float32r
F16 = mybir.dt.float16
U32 = mybir.dt.uint32
Copy = mybir.ActivationFunctionType.Copy

B = 2048            # queries
D = 512             # embedding dim
N_EMB = 100000      # database size
K_NEIGH = 10
NUM_CLASSES = 1000
EPS = 1e-8

CORES = 8
N_PAD = 102400      # padded database size (8 * 12800)
N_CORE = N_PAD // CORES     # 12800 candidates per core
CHUNK = 512                 # candidates per sim tile (one PSUM bank)
NCHUNK = N_CORE // CHUNK    # 25
QT = B // 128               # 16 query tiles
KT = D // 128               # 4 k-tiles
NOUT = NCHUNK * 8           # 200 output slots per query per core

# f16w variant: window-max + device window top-16 + host exact rescore
WWIN = 32                   # candidates per window
WPC = N_CORE // WWIN        # 400 windows per core
BIGCHUNK = 1024             # candidates per PSUM tile (2 banks)
NSEL = 16                   # windows kept per (query, core, half)
HALF_A = (7 * BIGCHUNK) // WWIN  # windows in selection half A (224)
MARGIN = 4e-3               # fp16-sim error margin on unit-normalized sims
                            # (measured max |fp16 sim err| ~6e-5, ~60x safety)

# f8w variant: same as f16w but fp8e4m3 DoubleRow matmuls (2 fp8 weights per
# PE cell, K=256 per matmul). Inputs are scaled by F8_SCALE before rounding
# to fp8, so device sims (and window maxes) are scaled by F8_SCALE^2.
F8_SCALE = 16.0
MARGIN_F8 = 2.5e-2          # fp8 margin on unit-normalized sims
                            # (measured max err 7.1e-3 on a sample, rms 1.6e-3)

MM_DTYPE = "f8m"   # "f32"|"f32r"|"f16x3"|"f16w"|"f8w"|"f8d"|"f8e"|"f8t"|"f8v"|"f8m"

_CACHE = {}


def _build(variant):
    """Build + compile the per-core Bass program. Same program on all cores;
    only the `ent*` input shards differ."""
    nc = bacc.Bacc("TRN2", target_bir_lowering=False, debug=False)

    if variant == "noop":  # minimal program for RPC-overhead baselining
        d_nin = nc.dram_tensor("nin", [128, 128], F32, kind="ExternalInput")
        d_nout = nc.dram_tensor("nout", [128, 128], F32, kind="ExternalOutput")
        with tile.TileContext(nc) as tc:
            with tc.tile_pool(name="np0", bufs=1) as pool:
                t = pool.tile([128, 128], F32, tag="t")
                nc.sync.dma_start(t[:, :], d_nin[:, :])
                nc.sync.dma_start(d_nout[:, :], t[:, :])
        nc.compile()
        return nc

    if variant == "f16w":
        return _build_f16w(nc)
    if variant == "f8w":
        return _build_f8w(nc)
    if variant == "f8d":
        return _build_f8d(nc)
    if variant == "f8e":
        return _build_f8e(nc)
    if variant == "f8t":
        return _build_f8t(nc)
    if variant == "f8v":
        return _build_f8v(nc)
    if variant == "f8m":
        return _build_f8m(nc)

    f16 = variant == "f16x3"
    if f16:
        d_xhi = nc.dram_tensor("xhi", [D, B], F16, kind="ExternalInput")
        d_xlo = nc.dram_tensor("xlo", [D, B], F16, kind="ExternalInput")
        d_ehi = nc.dram_tensor("ehi", [D, N_CORE], F16, kind="ExternalInput")
        d_elo = nc.dram_tensor("elo", [D, N_CORE], F16, kind="ExternalInput")
    else:
        in_dt = F32R if variant == "f32r" else F32
        d_xt = nc.dram_tensor("xt", [D, B], in_dt, kind="ExternalInput")
        d_ent = nc.dram_tensor("ent", [D, N_CORE], in_dt, kind="ExternalInput")

    d_vals = nc.dram_tensor("vals", [B, NOUT], F32, kind="ExternalOutput")
    d_idx = nc.dram_tensor("idx", [B, NOUT], U32, kind="ExternalOutput")

    with tile.TileContext(nc) as tc:
        with (
            tc.tile_pool(name="xpool", bufs=1) as xpool,
            tc.tile_pool(name="epool", bufs=3) as epool,
            tc.tile_pool(name="ps", bufs=6, space="PSUM") as ps_pool,
            tc.tile_pool(name="sim", bufs=6) as sim_pool,
            tc.tile_pool(name="acc", bufs=1) as acc_pool,
        ):
            # resident x (stationary operand), k-tiles side by side
            if f16:
                xhi_sb = xpool.tile([128, KT * B], F16, tag="xhi")
                xlo_sb = xpool.tile([128, KT * B], F16, tag="xlo")
                for k in range(KT):
                    nc.sync.dma_start(xhi_sb[:, k * B:(k + 1) * B],
                                      d_xhi[k * 128:(k + 1) * 128, :])
                    nc.sync.dma_start(xlo_sb[:, k * B:(k + 1) * B],
                                      d_xlo[k * 128:(k + 1) * 128, :])
            else:
                xt_sb = xpool.tile([128, KT * B], in_dt, tag="xt")
                for k in range(KT):
                    nc.sync.dma_start(xt_sb[:, k * B:(k + 1) * B],
                                      d_xt[k * 128:(k + 1) * 128, :])

            # result accumulators, [128, QT*NOUT], column q*NOUT + c*8 + j
            vals_sb = acc_pool.tile([128, QT * NOUT], F32, tag="vacc")
            idx_sb = acc_pool.tile([128, QT * NOUT], U32, tag="iacc")

            for c in range(NCHUNK):
                c0 = c * CHUNK
                if f16:
                    ehi_sb = epool.tile([128, KT * CHUNK], F16, tag="ehi")
                    elo_sb = epool.tile([128, KT * CHUNK], F16, tag="elo")
                    for k in range(KT):
                        nc.sync.dma_start(ehi_sb[:, k * CHUNK:(k + 1) * CHUNK],
                                          d_ehi[k * 128:(k + 1) * 128, c0:c0 + CHUNK])
                        nc.sync.dma_start(elo_sb[:, k * CHUNK:(k + 1) * CHUNK],
                                          d_elo[k * 128:(k + 1) * 128, c0:c0 + CHUNK])
                else:
                    en_sb = epool.tile([128, KT * CHUNK], in_dt, tag="en")
                    for k in range(KT):
                        nc.sync.dma_start(en_sb[:, k * CHUNK:(k + 1) * CHUNK],
                                          d_ent[k * 128:(k + 1) * 128, c0:c0 + CHUNK])

                for q in range(QT):
                    ps = ps_pool.tile([128, CHUNK], F32, tag="ps")
                    if variant == "f16x3":
                        nmm = 3 * KT
                        i = 0
                        for k in range(KT):
                            xh = xhi_sb[:, k * B + q * 128: k * B + (q + 1) * 128]
                            xl = xlo_sb[:, k * B + q * 128: k * B + (q + 1) * 128]
                            eh = ehi_sb[:, k * CHUNK:(k + 1) * CHUNK]
                            el = elo_sb[:, k * CHUNK:(k + 1) * CHUNK]
                            for (a, bb) in ((xh, eh), (xh, el), (xl, eh)):
                                nc.tensor.matmul(ps[:, :], a, bb,
                                                 start=(i == 0), stop=(i == nmm - 1))
                                i += 1
                    else:
                        for k in range(KT):
                            lhsT = xt_sb[:, k * B + q * 128: k * B + (q + 1) * 128]
                            rhs = en_sb[:, k * CHUNK:(k + 1) * CHUNK]
                            nc.tensor.matmul(ps[:, :], lhsT, rhs,
                                             start=(k == 0), stop=(k == KT - 1))

                    sim = sim_pool.tile([128, CHUNK], F32, tag="sim")
                    nc.scalar.activation(sim[:, :], ps[:, :], Copy)

                    o = q * NOUT + c * 8
                    nc.vector.max(vals_sb[:, o:o + 8], sim[:, :])
                    nc.vector.max_index(idx_sb[:, o:o + 8], vals_sb[:, o:o + 8],
                                        sim[:, :])

            for q in range(QT):
                nc.sync.dma_start(d_vals[q * 128:(q + 1) * 128, :],
                                  vals_sb[:, q * NOUT:(q + 1) * NOUT])
                nc.sync.dma_start(d_idx[q * 128:(q + 1) * 128, :],
                                  idx_sb[:, q * NOUT:(q + 1) * NOUT])

    nc.compile()
    return nc


def _build_f16w(nc):
    """fp16 single-pass matmul; per-tile 16-wide window max (DVE reduce,
    PSUM-direct); per-core-half top-16 windows per query via
    max/match_replace (first half's selection overlaps the main loop);
    host rescores the selected windows exactly."""
    Max = mybir.AluOpType.max
    X = mybir.AxisListType.X

    d_xh = nc.dram_tensor("xh", [D, B], F16, kind="ExternalInput")
    d_eh = nc.dram_tensor("eh", [D, N_CORE], F16, kind="ExternalInput")
    d_wvals = nc.dram_tensor("wvals", [B, 2 * NSEL], F32, kind="ExternalOutput")
    d_widx = nc.dram_tensor("widx", [B, 2 * NSEL], U32, kind="ExternalOutput")

    # chunk layout: 12 x 1024 + 1 x 512 = 12800
    chunks = [(i * BIGCHUNK, BIGCHUNK) for i in range(N_CORE // BIGCHUNK)]
    rem = N_CORE - (N_CORE // BIGCHUNK) * BIGCHUNK
    if rem:
        chunks.append((N_CORE - rem, rem))
    # selection halves aligned to chunk boundaries:
    # half A = chunks 0-6 (448 windows), half B = chunks 7-12 (352 windows)
    HALF_B = WPC - HALF_A

    def select(wq, vout, iout, o, width, mr_pool):
        nc.vector.max(vout[:, o:o + 8], wq)
        nc.vector.max_index(iout[:, o:o + 8], vout[:, o:o + 8], wq)
        mr = mr_pool.tile([128, width], F32, tag="mr")
        nc.vector.match_replace(mr[:, :width], vout[:, o:o + 8], wq, -1e30)
        nc.vector.max(vout[:, o + 8:o + 16], mr[:, :width])
        nc.vector.max_index(iout[:, o + 8:o + 16],
                            vout[:, o + 8:o + 16], mr[:, :width])

    with tile.TileContext(nc) as tc:
        with (
            tc.tile_pool(name="xpool", bufs=1) as xpool,
            tc.tile_pool(name="epool", bufs=3) as epool,
            tc.tile_pool(name="ps", bufs=3, space="PSUM") as ps_pool,
            tc.tile_pool(name="wacc", bufs=1) as wacc_pool,
            tc.tile_pool(name="mrp", bufs=4) as mr_pool,
            tc.tile_pool(name="outp", bufs=1) as out_pool,
        ):
            xh_sb = xpool.tile([128, KT * B], F16, tag="xh")
            for k in range(KT):
                nc.sync.dma_start(xh_sb[:, k * B:(k + 1) * B],
                                  d_xh[k * 128:(k + 1) * 128, :])

            wmax_sb = wacc_pool.tile([128, QT * WPC], F32, tag="wacc")
            vout_sb = out_pool.tile([128, QT * 2 * NSEL], F32, tag="vout")
            iout_sb = out_pool.tile([128, QT * 2 * NSEL], U32, tag="iout")

            for ci, (c0, cw) in enumerate(chunks):
                eh_sb = epool.tile([128, KT * BIGCHUNK], F16, tag="eh")
                for k in range(KT):
                    nc.sync.dma_start(eh_sb[:, k * cw:(k + 1) * cw],
                                      d_eh[k * 128:(k + 1) * 128, c0:c0 + cw])
                for q in range(QT):
                    ps = ps_pool.tile([128, BIGCHUNK], F32, tag="ps")
                    for s in range(cw // 512):
                        for k in range(KT):
                            nc.tensor.matmul(
                                ps[:, s * 512:(s + 1) * 512],
                                xh_sb[:, k * B + q * 128: k * B + (q + 1) * 128],
                                eh_sb[:, k * cw + s * 512: k * cw + s * 512 + 512],
                                start=(k == 0), stop=(k == KT - 1))
                    nwin = cw // WWIN
                    wslot = q * WPC + c0 // WWIN
                    nc.vector.tensor_reduce(
                        wmax_sb[:, wslot:wslot + nwin],
                        ps[:, :cw].rearrange("p (w i) -> p w i", i=WWIN),
                        axis=X, op=Max)
                # half A (windows [0, HALF_A)) is complete after chunk 6;
                # spread its per-q selection over chunks 6..12 (2-3 q each)
                if ci >= 6:
                    n_grp = len(chunks) - 6
                    qs = [q for q in range(QT) if q % n_grp == ci - 6]
                    for q in qs:
                        select(wmax_sb[:, q * WPC:q * WPC + HALF_A],
                               vout_sb, iout_sb, q * 2 * NSEL, HALF_A, mr_pool)

            for q in range(QT):  # half B (windows [HALF_A, WPC))
                select(wmax_sb[:, q * WPC + HALF_A:(q + 1) * WPC],
                       vout_sb, iout_sb, q * 2 * NSEL + NSEL, HALF_B, mr_pool)

            for q in range(QT):
                nc.sync.dma_start(d_wvals[q * 128:(q + 1) * 128, :],
                                  vout_sb[:, q * 2 * NSEL:(q + 1) * 2 * NSEL])
                nc.sync.dma_start(d_widx[q * 128:(q + 1) * 128, :],
                                  iout_sb[:, q * 2 * NSEL:(q + 1) * 2 * NSEL])

    nc.compile()
    return nc


def _build_f8w(nc):
    """Same structure as f16w, but fp8e4m3 DoubleRow matmuls: operands carry
    [partition, j(2), cols] APs; each matmul contracts 256 dims (2 k-groups
    of 128), so K=512 takes 2 matmuls per 512-wide output slice."""
    Max = mybir.AluOpType.max
    X = mybir.AxisListType.X
    F8 = mybir.dt.float8e4
    DR = mybir.MatmulPerfMode.DoubleRow

    d_x8 = nc.dram_tensor("x8", [D, B], F8, kind="ExternalInput")
    d_e8 = nc.dram_tensor("e8", [D, N_CORE], F8, kind="ExternalInput")
    d_wvals = nc.dram_tensor("wvals", [B, 2 * NSEL], F32, kind="ExternalOutput")
    d_widx = nc.dram_tensor("widx", [B, 2 * NSEL], U32, kind="ExternalOutput")

    chunks = [(i * BIGCHUNK, BIGCHUNK) for i in range(N_CORE // BIGCHUNK)]
    rem = N_CORE - (N_CORE // BIGCHUNK) * BIGCHUNK
    if rem:
        chunks.append((N_CORE - rem, rem))
    HALF_B = WPC - HALF_A

    def select(wq, vout, iout, o, width, mr_pool):
        nc.vector.max(vout[:, o:o + 8], wq)
        nc.vector.max_index(iout[:, o:o + 8], vout[:, o:o + 8], wq)
        mr = mr_pool.tile([128, width], F32, tag="mr")
        nc.vector.match_replace(mr[:, :width], vout[:, o:o + 8], wq, -1e30)
        nc.vector.max(vout[:, o + 8:o + 16], mr[:, :width])
        nc.vector.max_index(iout[:, o + 8:o + 16],
                            vout[:, o + 8:o + 16], mr[:, :width])

    with tile.TileContext(nc) as tc:
        with (
            tc.tile_pool(name="xpool", bufs=1) as xpool,
            tc.tile_pool(name="epool", bufs=3) as epool,
            tc.tile_pool(name="ps", bufs=3, space="PSUM") as ps_pool,
            tc.tile_pool(name="wacc", bufs=1) as wacc_pool,
            tc.tile_pool(name="mrp", bufs=4) as mr_pool,
            tc.tile_pool(name="outp", bufs=1) as out_pool,
        ):
            # [g][j][cols] layout: row-range g*256 + j*128 of the [D, *] input
            x_sb = xpool.tile([128, 4 * B], F8, tag="x8")
            for g in range(2):
                for j in range(2):
                    r0 = g * 256 + j * 128
                    nc.sync.dma_start(x_sb[:, (g * 2 + j) * B:(g * 2 + j + 1) * B],
                                      d_x8[r0:r0 + 128, :])

            wmax_sb = wacc_pool.tile([128, QT * WPC], F32, tag="wacc")
            vout_sb = out_pool.tile([128, QT * 2 * NSEL], F32, tag="vout")
            iout_sb = out_pool.tile([128, QT * 2 * NSEL], U32, tag="iout")

            for ci, (c0, cw) in enumerate(chunks):
                eh_sb = epool.tile([128, 4 * BIGCHUNK], F8, tag="e8")
                for g in range(2):
                    for j in range(2):
                        r0 = g * 256 + j * 128
                        nc.sync.dma_start(
                            eh_sb[:, (g * 2 + j) * cw:(g * 2 + j + 1) * cw],
                            d_e8[r0:r0 + 128, c0:c0 + cw])
                for q in range(QT):
                    ps = ps_pool.tile([128, BIGCHUNK], F32, tag="ps")
                    for s in range(cw // 512):
                        for g in range(2):
                            lhsT = x_sb[:, g * 2 * B:(g + 1) * 2 * B].rearrange(
                                "p (j b) -> p j b", j=2)[:, :, q * 128:(q + 1) * 128]
                            rhs = eh_sb[:, g * 2 * cw:(g + 1) * 2 * cw].rearrange(
                                "p (j n) -> p j n", j=2)[:, :, s * 512:(s + 1) * 512]
                            nc.tensor.matmul(ps[:, s * 512:(s + 1) * 512],
                                             lhsT, rhs, perf_mode=DR,
                                             start=(g == 0), stop=(g == 1))
                    nwin = cw // WWIN
                    wslot = q * WPC + c0 // WWIN
                    nc.vector.tensor_reduce(
                        wmax_sb[:, wslot:wslot + nwin],
                        ps[:, :cw].rearrange("p (w i) -> p w i", i=WWIN),
                        axis=X, op=Max)
                if ci >= 6:
                    n_grp = len(chunks) - 6
                    qs = [q for q in range(QT) if q % n_grp == ci - 6]
                    for q in qs:
                        select(wmax_sb[:, q * WPC:q * WPC + HALF_A],
                               vout_sb, iout_sb, q * 2 * NSEL, HALF_A, mr_pool)

            for q in range(QT):
                select(wmax_sb[:, q * WPC + HALF_A:(q + 1) * WPC],
                       vout_sb, iout_sb, q * 2 * NSEL + NSEL, HALF_B, mr_pool)

            for q in range(QT):
                nc.sync.dma_start(d_wvals[q * 128:(q + 1) * 128, :],
                                  vout_sb[:, q * 2 * NSEL:(q + 1) * 2 * NSEL])
                nc.sync.dma_start(d_widx[q * 128:(q + 1) * 128, :],
                                  iout_sb[:, q * 2 * NSEL:(q + 1) * 2 * NSEL])

    nc.compile()
    return nc


_F8_LUT = None


def _to_f8(a):
    """Fast float->fp8e4m3: fp16 hardware cast, then a 64K-entry LUT over the
    fp16 bit patterns (ml_dtypes' elementwise astype is ~50x slower). The
    double rounding vs a direct fp32->fp8 cast is harmless here: any
    consistent rounding is covered by the selection margin."""
    global _F8_LUT
    import ml_dtypes
    if _F8_LUT is None:
        with np.errstate(all="ignore"):
            all16 = np.arange(65536, dtype=np.uint16).view(np.float16)
            _F8_LUT = (all16.astype(np.float32)
                       .astype(ml_dtypes.float8_e4m3).view(np.uint8))
    h = a.astype(np.float16).view(np.uint16)
    return _F8_LUT[h].view(ml_dtypes.float8_e4m3)


def _build_f8d(nc):
    """f8w minus on-device window selection: the full per-window max array
    ships to the host (3.3MB/core), which does the margin selection itself.
    ScalarE stages PSUM->SBUF so the DVE reduce pays the SBUF (not PSUM)
    access bubble; DVE runs nothing but the 208 window-max reduces."""
    Max = mybir.AluOpType.max
    X = mybir.AxisListType.X
    F8 = mybir.dt.float8e4
    DR = mybir.MatmulPerfMode.DoubleRow
    Copy = mybir.ActivationFunctionType.Copy

    d_x8 = nc.dram_tensor("x8", [D, B], F8, kind="ExternalInput")
    d_e8 = nc.dram_tensor("e8", [D, N_CORE], F8, kind="ExternalInput")
    d_wmax = nc.dram_tensor("wmax", [B, WPC], F32, kind="ExternalOutput")

    chunks = [(i * BIGCHUNK, BIGCHUNK) for i in range(N_CORE // BIGCHUNK)]
    rem = N_CORE - (N_CORE // BIGCHUNK) * BIGCHUNK
    if rem:
        chunks.append((N_CORE - rem, rem))

    with tile.TileContext(nc) as tc:
        with (
            tc.tile_pool(name="xpool", bufs=1) as xpool,
            tc.tile_pool(name="epool", bufs=3) as epool,
            tc.tile_pool(name="ps", bufs=3, space="PSUM") as ps_pool,
            tc.tile_pool(name="stg", bufs=3) as stg_pool,
            tc.tile_pool(name="wacc", bufs=1) as wacc_pool,
        ):
            x_sb = xpool.tile([128, 4 * B], F8, tag="x8")
            for g in range(2):
                for j in range(2):
                    r0 = g * 256 + j * 128
                    nc.sync.dma_start(x_sb[:, (g * 2 + j) * B:(g * 2 + j + 1) * B],
                                      d_x8[r0:r0 + 128, :])

            wmax_sb = wacc_pool.tile([128, QT * WPC], F32, tag="wacc")

            for (c0, cw) in chunks:
                eh_sb = epool.tile([128, 4 * BIGCHUNK], F8, tag="e8")
                for g in range(2):
                    for j in range(2):
                        r0 = g * 256 + j * 128
                        nc.sync.dma_start(
                            eh_sb[:, (g * 2 + j) * cw:(g * 2 + j + 1) * cw],
                            d_e8[r0:r0 + 128, c0:c0 + cw])
                for q in range(QT):
                    ps = ps_pool.tile([128, BIGCHUNK], F32, tag="ps")
                    for s in range(cw // 512):
                        for g in range(2):
                            lhsT = x_sb[:, g * 2 * B:(g + 1) * 2 * B].rearrange(
                                "p (j b) -> p j b", j=2)[:, :, q * 128:(q + 1) * 128]
                            rhs = eh_sb[:, g * 2 * cw:(g + 1) * 2 * cw].rearrange(
                                "p (j n) -> p j n", j=2)[:, :, s * 512:(s + 1) * 512]
                            nc.tensor.matmul(ps[:, s * 512:(s + 1) * 512],
                                             lhsT, rhs, perf_mode=DR,
                                             start=(g == 0), stop=(g == 1))
                    stg = stg_pool.tile([128, BIGCHUNK], F32, tag="stg")
                    nc.scalar.activation(stg[:, :cw], ps[:, :cw], Copy)
                    nwin = cw // WWIN
                    wslot = q * WPC + c0 // WWIN
                    nc.vector.tensor_reduce(
                        wmax_sb[:, wslot:wslot + nwin],
                        stg[:, :cw].rearrange("p (w i) -> p w i", i=WWIN),
                        axis=X, op=Max)

            for q in range(QT):
                nc.sync.dma_start(d_wmax[q * 128:(q + 1) * 128, :],
                                  wmax_sb[:, q * WPC:(q + 1) * WPC])

    nc.compile()
    return nc


def _build_f8e(nc):
    """f8d with wider DVE reduces (two staged PSUM tiles -> one 2048-wide
    window-max, halving the per-op SBUF bubble count) and per-half early
    wmax DMA-out so the output transfer overlaps the main loop."""
    Max = mybir.AluOpType.max
    X = mybir.AxisListType.X
    F8 = mybir.dt.float8e4
    DR = mybir.MatmulPerfMode.DoubleRow
    Copy = mybir.ActivationFunctionType.Copy

    d_x8 = nc.dram_tensor("x8", [D, B], F8, kind="ExternalInput")
    d_e8 = nc.dram_tensor("e8", [D, N_CORE], F8, kind="ExternalInput")
    d_wmax = nc.dram_tensor("wmax", [B, WPC], F32, kind="ExternalOutput")

    BC = 2048  # 4 PSUM banks per tile; 6x2048 + 1x512 = 12800
    chunks = [(i * BC, BC) for i in range(N_CORE // BC)]
    rem = N_CORE - (N_CORE // BC) * BC
    if rem:
        chunks.append((N_CORE - rem, rem))
    AWIN = (4 * BC) // WWIN  # 256 windows (chunks 0-3) ship mid-loop

    with tile.TileContext(nc) as tc:
        with (
            tc.tile_pool(name="xpool", bufs=1) as xpool,
            tc.tile_pool(name="epool", bufs=3) as epool,
            tc.tile_pool(name="ps", bufs=2, space="PSUM") as ps_pool,
            tc.tile_pool(name="stg", bufs=3) as stg_pool,
            tc.tile_pool(name="wacc", bufs=1) as wacc_pool,
        ):
            x_sb = xpool.tile([128, 4 * B], F8, tag="x8")
            for g in range(2):
                for j in range(2):
                    r0 = g * 256 + j * 128
                    nc.sync.dma_start(x_sb[:, (g * 2 + j) * B:(g * 2 + j + 1) * B],
                                      d_x8[r0:r0 + 128, :])

            wmax_sb = wacc_pool.tile([128, QT * WPC], F32, tag="wacc")

            for ci, (c0, cw) in enumerate(chunks):
                eh_sb = epool.tile([128, 4 * BC], F8, tag="e8")
                for g in range(2):
                    for j in range(2):
                        r0 = g * 256 + j * 128
                        nc.sync.dma_start(
                            eh_sb[:, (g * 2 + j) * cw:(g * 2 + j + 1) * cw],
                            d_e8[r0:r0 + 128, c0:c0 + cw])
                for q in range(QT):
                    ps = ps_pool.tile([128, BC], F32, tag="ps")
                    for s in range(cw // 512):
                        for g in range(2):
                            lhsT = x_sb[:, g * 2 * B:(g + 1) * 2 * B].rearrange(
                                "p (j b) -> p j b", j=2)[:, :, q * 128:(q + 1) * 128]
                            rhs = eh_sb[:, g * 2 * cw:(g + 1) * 2 * cw].rearrange(
                                "p (j n) -> p j n", j=2)[:, :, s * 512:(s + 1) * 512]
                            nc.tensor.matmul(ps[:, s * 512:(s + 1) * 512],
                                             lhsT, rhs, perf_mode=DR,
                                             start=(g == 0), stop=(g == 1))
                    stg = stg_pool.tile([128, BC], F32, tag="stg")
                    nc.scalar.activation(stg[:, :cw], ps[:, :cw], Copy)
                    nwin = cw // WWIN
                    wslot = q * WPC + c0 // WWIN
                    nc.vector.tensor_reduce(
                        wmax_sb[:, wslot:wslot + nwin],
                        stg[:, :cw].rearrange("p (w i) -> p w i", i=WWIN),
                        axis=X, op=Max)
                    if ci == 3:  # chunks 0-3 reduced for q: ship 256 windows
                        nc.sync.dma_start(
                            d_wmax[q * 128:(q + 1) * 128, :AWIN],
                            wmax_sb[:, q * WPC:q * WPC + AWIN])

            for q in range(QT):
                nc.sync.dma_start(d_wmax[q * 128:(q + 1) * 128, AWIN:],
                                  wmax_sb[:, q * WPC + AWIN:(q + 1) * WPC])

    nc.compile()
    return nc


# f8t variant: pairwise-max fold tree over the sim tile, balanced across
# DVE + Pool + ScalarE so the fp8 matmuls (PE) are the bottleneck. Window
# width 4 (chunk cols {j, j+cw/4, j+cw/2, j+3cw/4}); full per-window max
# array ships to host for margin selection + exact rescore.
FT_BC = 2048                 # main chunk width (4 PSUM banks)
FT_WD = 124                  # fold1 pairs on DVE (direct fp32 PSUM)
FT_WP = 520                  # fold1 pairs on Pool (direct fp32 PSUM)
FT_WS = FT_BC // 2 - FT_WD - FT_WP   # fold1 pairs via ScalarE fp16 staging
FT_RD = 66                   # remainder-chunk fold1 pairs on DVE
FT_NW = (N_CORE // FT_BC) * (FT_BC // 4) + (N_CORE % FT_BC) // 4  # 3200


def _build_f8t(nc):
    Max = mybir.AluOpType.max
    F8 = mybir.dt.float8e4
    DR = mybir.MatmulPerfMode.DoubleRow
    Copy = mybir.ActivationFunctionType.Copy

    # [p, slot, cols] inputs: slot k holds rows 128k..128k+127 of the
    # transposed operand; slot order == (g, j) DoubleRow order.
    d_x8 = nc.dram_tensor("x8", [128, 4, B], F8, kind="ExternalInput")
    d_e8 = nc.dram_tensor("e8", [128, 4, N_CORE], F8, kind="ExternalInput")
    # [p, q, w]: query q*128+p, window w (host transposes back)
    d_wmax = nc.dram_tensor("wmax", [128, QT, FT_NW], F16, kind="ExternalOutput")

    chunks = [(i * FT_BC, FT_BC) for i in range(N_CORE // FT_BC)]
    rem = N_CORE - (N_CORE // FT_BC) * FT_BC
    if rem:
        chunks.append((N_CORE - rem, rem))

    with tile.TileContext(nc) as tc:
        with (
            tc.tile_pool(name="xpool", bufs=1) as xpool,
            tc.tile_pool(name="epool", bufs=3) as epool,
            tc.tile_pool(name="ps", bufs=2, space="PSUM") as ps_pool,
            tc.tile_pool(name="stg", bufs=3) as stg_pool,
            tc.tile_pool(name="s1p", bufs=3) as s1_pool,
            tc.tile_pool(name="wm", bufs=2) as wm_pool,
        ):
            x_sb = xpool.tile([128, 4 * B], F8, tag="x8")
            nc.sync.dma_start(
                x_sb[:, :].rearrange("p (k b) -> p k b", k=4), d_x8[:, :, :])

            wout = 0
            for ci, (c0, cw) in enumerate(chunks):
                half = cw // 2
                quar = cw // 4
                e_sb = epool.tile([128, 4 * cw], F8, tag="e8")
                nc.sync.dma_start(
                    e_sb[:, :].rearrange("p (k n) -> p k n", k=4),
                    d_e8[:, :, c0:c0 + cw])
                wm_sb = wm_pool.tile([128, QT * quar], F16, tag="wm")
                for q in range(QT):
                    ps = ps_pool.tile([128, cw], F32, tag="ps")
                    for s in range(cw // 512):
                        for g in range(2):
                            lhsT = x_sb[:, g * 2 * B:(g + 1) * 2 * B].rearrange(
                                "p (j b) -> p j b", j=2)[:, :, q * 128:(q + 1) * 128]
                            rhs = e_sb[:, g * 2 * cw:(g + 1) * 2 * cw].rearrange(
                                "p (j n) -> p j n", j=2)[:, :, s * 512:(s + 1) * 512]
                            nc.tensor.matmul(ps[:, s * 512:(s + 1) * 512],
                                             lhsT, rhs, perf_mode=DR,
                                             start=(g == 0), stop=(g == 1))
                    # fold1: s1[j] = max(ps[j], ps[j+half]), j in [0, half)
                    s1 = s1_pool.tile([128, half], F16, tag="s1")
                    if cw == FT_BC:
                        wd, wp, ws = FT_WD, FT_WP, FT_WS
                    else:
                        wd, wp, ws = FT_RD, half - FT_RD, 0
                    nc.vector.tensor_tensor(
                        s1[:, :wd], ps[:, :wd], ps[:, half:half + wd], op=Max)
                    nc.gpsimd.tensor_tensor(
                        s1[:, wd:wd + wp], ps[:, wd:wd + wp],
                        ps[:, half + wd:half + wd + wp], op=Max)
                    if ws:
                        stg = stg_pool.tile([128, 2 * ws], F16, tag="stg")
                        psv = ps[:, :].rearrange("p (h j) -> p h j", h=2)
                        nc.scalar.activation(
                            stg[:, :].rearrange("p (h j) -> p h j", h=2),
                            psv[:, :, wd + wp:half], Copy)
                        nc.vector.tensor_tensor(
                            s1[:, wd + wp:], stg[:, :ws], stg[:, ws:], op=Max)
                    # fold2: wm[w] = max(s1[w], s1[w+quar]) (fp16, 2x mode)
                    nc.vector.tensor_tensor(
                        wm_sb[:, q * quar:(q + 1) * quar],
                        s1[:, :quar], s1[:, quar:], op=Max)
                nc.sync.dma_start(
                    d_wmax[:, :, wout:wout + quar],
                    wm_sb[:, :].rearrange("p (q w) -> p q w", q=QT))
                wout += quar

    nc.compile()
    return nc


# f8v variant: per-iteration candidate range split into two half-chunks with
# SEPARATE PSUM tiles so each has exactly one reader engine (the tile sync
# compiler has one sem-wait slot per instruction; multiple reader engines on
# one tile get chained serially and stall the PE).
#  - DVE half: fold1 fp32->fp16 pairwise max + fp16 fold2 -> width-4 windows
#  - Pool half: single fp32 pairwise max -> width-2 windows (shipped as-is)
FV_HALF = 1024               # cols per half-chunk (2 PSUM banks)
FV_NWC = FV_HALF // 4 + FV_HALF // 2   # 768 window cols per full chunk
FV_NW = 6 * FV_NWC + (FV_NWC // 4)     # 4800 per core (incl 512-remainder)


def _build_f8v(nc):
    Max = mybir.AluOpType.max
    F8 = mybir.dt.float8e4
    DR = mybir.MatmulPerfMode.DoubleRow

    d_x8 = nc.dram_tensor("x8", [128, 4, B], F8, kind="ExternalInput")
    d_e8 = nc.dram_tensor("e8", [128, 4, N_CORE], F8, kind="ExternalInput")
    d_wmax = nc.dram_tensor("wmax", [128, QT, FV_NW], F16, kind="ExternalOutput")

    chunks = [(i * FT_BC, FT_BC) for i in range(N_CORE // FT_BC)]
    rem = N_CORE - (N_CORE // FT_BC) * FT_BC
    if rem:
        chunks.append((N_CORE - rem, rem))

    def mm(ps, x_sb, e_sb, cw, q, lo, hi):
        """fp8 DR matmuls for chunk cols [lo, hi) into ps[:, 0:hi-lo]."""
        for s0 in range(lo, hi, 512):
            sw = min(512, hi - s0)
            for g in range(2):
                lhsT = x_sb[:, g * 2 * B:(g + 1) * 2 * B].rearrange(
                    "p (j b) -> p j b", j=2)[:, :, q * 128:(q + 1) * 128]
                rhs = e_sb[:, g * 2 * cw:(g + 1) * 2 * cw].rearrange(
                    "p (j n) -> p j n", j=2)[:, :, s0:s0 + sw]
                nc.tensor.matmul(ps[:, s0 - lo:s0 - lo + sw], lhsT, rhs,
                                 perf_mode=DR, start=(g == 0), stop=(g == 1))

    with tile.TileContext(nc) as tc:
        with (
            tc.tile_pool(name="xpool", bufs=1) as xpool,
            tc.tile_pool(name="epool", bufs=3) as epool,
            tc.tile_pool(name="psd", bufs=2, space="PSUM") as psd_pool,
            tc.tile_pool(name="psp", bufs=2, space="PSUM") as psp_pool,
            tc.tile_pool(name="s1p", bufs=3) as s1_pool,
            tc.tile_pool(name="wm", bufs=2) as wm_pool,
        ):
            x_sb = xpool.tile([128, 4 * B], F8, tag="x8")
            nc.sync.dma_start(
                x_sb[:, :].rearrange("p (k b) -> p k b", k=4), d_x8[:, :, :])

            wout = 0
            for ci, (c0, cw) in enumerate(chunks):
                half = cw // 2          # cols per engine region
                hq = half // 2          # fold1 pair count per region
                w4 = half // 4          # width-4 window count (DVE region)
                nwc = w4 + half // 2    # window cols this chunk
                e_sb = epool.tile([128, 4 * cw], F8, tag="e8")
                nc.sync.dma_start(
                    e_sb[:, :].rearrange("p (k n) -> p k n", k=4),
                    d_e8[:, :, c0:c0 + cw])
                wm_sb = wm_pool.tile([128, QT * nwc], F16, tag="wm")
                for q in range(QT):
                    # DVE region: chunk cols [0, half)
                    ps_d = psd_pool.tile([128, half], F32, tag="psd")
                    mm(ps_d, x_sb, e_sb, cw, q, 0, half)
                    s1 = s1_pool.tile([128, hq], F16, tag="s1")
                    nc.vector.tensor_tensor(
                        s1[:, :], ps_d[:, :hq], ps_d[:, hq:], op=Max)
                    o = q * nwc
                    nc.vector.tensor_tensor(
                        wm_sb[:, o:o + w4], s1[:, :w4], s1[:, w4:], op=Max)
                    # Pool region: chunk cols [half, cw) -> width-2 windows
                    ps_p = psp_pool.tile([128, half], F32, tag="psp")
                    mm(ps_p, x_sb, e_sb, cw, q, half, cw)
                    nc.gpsimd.tensor_tensor(
                        wm_sb[:, o + w4:o + nwc], ps_p[:, :hq], ps_p[:, hq:],
                        op=Max)
                nc.sync.dma_start(
                    d_wmax[:, :, wout:wout + nwc],
                    wm_sb[:, :].rearrange("p (q w) -> p q w", q=QT))
                wout += nwc

    nc.compile()
    return nc


# f8m variant: hardware-legal consumption of the sim matrix. Real TRN2
# constraints (walrus verifier): an instruction reads at most ONE operand
# from PSUM; gpsimd (Pool) cannot run TensorTensor at all; DVE pool_max is
# rejected. Legal fast path: Act copies half the candidates PSUM->fp16 SBUF
# (0.83 ns/elem) while DVE retires a pair per cycle via
# max(psum_half, staged_half) (one PSUM operand). 1-bank PSUM tiles at
# depth 4 hide the mm->act->t_t chain; stg/wm are per-chunk tiles so WAR
# deps land on ancient instructions.
FM_NW = 6 * 1024 + 256       # width-2 window cols per core (6400)


def _build_f8m(nc):
    Max = mybir.AluOpType.max
    F8 = mybir.dt.float8e4
    DR = mybir.MatmulPerfMode.DoubleRow
    Copy = mybir.ActivationFunctionType.Copy

    d_x8 = nc.dram_tensor("x8", [128, 4, B], F8, kind="ExternalInput")
    d_e8 = nc.dram_tensor("e8", [128, 4, N_CORE], F8, kind="ExternalInput")
    d_wmax = nc.dram_tensor("wmax", [128, QT, FM_NW], F16, kind="ExternalOutput")

    chunks = [(i * FT_BC, FT_BC) for i in range(N_CORE // FT_BC)]
    rem = N_CORE - (N_CORE // FT_BC) * FT_BC
    if rem:
        chunks.append((N_CORE - rem, rem))

    def mm(ps, x_sb, e_sb, cw, q, lo, hi):
        for s0 in range(lo, hi, 512):
            sw = min(512, hi - s0)
            for g in range(2):
                lhsT = x_sb[:, g * 2 * B:(g + 1) * 2 * B].rearrange(
                    "p (j b) -> p j b", j=2)[:, :, q * 128:(q + 1) * 128]
                rhs = e_sb[:, g * 2 * cw:(g + 1) * 2 * cw].rearrange(
                    "p (j n) -> p j n", j=2)[:, :, s0:s0 + sw]
                nc.tensor.matmul(ps[:, s0 - lo:s0 - lo + sw], lhsT, rhs,
                                 perf_mode=DR, start=(g == 0), stop=(g == 1))

    with tile.TileContext(nc) as tc:
        with (
            tc.tile_pool(name="xpool", bufs=1) as xpool,
            tc.tile_pool(name="epool", bufs=3) as epool,
            tc.tile_pool(name="psd", bufs=4, space="PSUM") as psd_pool,
            tc.tile_pool(name="psa", bufs=4, space="PSUM") as psa_pool,
            tc.tile_pool(name="stg", bufs=2) as stg_pool,
            tc.tile_pool(name="wm", bufs=2) as wm_pool,
        ):
            x_sb = xpool.tile([128, 4 * B], F8, tag="x8")
            x_view = x_sb[:, :].rearrange("p (k b) -> p k b", k=4)

            wout = 0
            for ci, (c0, cw) in enumerate(chunks):
                nhalf = cw // 1024 if cw >= 1024 else 1
                hw_ = cw // nhalf        # cols per iteration (1024 or 512)
                pw = hw_ // 2            # pair count per iteration
                nwc = QT * nhalf * pw    # wm cols this chunk
                e_sb = epool.tile([128, 4 * cw], F8, tag="e8")
                e_view = e_sb[:, :].rearrange("p (k n) -> p k n", k=4)
                # chunk 0: piece-wise loads (first e8 piece, then x8 by
                # q-group) so the first matmuls start ~3us earlier
                if ci == 0:
                    # load order: x q-group 0 + e8 lower half (unblocks all
                    # of half 0 for q0-3 at once, keeping PE in-order), then
                    # the rest; e8 upper half last (needed ~16 iters later)
                    nc.sync.dma_start(x_view[:, :, 0:512], d_x8[:, :, 0:512])
                    nc.sync.dma_start(e_view[:, :, 0:hw_],
                                      d_e8[:, :, c0:c0 + hw_])
                    for xi in range(1, 4):
                        nc.sync.dma_start(
                            x_view[:, :, xi * 512:(xi + 1) * 512],
                            d_x8[:, :, xi * 512:(xi + 1) * 512])
                    nc.sync.dma_start(e_view[:, :, hw_:cw],
                                      d_e8[:, :, c0 + hw_:c0 + cw])
                else:
                    nc.sync.dma_start(e_view[:, :, :], d_e8[:, :, c0:c0 + cw])
                wm_sb = wm_pool.tile([128, nwc], F16, tag="wm")
                stg_sb = stg_pool.tile([128, QT * nhalf * pw], F16, tag="stg")
                ng = 4 if ci >= len(chunks) - 2 else 2
                qg = QT // ng
                # (h, q) order: all 16 queries on half 0 before half 1, so
                # chunk 0's later e8 pieces aren't needed until ~iteration 16
                for h in range(nhalf):
                    lo = h * hw_
                    hb = h * QT * pw     # this half's stg/wm column base
                    for q in range(QT):
                        ps_d = psd_pool.tile([128, pw], F32, tag="psd")
                        mm(ps_d, x_sb, e_sb, cw, q, lo, lo + pw)
                        ps_a = psa_pool.tile([128, pw], F32, tag="psa")
                        mm(ps_a, x_sb, e_sb, cw, q, lo + pw, lo + hw_)
                        st = stg_sb[:, hb + q * pw:hb + (q + 1) * pw]
                        nc.scalar.activation(st, ps_a[:, :], Copy)
                        nc.vector.tensor_tensor(
                            wm_sb[:, hb + q * pw:hb + (q + 1) * pw],
                            ps_d[:, :], st, op=Max)
                        # ship finished q-groups mid-chunk so the output
                        # transfer overlaps compute; finer groups on the
                        # last chunks to shrink the drain tail
                        if (q + 1) % qg == 0:
                            g0 = (q + 1) // qg - 1
                            nc.sync.dma_start(
                                d_wmax[:, g0 * qg:(g0 + 1) * qg,
                                       wout + h * pw:wout + (h + 1) * pw],
                                wm_sb[:, hb + g0 * qg * pw:hb + (g0 + 1) * qg * pw]
                                .rearrange("p (q w) -> p q w", q=qg))
                wout += nhalf * pw

    nc.compile()
    return nc


def _fm_members():
    """[FM_NW, 4] member map for f8m: all windows width-2 {c, c+pw}."""
    M = np.full((FM_NW, 4), N_EMB, np.int64)
    w0 = 0
    c0 = 0
    while c0 < N_CORE:
        cw = min(FT_BC, N_CORE - c0)
        nhalf = cw // 1024 if cw >= 1024 else 1
        pw = cw // nhalf // 2
        for h in range(nhalf):
            j = np.arange(pw)[:, None]
            M[w0:w0 + pw, :2] = c0 + h * 2 * pw + j + np.arange(2)[None, :] * pw
            w0 += pw
        c0 += cw
    assert w0 == FM_NW
    return M


def _prep_f8w(xn, e, inv):
    """in_maps for the f8w variant: fp8e4m3 transposed normalized shards,
    scaled by F8_SCALE to stay clear of the fp8 subnormal range."""
    import ml_dtypes
    f8 = ml_dtypes.float8_e4m3
    x8 = _to_f8(np.ascontiguousarray(xn.T) * np.float32(F8_SCALE))
    in_maps = []
    for i in range(CORES):
        lo_r, hi_r = i * N_CORE, (i + 1) * N_CORE
        n_real = max(0, min(hi_r, N_EMB) - lo_r)
        e8 = np.zeros((D, N_CORE), dtype=f8)
        if n_real > 0:
            sl = e[lo_r:lo_r + n_real] * (inv[lo_r:lo_r + n_real]
                                          * np.float32(F8_SCALE))[:, None]
            e8[:, :n_real] = _to_f8(sl.T)
        in_maps.append({"x8": x8, "e8": e8})
    return in_maps


def _prep_f8t(xn, e, inv):
    """f8t in_maps: fp8 shards in [p, slot, cols] layout (slot k = rows
    128k..128k+127 of the [D, *] transposed operand)."""
    import ml_dtypes
    f8 = ml_dtypes.float8_e4m3
    x8 = _to_f8(np.ascontiguousarray(xn.T) * np.float32(F8_SCALE))
    x8 = np.ascontiguousarray(x8.reshape(4, 128, B).transpose(1, 0, 2))
    in_maps = []
    for i in range(CORES):
        lo_r, hi_r = i * N_CORE, (i + 1) * N_CORE
        n_real = max(0, min(hi_r, N_EMB) - lo_r)
        e8 = np.zeros((D, N_CORE), dtype=f8)
        if n_real > 0:
            sl = e[lo_r:lo_r + n_real] * (inv[lo_r:lo_r + n_real]
                                          * np.float32(F8_SCALE))[:, None]
            e8[:, :n_real] = _to_f8(sl.T)
        e8 = np.ascontiguousarray(e8.reshape(4, 128, N_CORE).transpose(1, 0, 2))
        in_maps.append({"x8": x8, "e8": e8})
    return in_maps


def _f8t_members():
    """[FT_NW, 4] member map: core-relative candidate ids of each window."""
    M = np.empty((FT_NW, 4), np.int64)
    w0 = 0
    c0 = 0
    while c0 < N_CORE:
        cw = min(FT_BC, N_CORE - c0)
        quar = cw // 4
        M[w0:w0 + quar] = (c0 + np.arange(quar)[:, None]
                           + np.arange(4)[None, :] * quar)
        w0 += quar
        c0 += cw
    assert w0 == FT_NW
    return M


def _fv_members():
    """[FV_NW, 4] member map for f8v; width-2 windows pad with N_EMB (which
    the merge masks to -inf)."""
    M = np.full((FV_NW, 4), N_EMB, np.int64)
    w0 = 0
    c0 = 0
    while c0 < N_CORE:
        cw = min(FT_BC, N_CORE - c0)
        half, hq, w4 = cw // 2, cw // 4, cw // 8
        j = np.arange(w4)[:, None]
        M[w0:w0 + w4] = c0 + j + np.arange(4)[None, :] * w4
        c = np.arange(hq)[:, None]
        M[w0 + w4:w0 + w4 + hq, :2] = c0 + half + c + np.arange(2)[None, :] * hq
        w0 += w4 + hq
        c0 += cw
    assert w0 == FV_NW
    return M


def _merge_fv(results, labels, xn, e, inv, margin, nw, members):
    """Margin-select windows from per-window maxes, exact rescore of each
    kept window's members, exact top-10 + mode. Works for any window->member
    map `members` [nw, 4] (pad slots with ids >= N_EMB)."""
    tot = CORES * nw
    wv = np.empty((B, tot), np.float32)                      # [B, 8*nw]
    for i, r in enumerate(results):
        src = np.asarray(r["wmax"]).astype(np.float32)       # [128, QT, nw]
        wv[:, i * nw:(i + 1) * nw] = src.transpose(1, 0, 2).reshape(B, nw)
    w10 = np.partition(wv, tot - K_NEIGH, axis=1)[:, tot - K_NEIGH]
    keep = wv >= (w10[:, None] - margin)                     # [B, 8*nw]

    rows_idx, wins = np.nonzero(keep)        # wins: global window ids
    counts = keep.sum(axis=1)
    starts = np.concatenate(([0], np.cumsum(counts[:-1])))
    slots = np.arange(len(wins)) - starts[rows_idx]          # index within row
    smax = int(counts.max())

    e = np.asarray(e, dtype=np.float32)
    xn32 = np.ascontiguousarray(xn, dtype=np.float32)

    # exact rescore, vectorized over (kept window, member) pairs
    mem4 = (wins // nw)[:, None] * N_CORE + members[wins % nw]   # [K, 4]
    valid = mem4 < N_EMB
    pr = np.broadcast_to(rows_idx[:, None], mem4.shape)[valid]
    ps_ = np.broadcast_to(slots[:, None], mem4.shape)[valid]
    pk = np.broadcast_to(np.arange(4)[None, :], mem4.shape)[valid]
    pc = mem4[valid]

    sims = np.full((B, smax, 4), -np.inf, dtype=np.float32)
    wfull = np.zeros((B, smax), dtype=np.int64)
    wfull[rows_idx, slots] = wins
    CH = 1 << 19
    for o in range(0, len(pc), CH):
        r, c = pr[o:o + CH], pc[o:o + CH]
        s = np.einsum("ij,ij->i", xn32[r], e[c], optimize=True) * inv[c]
        sims[r, ps_[o:o + CH], pk[o:o + CH]] = s

    cores_f = wfull // nw
    cand = np.minimum(cores_f[:, :, None] * N_CORE + members[wfull % nw],
                      N_EMB).reshape(B, -1)
    sims = sims.reshape(B, -1)
    u = sims.view(np.uint32)
    mono = np.where(u & 0x80000000, ~u, u | 0x80000000).astype(np.uint64)
    combo = ((np.uint64(0xFFFFFFFF) - mono) << np.uint64(17)) | \
        cand.astype(np.uint64)
    combo[sims == -np.inf] = np.uint64(0xFFFFFFFFFFFFFFFF)
    ordr = np.argsort(combo, axis=1, kind="stable")[:, :K_NEIGH]
    neighbors = np.take_along_axis(cand, ordr, axis=1)
    return _mode_pred(neighbors, labels)


def _merge_f8t(results, labels, xn, e, inv, margin):
    """Margin-select windows from per-window maxes, exact rescore of the
    4 members of each kept window, exact top-10 + mode."""
    # device output [p, q, w] -> rows q*128+p
    wv = np.concatenate(
        [np.asarray(r["wmax"]).transpose(1, 0, 2).reshape(B, FT_NW)
         for r in results], axis=1).astype(np.float32)       # [B, 8*FT_NW]
    nw = wv.shape[1]
    w10 = np.partition(wv, nw - K_NEIGH, axis=1)[:, nw - K_NEIGH]
    keep = wv >= (w10[:, None] - margin)                     # [B, 8*FT_NW]

    rows_idx, wins = np.nonzero(keep)        # wins: global window ids
    slots = (np.cumsum(keep, axis=1) - 1)[rows_idx, wins]
    smax = int(keep.sum(axis=1).max())

    M = _f8t_members()                       # [FT_NW, 4]
    e = np.asarray(e, dtype=np.float32)
    xn32 = np.ascontiguousarray(xn, dtype=np.float32)
    order = np.argsort(wins, kind="stable")
    rows_s, slots_s, wins_s = rows_idx[order], slots[order], wins[order]
    uniq, starts = np.unique(wins_s, return_index=True)
    bounds = np.append(starts, len(wins_s))

    sims = np.full((B, smax, 4), -np.inf, dtype=np.float32)
    wfull = np.zeros((B, smax), dtype=np.int64)
    wfull[rows_idx, slots] = wins
    for ui in range(len(uniq)):
        w = int(uniq[ui])
        core, wloc = divmod(w, FT_NW)
        mem = core * N_CORE + M[wloc]                        # [4] global ids
        valid = mem < N_EMB
        if not valid.any():
            continue
        mv = mem[valid]
        s0, s1 = bounds[ui], bounds[ui + 1]
        en_w = e[mv] * inv[mv][:, None]
        sblk = xn32[rows_s[s0:s1]] @ en_w.T                  # [nrows, <=4]
        sims[rows_s[s0:s1][:, None], slots_s[s0:s1][:, None],
             np.nonzero(valid)[0][None, :]] = sblk

    cores_f = wfull // FT_NW
    cand = (cores_f[:, :, None] * N_CORE + M[wfull % FT_NW]).reshape(B, -1)
    sims = sims.reshape(B, -1)
    u = sims.view(np.uint32)
    mono = np.where(u & 0x80000000, ~u, u | 0x80000000).astype(np.uint64)
    combo = ((np.uint64(0xFFFFFFFF) - mono) << np.uint64(17)) | \
        cand.astype(np.uint64)
    combo[sims == -np.inf] = np.uint64(0xFFFFFFFFFFFFFFFF)
    ordr = np.argsort(combo, axis=1, kind="stable")[:, :K_NEIGH]
    neighbors = np.take_along_axis(cand, ordr, axis=1)
    return _mode_pred(neighbors, labels)


def _get_nc(variant=None):
    variant = variant or MM_DTYPE
    if variant not in _CACHE:
        _CACHE[variant] = _build(variant)
    return _CACHE[variant]


def _normalize(x, embeddings):
    x = np.asarray(x, dtype=np.float32)
    e = np.asarray(embeddings, dtype=np.float32)
    xn = x / np.maximum(np.linalg.norm(x, axis=1, keepdims=True), EPS)
    inv = (1.0 / np.maximum(np.linalg.norm(e, axis=1), EPS)).astype(np.float32)
    return xn, e, inv


def _prep_f16w(xn, e, inv):
    """in_maps for the f16w variant: fp16 transposed normalized shards."""
    xh = np.ascontiguousarray(xn.T).astype(np.float16)
    in_maps = []
    for i in range(CORES):
        lo_r, hi_r = i * N_CORE, (i + 1) * N_CORE
        n_real = max(0, min(hi_r, N_EMB) - lo_r)
        eh = np.zeros((D, N_CORE), dtype=np.float16)
        if n_real > 0:
            sl = e[lo_r:lo_r + n_real] * inv[lo_r:lo_r + n_real][:, None]
            eh[:, :n_real] = sl.T.astype(np.float16)
        in_maps.append({"xh": xh, "eh": eh})
    return in_maps


def _prep_inputs(x, embeddings, variant):
    """Host prep: normalize embeddings, pad, transpose, shard; returns in_maps.

    Works per-core-shard to keep intermediates cache-sized."""
    if variant == "f16w":
        xn, e, inv = _normalize(x, embeddings)
        return _prep_f16w(xn, e, inv)
    if variant in ("f8t", "f8v", "f8m"):
        xn, e, inv = _normalize(x, embeddings)
        return _prep_f8t(xn, e, inv)
    if variant in ("f8w", "f8d", "f8e"):
        xn, e, inv = _normalize(x, embeddings)
        return _prep_f8w(xn, e, inv)
    x = np.asarray(x, dtype=np.float32)
    e = np.asarray(embeddings, dtype=np.float32)
    inv = (1.0 / np.maximum(np.linalg.norm(e, axis=1), EPS)).astype(np.float32)
    xt = np.ascontiguousarray(x.T)               # [D, B]

    in_maps = []
    for i in range(CORES):
        lo_r, hi_r = i * N_CORE, (i + 1) * N_CORE
        n_real = max(0, min(hi_r, N_EMB) - lo_r)
        ent = np.zeros((D, N_CORE), dtype=np.float32)
        if n_real > 0:
            sl = e[lo_r:lo_r + n_real]
            ent[:, :n_real] = sl.T * inv[lo_r:lo_r + n_real][None, :]
        if variant == "f16x3":
            ehi = ent.astype(np.float16)
            elo = (ent - ehi).astype(np.float16)
            in_maps.append({"ehi": ehi, "elo": elo})
        else:
            in_maps.append({"ent": ent})

    if variant == "f16x3":
        xhi = xt.astype(np.float16)
        xlo = (xt - xhi).astype(np.float16)
        for m in in_maps:
            m["xhi"] = xhi
            m["xlo"] = xlo
    else:
        for m in in_maps:
            m["xt"] = xt
    return in_maps


def _merge(results, labels):
    """Host merge: exact global top-10 from per-core per-chunk top-8 pools,
    then the reference's mode computation."""
    vals = np.concatenate([r["vals"] for r in results], axis=1)   # [B, 8*NOUT]
    idx8 = np.concatenate([r["idx"] for r in results], axis=1).astype(np.int64)

    col_base = (np.arange(NOUT, dtype=np.int64) // 8) * CHUNK      # chunk offset
    core_base = np.repeat(np.arange(CORES, dtype=np.int64) * N_CORE, NOUT)
    g = idx8 + np.tile(col_base, CORES)[None, :] + core_base[None, :]

    # padding rows (g >= N_EMB) are zero embeddings: exclude
    u = vals.view(np.uint32)
    key = np.where(u & 0x80000000, ~u, u | 0x80000000).astype(np.uint64)
    combo = ((np.uint64(0xFFFFFFFF) - key) << np.uint64(17)) | g.astype(np.uint64)
    combo[g >= N_EMB] = np.uint64(0xFFFFFFFFFFFFFFFF)
    order = np.argsort(combo, axis=1, kind="stable")[:, :K_NEIGH]
    neighbors = np.take_along_axis(g, order, axis=1)               # [B, 10]

    labels = np.asarray(labels)
    nl = labels[neighbors].astype(np.int64)                        # [B, 10]
    eq = nl[:, :, None] == nl[:, None, :]
    counts = eq.sum(-1)
    mkey = counts * (NUM_CLASSES + 1) + (NUM_CLASSES - nl)
    mi = np.argmax(mkey, axis=1)
    pred = np.take_along_axis(nl, mi[:, None], axis=1)[:, 0]
    return pred.astype(labels.dtype)


class _Runner:
    """Caches the shard_map-jitted executable across calls (mirrors
    bass2jax.run_bass_via_pjrt's multi-core path, which re-traces per call)."""

    def __init__(self, variant):
        import jax
        import concourse.mybir as mb
        from concourse import bass2jax
        from jax.experimental.shard_map import shard_map
        from jax.sharding import Mesh, PartitionSpec

        bass2jax.install_neuronx_cc_hook()
        self.jax = jax
        nc = _get_nc(variant)
        partition_name = (nc.partition_id_tensor.name
                          if nc.partition_id_tensor else None)
        in_names, out_names, out_avals, zeros = [], [], [], []
        for alloc in nc.m.functions[0].allocations:
            if not isinstance(alloc, mb.MemoryLocationSet):
                continue
            name = alloc.memorylocations[0].name
            if alloc.kind == "ExternalInput":
                if name != partition_name:
                    in_names.append(name)
            elif alloc.kind == "ExternalOutput":
                shape = tuple(alloc.tensor_shape)
                dtype = mb.dt.np(alloc.dtype)
                out_avals.append(jax.core.ShapedArray(shape, dtype))
                out_names.append(name)
                zeros.append(np.zeros((CORES * shape[0],) + shape[1:], dtype))
        self.in_names = list(in_names)
        self.out_names = out_names
        self.out_avals = out_avals
        self.zeros = zeros
        n_params = len(in_names)
        all_names = in_names + out_names
        if partition_name is not None:
            all_names = all_names + [partition_name]
        donate = tuple(range(n_params, n_params + len(out_names)))

        def _body(*args):
            operands = list(args)
            if partition_name is not None:
                operands.append(bass2jax.partition_id_tensor())
            outs = bass2jax._bass_exec_p.bind(
                *operands,
                out_avals=tuple(out_avals),
                in_names=tuple(all_names),
                out_names=tuple(out_names),
                lowering_input_output_aliases=(),
                sim_require_finite=True,
                sim_require_nnan=True,
                nc=nc,
            )
            return tuple(outs)

        devices = jax.devices()[:CORES]
        self.mesh = Mesh(np.asarray(devices), ("core",))
        self.pspec = PartitionSpec("core")
        in_specs = (self.pspec,) * (n_params + len(out_names))
        out_specs = (self.pspec,) * len(out_names)
        self.sharded = jax.jit(
            shard_map(_body, mesh=self.mesh, in_specs=in_specs,
                      out_specs=out_specs, check_rep=False),
            donate_argnums=donate, keep_unused=True,
        )

    def concat_inputs(self, in_maps):
        return [
            np.concatenate([np.asarray(m[name]) for m in in_maps], axis=0)
            for name in self.in_names
        ]

    def device_put(self, concat_in):
        from jax.sharding import NamedSharding
        sh = NamedSharding(self.mesh, self.pspec)
        return [self.jax.device_put(a, sh) for a in concat_in]

    def execute(self, concat_in):
        zeros = [np.zeros_like(z) for z in self.zeros]
        out_arrs = self.sharded(*concat_in, *zeros)
        return out_arrs

    def run(self, in_maps):
        out_arrs = self.execute(self.concat_inputs(in_maps))
        return [
            {
                name: np.asarray(out_arrs[i]).reshape(
                    CORES, *self.out_avals[i].shape)[c]
                for i, name in enumerate(self.out_names)
            }
            for c in range(CORES)
        ]


_RUNNERS = {}


def _get_runner(variant=None):
    variant = variant or MM_DTYPE
    if variant not in _RUNNERS:
        _RUNNERS[variant] = _Runner(variant)
    return _RUNNERS[variant]


def _mode_pred(neighbors, labels):
    """Reference's torch.mode semantics on gathered neighbor labels."""
    labels = np.asarray(labels)
    nl = labels[neighbors].astype(np.int64)                        # [B, 10]
    eq = nl[:, :, None] == nl[:, None, :]
    counts = eq.sum(-1)
    mkey = counts * (NUM_CLASSES + 1) + (NUM_CLASSES - nl)
    mi = np.argmax(mkey, axis=1)
    pred = np.take_along_axis(nl, mi[:, None], axis=1)[:, 0]
    return pred.astype(labels.dtype)


def _merge_f16w(results, labels, xn, e, inv, margin=MARGIN):
    """Select windows >= (10th-best window max) - margin, rescore those
    candidates exactly in fp64, exact global top-10, then mode."""
    wv = np.stack([r["wvals"] for r in results], axis=1)      # [B, 8, 32]
    wi = np.stack([r["widx"] for r in results], axis=1).astype(np.int64)
    wi[:, :, NSEL:] += HALF_A   # half-B indices are relative to its slice
    gw = wi + (np.arange(CORES, dtype=np.int64) * WPC)[None, :, None]
    wv = wv.reshape(B, CORES * 2 * NSEL)
    gw = gw.reshape(B, CORES * 2 * NSEL)

    w10 = np.partition(wv, wv.shape[1] - K_NEIGH, axis=1)[:, wv.shape[1] - K_NEIGH]
    keep = wv >= (w10[:, None] - margin)
    smax = int(keep.sum(axis=1).max())

    # top-smax windows per row by value; mask out ones below the cutoff
    order = np.argsort(-wv, axis=1, kind="stable")[:, :smax]
    sel_g = np.take_along_axis(gw, order, axis=1)              # [B, smax]
    sel_keep = np.take_along_axis(keep, order, axis=1)

    # rescore grouped by window: each window's embeddings are one contiguous
    # 32-row slice, shared by every query that selected it (~6400 windows
    # total vs ~170k (row, window) pairs -> tiny gathers, BLAS-sized GEMMs)
    e = np.asarray(e, dtype=np.float32)
    xn32 = np.ascontiguousarray(xn, dtype=np.float32)
    rows_idx, slots = np.nonzero(sel_keep)
    wins = sel_g[rows_idx, slots]
    order = np.argsort(wins, kind="stable")
    rows_idx, slots, wins = rows_idx[order], slots[order], wins[order]
    uniq, starts = np.unique(wins, return_index=True)
    bounds = np.append(starts, len(wins))

    sims = np.full((B, smax, WWIN), -np.inf, dtype=np.float32)
    for ui in range(len(uniq)):
        w = int(uniq[ui])
        c0, c1 = w * WWIN, min(w * WWIN + WWIN, N_EMB)
        if c1 <= c0:
            continue
        s0, s1 = bounds[ui], bounds[ui + 1]
        en_w = e[c0:c1] * inv[c0:c1][:, None]                  # [<=32, D]
        sblk = xn32[rows_idx[s0:s1]] @ en_w.T                  # [nrows, <=32]
        sims[rows_idx[s0:s1], slots[s0:s1], :c1 - c0] = sblk

    cand = (sel_g[:, :, None] * WWIN +
            np.arange(WWIN, dtype=np.int64)[None, None, :]).reshape(B, -1)
    sims = sims.reshape(B, -1)

    # exact top-10 by (-sim, cand) via an order-preserving uint64 key
    u = sims.view(np.uint32)
    mono = np.where(u & 0x80000000, ~u, u | 0x80000000).astype(np.uint64)
    combo = ((np.uint64(0xFFFFFFFF) - mono) << np.uint64(17)) | \
        cand.astype(np.uint64)
    combo[sims == -np.inf] = np.uint64(0xFFFFFFFFFFFFFFFF)
    ordr = np.argsort(combo, axis=1, kind="stable")[:, :K_NEIGH]
    neighbors = np.take_along_axis(cand, ordr, axis=1)
    return _mode_pred(neighbors, labels)


def _merge_f8d(results, labels, xn, e, inv, margin):
    """Host-side window selection from the full per-window-max arrays, then
    the window-grouped exact rescore."""
    wv = np.concatenate([r["wmax"] for r in results], axis=1)   # [B, 8*WPC]
    nw = wv.shape[1]
    w10 = np.partition(wv, nw - K_NEIGH, axis=1)[:, nw - K_NEIGH]
    keep = wv >= (w10[:, None] - margin)                        # [B, 8*WPC]

    rows_idx, wins = np.nonzero(keep)        # wins are global window ids
    slots = (np.cumsum(keep, axis=1) - 1)[rows_idx, wins]
    smax = int(keep.sum(axis=1).max())

    e = np.asarray(e, dtype=np.float32)
    xn32 = np.ascontiguousarray(xn, dtype=np.float32)
    order = np.argsort(wins, kind="stable")
    rows_s, slots_s, wins_s = rows_idx[order], slots[order], wins[order]
    uniq, starts = np.unique(wins_s, return_index=True)
    bounds = np.append(starts, len(wins_s))

    sims = np.full((B, smax, WWIN), -np.inf, dtype=np.float32)
    wfull = np.zeros((B, smax), dtype=np.int64)
    wfull[rows_idx, slots] = wins
    for ui in range(len(uniq)):
        w = int(uniq[ui])
        c0, c1 = w * WWIN, min(w * WWIN + WWIN, N_EMB)
        if c1 <= c0:
            continue
        s0, s1 = bounds[ui], bounds[ui + 1]
        en_w = e[c0:c1] * inv[c0:c1][:, None]
        sblk = xn32[rows_s[s0:s1]] @ en_w.T
        sims[rows_s[s0:s1], slots_s[s0:s1], :c1 - c0] = sblk

    cand = (wfull[:, :, None] * WWIN +
            np.arange(WWIN, dtype=np.int64)[None, None, :]).reshape(B, -1)
    sims = sims.reshape(B, -1)
    u = sims.view(np.uint32)
    mono = np.where(u & 0x80000000, ~u, u | 0x80000000).astype(np.uint64)
    combo = ((np.uint64(0xFFFFFFFF) - mono) << np.uint64(17)) | \
        cand.astype(np.uint64)
    combo[sims == -np.inf] = np.uint64(0xFFFFFFFFFFFFFFFF)
    ordr = np.argsort(combo, axis=1, kind="stable")[:, :K_NEIGH]
    neighbors = np.take_along_axis(cand, ordr, axis=1)
    return _mode_pred(neighbors, labels)


def run_on_hw(x, embeddings, variant=None):
    runner = _get_runner(variant)
    in_maps = _prep_inputs(x, embeddings, variant or MM_DTYPE)
    return runner.run(in_maps)


def kernel(x, embeddings, labels):
    variant = MM_DTYPE
    if variant == "f16w":
        xn, e, inv = _normalize(x, embeddings)
        runner = _get_runner(variant)
        results = runner.run(_prep_f16w(xn, e, inv))
        return _merge_f16w(results, labels, xn, e, inv)
    if variant == "f8w":
        xn, e, inv = _normalize(x, embeddings)
        runner = _get_runner(variant)
        results = runner.run(_prep_f8w(xn, e, inv))
        return _merge_f16w(results, labels, xn, e, inv,
                           margin=MARGIN_F8 * F8_SCALE * F8_SCALE)
    if variant == "f8t":
        xn, e, inv = _normalize(x, embeddings)
        runner = _get_runner(variant)
        results = runner.run(_prep_f8t(xn, e, inv))
        # margin: fp8 sim error (scaled) + 2x fp16 rounding of the maxes
        return _merge_f8t(results, labels, xn, e, inv,
                          margin=MARGIN_F8 * F8_SCALE * F8_SCALE + 0.5)
    if variant == "f8v":
        xn, e, inv = _normalize(x, embeddings)
        runner = _get_runner(variant)
        results = runner.run(_prep_f8t(xn, e, inv))
        return _merge_fv(results, labels, xn, e, inv,
                         MARGIN_F8 * F8_SCALE * F8_SCALE + 0.5,
                         FV_NW, _fv_members())
    if variant == "f8m":
        xn, e, inv = _normalize(x, embeddings)
        runner = _get_runner(variant)
        results = runner.run(_prep_f8t(xn, e, inv))
        return _merge_fv(results, labels, xn, e, inv,
                         MARGIN_F8 * F8_SCALE * F8_SCALE + 0.5,
                         FM_NW, _fm_members())
    if variant in ("f8d", "f8e"):
        xn, e, inv = _normalize(x, embeddings)
        runner = _get_runner(variant)
        results = runner.run(_prep_f8w(xn, e, inv))
        return _merge_f8d(results, labels, xn, e, inv,
                          margin=MARGIN_F8 * F8_SCALE * F8_SCALE)
    results = run_on_hw(x, embeddings)
    return _merge(results, labels)



# revision 42
# speedup vs baseline: 1.6761x; 1.0177x over previous
"""Trainium2 Bass kernel for BaselineKNNModel (cosine-sim KNN classifier).

Contract: kernel(**inputs) takes FULL inputs (x [2048,512] f32,
embeddings [100000,512] f32, labels [100000] int) and returns the FULL
output (pred [2048] labels.dtype), distributing work across 8 NeuronCores.

Strategy (database-parallel, per sharding hint; active variant "f8m"):
 - Host: normalize embeddings (cosine denominator), pad N 100000->102400,
   shard along N across 8 cores (12800 each), quantize to fp8e4m3 (scaled
   by 16 to clear the subnormal range). x normalization is skipped:
   per-query positive scaling cannot change that query's top-k ranking.
 - Device (SPMD, per core): fp8 DoubleRow matmuls (K=256 per pass) produce
   sim tiles in PSUM. TRN2 allows at most one PSUM operand per instruction
   and gpsimd has no PSUM access, so the sim matrix is consumed by the two
   engines that can touch it: ScalarE copies half the candidates
   PSUM->fp16 SBUF while DVE retires a width-2 window per cycle via
   max(psum_half, staged_half). 1-bank PSUM tiles at depth 4 hide the
   mm->act->t_t dependency chain (the tile sync compiler has a single
   sem-wait slot per instruction, so chained deps otherwise stall PE).
   Per-window fp16 maxes (102400/2 per query per core) DMA out in
   q-groups overlapped with compute.
 - Host: margin selection (keep windows within fp8+fp16 error margin of
   the 10th-best window max), exact fp32 rescore of kept windows' members,
   exact top-10 by (value desc, index asc) = jax.lax.top_k tie order, then
   the reference's mode computation.
"""
import sys

for _p in ("/opt/trn_rl_repo", "/root/.axon_site/_ro/trn_rl_repo"):
    if _p not in sys.path:
        sys.path.insert(0, _p)

import numpy as np

import concourse.bacc as bacc
import concourse.mybir as mybir
import concourse.tile as tile
from concourse import bass_utils

F32 = mybir.dt.float32
F32R = mybir.dt.float32r
F16 = mybir.dt.float16
U32 = mybir.dt.uint32
Copy = mybir.ActivationFunctionType.Copy

B = 2048            # queries
D = 512             # embedding dim
N_EMB = 100000      # database size
K_NEIGH = 10
NUM_CLASSES = 1000
EPS = 1e-8

CORES = 8
N_PAD = 102400      # padded database size (8 * 12800)
N_CORE = N_PAD // CORES     # 12800 candidates per core
CHUNK = 512                 # candidates per sim tile (one PSUM bank)
NCHUNK = N_CORE // CHUNK    # 25
QT = B // 128               # 16 query tiles
KT = D // 128               # 4 k-tiles
NOUT = NCHUNK * 8           # 200 output slots per query per core

# f16w variant: window-max + device window top-16 + host exact rescore
WWIN = 32                   # candidates per window
WPC = N_CORE // WWIN        # 400 windows per core
BIGCHUNK = 1024             # candidates per PSUM tile (2 banks)
NSEL = 16                   # windows kept per (query, core, half)
HALF_A = (7 * BIGCHUNK) // WWIN  # windows in selection half A (224)
MARGIN = 4e-3               # fp16-sim error margin on unit-normalized sims
                            # (measured max |fp16 sim err| ~6e-5, ~60x safety)

# f8w variant: same as f16w but fp8e4m3 DoubleRow matmuls (2 fp8 weights per
# PE cell, K=256 per matmul). Inputs are scaled by F8_SCALE before rounding
# to fp8, so device sims (and window maxes) are scaled by F8_SCALE^2.
F8_SCALE = 16.0
MARGIN_F8 = 2.5e-2          # fp8 margin on unit-normalized sims
                            # (measured max err 7.1e-3 on a sample, rms 1.6e-3)

MM_DTYPE = "f8m"   # "f32"|"f32r"|"f16x3"|"f16w"|"f8w"|"f8d"|"f8e"|"f8t"|"f8v"|"f8m"

_CACHE = {}


def _build(variant):
    """Build + compile the per-core Bass program. Same program on all cores;
    only the `ent*` input shards differ."""
    nc = bacc.Bacc("TRN2", target_bir_lowering=False, debug=False)

    if variant == "noop":  # minimal program for RPC-overhead baselining
        d_nin = nc.dram_tensor("nin", [128, 128], F32, kind="ExternalInput")
        d_nout = nc.dram_tensor("nout", [128, 128], F32, kind="ExternalOutput")
        with tile.TileContext(nc) as tc:
            with tc.tile_pool(name="np0", bufs=1) as pool:
                t = pool.tile([128, 128], F32, tag="t")
                nc.sync.dma_start(t[:, :], d_nin[:, :])
                nc.sync.dma_start(d_nout[:, :], t[:, :])
        nc.compile()
        return nc

    if variant == "f16w":
        return _build_f16w(nc)
    if variant == "f8w":
        return _build_f8w(nc)
    if variant == "f8d":
        return _build_f8d(nc)
    if variant == "f8e":
        return _build_f8e(nc)
    if variant == "f8t":
        return _build_f8t(nc)
    if variant == "f8v":
        return _build_f8v(nc)
    if variant == "f8m":
        return _build_f8m(nc)

    f16 = variant == "f16x3"
    if f16:
        d_xhi = nc.dram_tensor("xhi", [D, B], F16, kind="ExternalInput")
        d_xlo = nc.dram_tensor("xlo", [D, B], F16, kind="ExternalInput")
        d_ehi = nc.dram_tensor("ehi", [D, N_CORE], F16, kind="ExternalInput")
        d_elo = nc.dram_tensor("elo", [D, N_CORE], F16, kind="ExternalInput")
    else:
        in_dt = F32R if variant == "f32r" else F32
        d_xt = nc.dram_tensor("xt", [D, B], in_dt, kind="ExternalInput")
        d_ent = nc.dram_tensor("ent", [D, N_CORE], in_dt, kind="ExternalInput")

    d_vals = nc.dram_tensor("vals", [B, NOUT], F32, kind="ExternalOutput")
    d_idx = nc.dram_tensor("idx", [B, NOUT], U32, kind="ExternalOutput")

    with tile.TileContext(nc) as tc:
        with (
            tc.tile_pool(name="xpool", bufs=1) as xpool,
            tc.tile_pool(name="epool", bufs=3) as epool,
            tc.tile_pool(name="ps", bufs=6, space="PSUM") as ps_pool,
            tc.tile_pool(name="sim", bufs=6) as sim_pool,
            tc.tile_pool(name="acc", bufs=1) as acc_pool,
        ):
            # resident x (stationary operand), k-tiles side by side
            if f16:
                xhi_sb = xpool.tile([128, KT * B], F16, tag="xhi")
                xlo_sb = xpool.tile([128, KT * B], F16, tag="xlo")
                for k in range(KT):
                    nc.sync.dma_start(xhi_sb[:, k * B:(k + 1) * B],
                                      d_xhi[k * 128:(k + 1) * 128, :])
                    nc.sync.dma_start(xlo_sb[:, k * B:(k + 1) * B],
                                      d_xlo[k * 128:(k + 1) * 128, :])
            else:
                xt_sb = xpool.tile([128, KT * B], in_dt, tag="xt")
                for k in range(KT):
                    nc.sync.dma_start(xt_sb[:, k * B:(k + 1) * B],
                                      d_xt[k * 128:(k + 1) * 128, :])

            # result accumulators, [128, QT*NOUT], column q*NOUT + c*8 + j
            vals_sb = acc_pool.tile([128, QT * NOUT], F32, tag="vacc")
            idx_sb = acc_pool.tile([128, QT * NOUT], U32, tag="iacc")

            for c in range(NCHUNK):
                c0 = c * CHUNK
                if f16:
                    ehi_sb = epool.tile([128, KT * CHUNK], F16, tag="ehi")
                    elo_sb = epool.tile([128, KT * CHUNK], F16, tag="elo")
                    for k in range(KT):
                        nc.sync.dma_start(ehi_sb[:, k * CHUNK:(k + 1) * CHUNK],
                                          d_ehi[k * 128:(k + 1) * 128, c0:c0 + CHUNK])
                        nc.sync.dma_start(elo_sb[:, k * CHUNK:(k + 1) * CHUNK],
                                          d_elo[k * 128:(k + 1) * 128, c0:c0 + CHUNK])
                else:
                    en_sb = epool.tile([128, KT * CHUNK], in_dt, tag="en")
                    for k in range(KT):
                        nc.sync.dma_start(en_sb[:, k * CHUNK:(k + 1) * CHUNK],
                                          d_ent[k * 128:(k + 1) * 128, c0:c0 + CHUNK])

                for q in range(QT):
                    ps = ps_pool.tile([128, CHUNK], F32, tag="ps")
                    if variant == "f16x3":
                        nmm = 3 * KT
                        i = 0
                        for k in range(KT):
                            xh = xhi_sb[:, k * B + q * 128: k * B + (q + 1) * 128]
                            xl = xlo_sb[:, k * B + q * 128: k * B + (q + 1) * 128]
                            eh = ehi_sb[:, k * CHUNK:(k + 1) * CHUNK]
                            el = elo_sb[:, k * CHUNK:(k + 1) * CHUNK]
                            for (a, bb) in ((xh, eh), (xh, el), (xl, eh)):
                                nc.tensor.matmul(ps[:, :], a, bb,
                                                 start=(i == 0), stop=(i == nmm - 1))
                                i += 1
                    else:
                        for k in range(KT):
                            lhsT = xt_sb[:, k * B + q * 128: k * B + (q + 1) * 128]
                            rhs = en_sb[:, k * CHUNK:(k + 1) * CHUNK]
                            nc.tensor.matmul(ps[:, :], lhsT, rhs,
                                             start=(k == 0), stop=(k == KT - 1))

                    sim = sim_pool.tile([128, CHUNK], F32, tag="sim")
                    nc.scalar.activation(sim[:, :], ps[:, :], Copy)

                    o = q * NOUT + c * 8
                    nc.vector.max(vals_sb[:, o:o + 8], sim[:, :])
                    nc.vector.max_index(idx_sb[:, o:o + 8], vals_sb[:, o:o + 8],
                                        sim[:, :])

            for q in range(QT):
                nc.sync.dma_start(d_vals[q * 128:(q + 1) * 128, :],
                                  vals_sb[:, q * NOUT:(q + 1) * NOUT])
                nc.sync.dma_start(d_idx[q * 128:(q + 1) * 128, :],
                                  idx_sb[:, q * NOUT:(q + 1) * NOUT])

    nc.compile()
    return nc


def _build_f16w(nc):
    """fp16 single-pass matmul; per-tile 16-wide window max (DVE reduce,
    PSUM-direct); per-core-half top-16 windows per query via
    max/match_replace (first half's selection overlaps the main loop);
    host rescores the selected windows exactly."""
    Max = mybir.AluOpType.max
    X = mybir.AxisListType.X

    d_xh = nc.dram_tensor("xh", [D, B], F16, kind="ExternalInput")
    d_eh = nc.dram_tensor("eh", [D, N_CORE], F16, kind="ExternalInput")
    d_wvals = nc.dram_tensor("wvals", [B, 2 * NSEL], F32, kind="ExternalOutput")
    d_widx = nc.dram_tensor("widx", [B, 2 * NSEL], U32, kind="ExternalOutput")

    # chunk layout: 12 x 1024 + 1 x 512 = 12800
    chunks = [(i * BIGCHUNK, BIGCHUNK) for i in range(N_CORE // BIGCHUNK)]
    rem = N_CORE - (N_CORE // BIGCHUNK) * BIGCHUNK
    if rem:
        chunks.append((N_CORE - rem, rem))
    # selection halves aligned to chunk boundaries:
    # half A = chunks 0-6 (448 windows), half B = chunks 7-12 (352 windows)
    HALF_B = WPC - HALF_A

    def select(wq, vout, iout, o, width, mr_pool):
        nc.vector.max(vout[:, o:o + 8], wq)
        nc.vector.max_index(iout[:, o:o + 8], vout[:, o:o + 8], wq)
        mr = mr_pool.tile([128, width], F32, tag="mr")
        nc.vector.match_replace(mr[:, :width], vout[:, o:o + 8], wq, -1e30)
        nc.vector.max(vout[:, o + 8:o + 16], mr[:, :width])
        nc.vector.max_index(iout[:, o + 8:o + 16],
                            vout[:, o + 8:o + 16], mr[:, :width])

    with tile.TileContext(nc) as tc:
        with (
            tc.tile_pool(name="xpool", bufs=1) as xpool,
            tc.tile_pool(name="epool", bufs=3) as epool,
            tc.tile_pool(name="ps", bufs=3, space="PSUM") as ps_pool,
            tc.tile_pool(name="wacc", bufs=1) as wacc_pool,
            tc.tile_pool(name="mrp", bufs=4) as mr_pool,
            tc.tile_pool(name="outp", bufs=1) as out_pool,
        ):
            xh_sb = xpool.tile([128, KT * B], F16, tag="xh")
            for k in range(KT):
                nc.sync.dma_start(xh_sb[:, k * B:(k + 1) * B],
                                  d_xh[k * 128:(k + 1) * 128, :])

            wmax_sb = wacc_pool.tile([128, QT * WPC], F32, tag="wacc")
            vout_sb = out_pool.tile([128, QT * 2 * NSEL], F32, tag="vout")
            iout_sb = out_pool.tile([128, QT * 2 * NSEL], U32, tag="iout")

            for ci, (c0, cw) in enumerate(chunks):
                eh_sb = epool.tile([128, KT * BIGCHUNK], F16, tag="eh")
                for k in range(KT):
                    nc.sync.dma_start(eh_sb[:, k * cw:(k + 1) * cw],
                                      d_eh[k * 128:(k + 1) * 128, c0:c0 + cw])
                for q in range(QT):
                    ps = ps_pool.tile([128, BIGCHUNK], F32, tag="ps")
                    for s in range(cw // 512):
                        for k in range(KT):
                            nc.tensor.matmul(
                                ps[:, s * 512:(s + 1) * 512],
                                xh_sb[:, k * B + q * 128: k * B + (q + 1) * 128],
                                eh_sb[:, k * cw + s * 512: k * cw + s * 512 + 512],
                                start=(k == 0), stop=(k == KT - 1))
                    nwin = cw // WWIN
                    wslot = q * WPC + c0 // WWIN
                    nc.vector.tensor_reduce(
                        wmax_sb[:, wslot:wslot + nwin],
                        ps[:, :cw].rearrange("p (w i) -> p w i", i=WWIN),
                        axis=X, op=Max)
                # half A (windows [0, HALF_A)) is complete after chunk 6;
                # spread its per-q selection over chunks 6..12 (2-3 q each)
                if ci >= 6:
                    n_grp = len(chunks) - 6
                    qs = [q for q in range(QT) if q % n_grp == ci - 6]
                    for q in qs:
                        select(wmax_sb[:, q * WPC:q * WPC + HALF_A],
                               vout_sb, iout_sb, q * 2 * NSEL, HALF_A, mr_pool)

            for q in range(QT):  # half B (windows [HALF_A, WPC))
                select(wmax_sb[:, q * WPC + HALF_A:(q + 1) * WPC],
                       vout_sb, iout_sb, q * 2 * NSEL + NSEL, HALF_B, mr_pool)

            for q in range(QT):
                nc.sync.dma_start(d_wvals[q * 128:(q + 1) * 128, :],
                                  vout_sb[:, q * 2 * NSEL:(q + 1) * 2 * NSEL])
                nc.sync.dma_start(d_widx[q * 128:(q + 1) * 128, :],
                                  iout_sb[:, q * 2 * NSEL:(q + 1) * 2 * NSEL])

    nc.compile()
    return nc


def _build_f8w(nc):
    """Same structure as f16w, but fp8e4m3 DoubleRow matmuls: operands carry
    [partition, j(2), cols] APs; each matmul contracts 256 dims (2 k-groups
    of 128), so K=512 takes 2 matmuls per 512-wide output slice."""
    Max = mybir.AluOpType.max
    X = mybir.AxisListType.X
    F8 = mybir.dt.float8e4
    DR = mybir.MatmulPerfMode.DoubleRow

    d_x8 = nc.dram_tensor("x8", [D, B], F8, kind="ExternalInput")
    d_e8 = nc.dram_tensor("e8", [D, N_CORE], F8, kind="ExternalInput")
    d_wvals = nc.dram_tensor("wvals", [B, 2 * NSEL], F32, kind="ExternalOutput")
    d_widx = nc.dram_tensor("widx", [B, 2 * NSEL], U32, kind="ExternalOutput")

    chunks = [(i * BIGCHUNK, BIGCHUNK) for i in range(N_CORE // BIGCHUNK)]
    rem = N_CORE - (N_CORE // BIGCHUNK) * BIGCHUNK
    if rem:
        chunks.append((N_CORE - rem, rem))
    HALF_B = WPC - HALF_A

    def select(wq, vout, iout, o, width, mr_pool):
        nc.vector.max(vout[:, o:o + 8], wq)
        nc.vector.max_index(iout[:, o:o + 8], vout[:, o:o + 8], wq)
        mr = mr_pool.tile([128, width], F32, tag="mr")
        nc.vector.match_replace(mr[:, :width], vout[:, o:o + 8], wq, -1e30)
        nc.vector.max(vout[:, o + 8:o + 16], mr[:, :width])
        nc.vector.max_index(iout[:, o + 8:o + 16],
                            vout[:, o + 8:o + 16], mr[:, :width])

    with tile.TileContext(nc) as tc:
        with (
            tc.tile_pool(name="xpool", bufs=1) as xpool,
            tc.tile_pool(name="epool", bufs=3) as epool,
            tc.tile_pool(name="ps", bufs=3, space="PSUM") as ps_pool,
            tc.tile_pool(name="wacc", bufs=1) as wacc_pool,
            tc.tile_pool(name="mrp", bufs=4) as mr_pool,
            tc.tile_pool(name="outp", bufs=1) as out_pool,
        ):
            # [g][j][cols] layout: row-range g*256 + j*128 of the [D, *] input
            x_sb = xpool.tile([128, 4 * B], F8, tag="x8")
            for g in range(2):
                for j in range(2):
                    r0 = g * 256 + j * 128
                    nc.sync.dma_start(x_sb[:, (g * 2 + j) * B:(g * 2 + j + 1) * B],
                                      d_x8[r0:r0 + 128, :])

            wmax_sb = wacc_pool.tile([128, QT * WPC], F32, tag="wacc")
            vout_sb = out_pool.tile([128, QT * 2 * NSEL], F32, tag="vout")
            iout_sb = out_pool.tile([128, QT * 2 * NSEL], U32, tag="iout")

            for ci, (c0, cw) in enumerate(chunks):
                eh_sb = epool.tile([128, 4 * BIGCHUNK], F8, tag="e8")
                for g in range(2):
                    for j in range(2):
                        r0 = g * 256 + j * 128
                        nc.sync.dma_start(
                            eh_sb[:, (g * 2 + j) * cw:(g * 2 + j + 1) * cw],
                            d_e8[r0:r0 + 128, c0:c0 + cw])
                for q in range(QT):
                    ps = ps_pool.tile([128, BIGCHUNK], F32, tag="ps")
                    for s in range(cw // 512):
                        for g in range(2):
                            lhsT = x_sb[:, g * 2 * B:(g + 1) * 2 * B].rearrange(
                                "p (j b) -> p j b", j=2)[:, :, q * 128:(q + 1) * 128]
                            rhs = eh_sb[:, g * 2 * cw:(g + 1) * 2 * cw].rearrange(
                                "p (j n) -> p j n", j=2)[:, :, s * 512:(s + 1) * 512]
                            nc.tensor.matmul(ps[:, s * 512:(s + 1) * 512],
                                             lhsT, rhs, perf_mode=DR,
                                             start=(g == 0), stop=(g == 1))
                    nwin = cw // WWIN
                    wslot = q * WPC + c0 // WWIN
                    nc.vector.tensor_reduce(
                        wmax_sb[:, wslot:wslot + nwin],
                        ps[:, :cw].rearrange("p (w i) -> p w i", i=WWIN),
                        axis=X, op=Max)
                if ci >= 6:
                    n_grp = len(chunks) - 6
                    qs = [q for q in range(QT) if q % n_grp == ci - 6]
                    for q in qs:
                        select(wmax_sb[:, q * WPC:q * WPC + HALF_A],
                               vout_sb, iout_sb, q * 2 * NSEL, HALF_A, mr_pool)

            for q in range(QT):
                select(wmax_sb[:, q * WPC + HALF_A:(q + 1) * WPC],
                       vout_sb, iout_sb, q * 2 * NSEL + NSEL, HALF_B, mr_pool)

            for q in range(QT):
                nc.sync.dma_start(d_wvals[q * 128:(q + 1) * 128, :],
                                  vout_sb[:, q * 2 * NSEL:(q + 1) * 2 * NSEL])
                nc.sync.dma_start(d_widx[q * 128:(q + 1) * 128, :],
                                  iout_sb[:, q * 2 * NSEL:(q + 1) * 2 * NSEL])

    nc.compile()
    return nc


_F8_LUT = None


def _to_f8(a):
    """Fast float->fp8e4m3: fp16 hardware cast, then a 64K-entry LUT over the
    fp16 bit patterns (ml_dtypes' elementwise astype is ~50x slower). The
    double rounding vs a direct fp32->fp8 cast is harmless here: any
    consistent rounding is covered by the selection margin."""
    global _F8_LUT
    import ml_dtypes
    if _F8_LUT is None:
        with np.errstate(all="ignore"):
            all16 = np.arange(65536, dtype=np.uint16).view(np.float16)
            _F8_LUT = (all16.astype(np.float32)
                       .astype(ml_dtypes.float8_e4m3).view(np.uint8))
    h = a.astype(np.float16).view(np.uint16)
    return _F8_LUT[h].view(ml_dtypes.float8_e4m3)


def _build_f8d(nc):
    """f8w minus on-device window selection: the full per-window max array
    ships to the host (3.3MB/core), which does the margin selection itself.
    ScalarE stages PSUM->SBUF so the DVE reduce pays the SBUF (not PSUM)
    access bubble; DVE runs nothing but the 208 window-max reduces."""
    Max = mybir.AluOpType.max
    X = mybir.AxisListType.X
    F8 = mybir.dt.float8e4
    DR = mybir.MatmulPerfMode.DoubleRow
    Copy = mybir.ActivationFunctionType.Copy

    d_x8 = nc.dram_tensor("x8", [D, B], F8, kind="ExternalInput")
    d_e8 = nc.dram_tensor("e8", [D, N_CORE], F8, kind="ExternalInput")
    d_wmax = nc.dram_tensor("wmax", [B, WPC], F32, kind="ExternalOutput")

    chunks = [(i * BIGCHUNK, BIGCHUNK) for i in range(N_CORE // BIGCHUNK)]
    rem = N_CORE - (N_CORE // BIGCHUNK) * BIGCHUNK
    if rem:
        chunks.append((N_CORE - rem, rem))

    with tile.TileContext(nc) as tc:
        with (
            tc.tile_pool(name="xpool", bufs=1) as xpool,
            tc.tile_pool(name="epool", bufs=3) as epool,
            tc.tile_pool(name="ps", bufs=3, space="PSUM") as ps_pool,
            tc.tile_pool(name="stg", bufs=3) as stg_pool,
            tc.tile_pool(name="wacc", bufs=1) as wacc_pool,
        ):
            x_sb = xpool.tile([128, 4 * B], F8, tag="x8")
            for g in range(2):
                for j in range(2):
                    r0 = g * 256 + j * 128
                    nc.sync.dma_start(x_sb[:, (g * 2 + j) * B:(g * 2 + j + 1) * B],
                                      d_x8[r0:r0 + 128, :])

            wmax_sb = wacc_pool.tile([128, QT * WPC], F32, tag="wacc")

            for (c0, cw) in chunks:
                eh_sb = epool.tile([128, 4 * BIGCHUNK], F8, tag="e8")
                for g in range(2):
                    for j in range(2):
                        r0 = g * 256 + j * 128
                        nc.sync.dma_start(
                            eh_sb[:, (g * 2 + j) * cw:(g * 2 + j + 1) * cw],
                            d_e8[r0:r0 + 128, c0:c0 + cw])
                for q in range(QT):
                    ps = ps_pool.tile([128, BIGCHUNK], F32, tag="ps")
                    for s in range(cw // 512):
                        for g in range(2):
                            lhsT = x_sb[:, g * 2 * B:(g + 1) * 2 * B].rearrange(
                                "p (j b) -> p j b", j=2)[:, :, q * 128:(q + 1) * 128]
                            rhs = eh_sb[:, g * 2 * cw:(g + 1) * 2 * cw].rearrange(
                                "p (j n) -> p j n", j=2)[:, :, s * 512:(s + 1) * 512]
                            nc.tensor.matmul(ps[:, s * 512:(s + 1) * 512],
                                             lhsT, rhs, perf_mode=DR,
                                             start=(g == 0), stop=(g == 1))
                    stg = stg_pool.tile([128, BIGCHUNK], F32, tag="stg")
                    nc.scalar.activation(stg[:, :cw], ps[:, :cw], Copy)
                    nwin = cw // WWIN
                    wslot = q * WPC + c0 // WWIN
                    nc.vector.tensor_reduce(
                        wmax_sb[:, wslot:wslot + nwin],
                        stg[:, :cw].rearrange("p (w i) -> p w i", i=WWIN),
                        axis=X, op=Max)

            for q in range(QT):
                nc.sync.dma_start(d_wmax[q * 128:(q + 1) * 128, :],
                                  wmax_sb[:, q * WPC:(q + 1) * WPC])

    nc.compile()
    return nc


def _build_f8e(nc):
    """f8d with wider DVE reduces (two staged PSUM tiles -> one 2048-wide
    window-max, halving the per-op SBUF bubble count) and per-half early
    wmax DMA-out so the output transfer overlaps the main loop."""
    Max = mybir.AluOpType.max
    X = mybir.AxisListType.X
    F8 = mybir.dt.float8e4
    DR = mybir.MatmulPerfMode.DoubleRow
    Copy = mybir.ActivationFunctionType.Copy

    d_x8 = nc.dram_tensor("x8", [D, B], F8, kind="ExternalInput")
    d_e8 = nc.dram_tensor("e8", [D, N_CORE], F8, kind="ExternalInput")
    d_wmax = nc.dram_tensor("wmax", [B, WPC], F32, kind="ExternalOutput")

    BC = 2048  # 4 PSUM banks per tile; 6x2048 + 1x512 = 12800
    chunks = [(i * BC, BC) for i in range(N_CORE // BC)]
    rem = N_CORE - (N_CORE // BC) * BC
    if rem:
        chunks.append((N_CORE - rem, rem))
    AWIN = (4 * BC) // WWIN  # 256 windows (chunks 0-3) ship mid-loop

    with tile.TileContext(nc) as tc:
        with (
            tc.tile_pool(name="xpool", bufs=1) as xpool,
            tc.tile_pool(name="epool", bufs=3) as epool,
            tc.tile_pool(name="ps", bufs=2, space="PSUM") as ps_pool,
            tc.tile_pool(name="stg", bufs=3) as stg_pool,
            tc.tile_pool(name="wacc", bufs=1) as wacc_pool,
        ):
            x_sb = xpool.tile([128, 4 * B], F8, tag="x8")
            for g in range(2):
                for j in range(2):
                    r0 = g * 256 + j * 128
                    nc.sync.dma_start(x_sb[:, (g * 2 + j) * B:(g * 2 + j + 1) * B],
                                      d_x8[r0:r0 + 128, :])

            wmax_sb = wacc_pool.tile([128, QT * WPC], F32, tag="wacc")

            for ci, (c0, cw) in enumerate(chunks):
                eh_sb = epool.tile([128, 4 * BC], F8, tag="e8")
                for g in range(2):
                    for j in range(2):
                        r0 = g * 256 + j * 128
                        nc.sync.dma_start(
                            eh_sb[:, (g * 2 + j) * cw:(g * 2 + j + 1) * cw],
                            d_e8[r0:r0 + 128, c0:c0 + cw])
                for q in range(QT):
                    ps = ps_pool.tile([128, BC], F32, tag="ps")
                    for s in range(cw // 512):
                        for g in range(2):
                            lhsT = x_sb[:, g * 2 * B:(g + 1) * 2 * B].rearrange(
                                "p (j b) -> p j b", j=2)[:, :, q * 128:(q + 1) * 128]
                            rhs = eh_sb[:, g * 2 * cw:(g + 1) * 2 * cw].rearrange(
                                "p (j n) -> p j n", j=2)[:, :, s * 512:(s + 1) * 512]
                            nc.tensor.matmul(ps[:, s * 512:(s + 1) * 512],
                                             lhsT, rhs, perf_mode=DR,
                                             start=(g == 0), stop=(g == 1))
                    stg = stg_pool.tile([128, BC], F32, tag="stg")
                    nc.scalar.activation(stg[:, :cw], ps[:, :cw], Copy)
                    nwin = cw // WWIN
                    wslot = q * WPC + c0 // WWIN
                    nc.vector.tensor_reduce(
                        wmax_sb[:, wslot:wslot + nwin],
                        stg[:, :cw].rearrange("p (w i) -> p w i", i=WWIN),
                        axis=X, op=Max)
                    if ci == 3:  # chunks 0-3 reduced for q: ship 256 windows
                        nc.sync.dma_start(
                            d_wmax[q * 128:(q + 1) * 128, :AWIN],
                            wmax_sb[:, q * WPC:q * WPC + AWIN])

            for q in range(QT):
                nc.sync.dma_start(d_wmax[q * 128:(q + 1) * 128, AWIN:],
                                  wmax_sb[:, q * WPC + AWIN:(q + 1) * WPC])

    nc.compile()
    return nc


# f8t variant: pairwise-max fold tree over the sim tile, balanced across
# DVE + Pool + ScalarE so the fp8 matmuls (PE) are the bottleneck. Window
# width 4 (chunk cols {j, j+cw/4, j+cw/2, j+3cw/4}); full per-window max
# array ships to host for margin selection + exact rescore.
FT_BC = 2048                 # main chunk width (4 PSUM banks)
FT_WD = 124                  # fold1 pairs on DVE (direct fp32 PSUM)
FT_WP = 520                  # fold1 pairs on Pool (direct fp32 PSUM)
FT_WS = FT_BC // 2 - FT_WD - FT_WP   # fold1 pairs via ScalarE fp16 staging
FT_RD = 66                   # remainder-chunk fold1 pairs on DVE
FT_NW = (N_CORE // FT_BC) * (FT_BC // 4) + (N_CORE % FT_BC) // 4  # 3200


def _build_f8t(nc):
    Max = mybir.AluOpType.max
    F8 = mybir.dt.float8e4
    DR = mybir.MatmulPerfMode.DoubleRow
    Copy = mybir.ActivationFunctionType.Copy

    # [p, slot, cols] inputs: slot k holds rows 128k..128k+127 of the
    # transposed operand; slot order == (g, j) DoubleRow order.
    d_x8 = nc.dram_tensor("x8", [128, 4, B], F8, kind="ExternalInput")
    d_e8 = nc.dram_tensor("e8", [128, 4, N_CORE], F8, kind="ExternalInput")
    # [p, q, w]: query q*128+p, window w (host transposes back)
    d_wmax = nc.dram_tensor("wmax", [128, QT, FT_NW], F16, kind="ExternalOutput")

    chunks = [(i * FT_BC, FT_BC) for i in range(N_CORE // FT_BC)]
    rem = N_CORE - (N_CORE // FT_BC) * FT_BC
    if rem:
        chunks.append((N_CORE - rem, rem))

    with tile.TileContext(nc) as tc:
        with (
            tc.tile_pool(name="xpool", bufs=1) as xpool,
            tc.tile_pool(name="epool", bufs=3) as epool,
            tc.tile_pool(name="ps", bufs=2, space="PSUM") as ps_pool,
            tc.tile_pool(name="stg", bufs=3) as stg_pool,
            tc.tile_pool(name="s1p", bufs=3) as s1_pool,
            tc.tile_pool(name="wm", bufs=2) as wm_pool,
        ):
            x_sb = xpool.tile([128, 4 * B], F8, tag="x8")
            nc.sync.dma_start(
                x_sb[:, :].rearrange("p (k b) -> p k b", k=4), d_x8[:, :, :])

            wout = 0
            for ci, (c0, cw) in enumerate(chunks):
                half = cw // 2
                quar = cw // 4
                e_sb = epool.tile([128, 4 * cw], F8, tag="e8")
                nc.sync.dma_start(
                    e_sb[:, :].rearrange("p (k n) -> p k n", k=4),
                    d_e8[:, :, c0:c0 + cw])
                wm_sb = wm_pool.tile([128, QT * quar], F16, tag="wm")
                for q in range(QT):
                    ps = ps_pool.tile([128, cw], F32, tag="ps")
                    for s in range(cw // 512):
                        for g in range(2):
                            lhsT = x_sb[:, g * 2 * B:(g + 1) * 2 * B].rearrange(
                                "p (j b) -> p j b", j=2)[:, :, q * 128:(q + 1) * 128]
                            rhs = e_sb[:, g * 2 * cw:(g + 1) * 2 * cw].rearrange(
                                "p (j n) -> p j n", j=2)[:, :, s * 512:(s + 1) * 512]
                            nc.tensor.matmul(ps[:, s * 512:(s + 1) * 512],
                                             lhsT, rhs, perf_mode=DR,
                                             start=(g == 0), stop=(g == 1))
                    # fold1: s1[j] = max(ps[j], ps[j+half]), j in [0, half)
                    s1 = s1_pool.tile([128, half], F16, tag="s1")
                    if cw == FT_BC:
                        wd, wp, ws = FT_WD, FT_WP, FT_WS
                    else:
                        wd, wp, ws = FT_RD, half - FT_RD, 0
                    nc.vector.tensor_tensor(
                        s1[:, :wd], ps[:, :wd], ps[:, half:half + wd], op=Max)
                    nc.gpsimd.tensor_tensor(
                        s1[:, wd:wd + wp], ps[:, wd:wd + wp],
                        ps[:, half + wd:half + wd + wp], op=Max)
                    if ws:
                        stg = stg_pool.tile([128, 2 * ws], F16, tag="stg")
                        psv = ps[:, :].rearrange("p (h j) -> p h j", h=2)
                        nc.scalar.activation(
                            stg[:, :].rearrange("p (h j) -> p h j", h=2),
                            psv[:, :, wd + wp:half], Copy)
                        nc.vector.tensor_tensor(
                            s1[:, wd + wp:], stg[:, :ws], stg[:, ws:], op=Max)
                    # fold2: wm[w] = max(s1[w], s1[w+quar]) (fp16, 2x mode)
                    nc.vector.tensor_tensor(
                        wm_sb[:, q * quar:(q + 1) * quar],
                        s1[:, :quar], s1[:, quar:], op=Max)
                nc.sync.dma_start(
                    d_wmax[:, :, wout:wout + quar],
                    wm_sb[:, :].rearrange("p (q w) -> p q w", q=QT))
                wout += quar

    nc.compile()
    return nc


# f8v variant: per-iteration candidate range split into two half-chunks with
# SEPARATE PSUM tiles so each has exactly one reader engine (the tile sync
# compiler has one sem-wait slot per instruction; multiple reader engines on
# one tile get chained serially and stall the PE).
#  - DVE half: fold1 fp32->fp16 pairwise max + fp16 fold2 -> width-4 windows
#  - Pool half: single fp32 pairwise max -> width-2 windows (shipped as-is)
FV_HALF = 1024               # cols per half-chunk (2 PSUM banks)
FV_NWC = FV_HALF // 4 + FV_HALF // 2   # 768 window cols per full chunk
FV_NW = 6 * FV_NWC + (FV_NWC // 4)     # 4800 per core (incl 512-remainder)


def _build_f8v(nc):
    Max = mybir.AluOpType.max
    F8 = mybir.dt.float8e4
    DR = mybir.MatmulPerfMode.DoubleRow

    d_x8 = nc.dram_tensor("x8", [128, 4, B], F8, kind="ExternalInput")
    d_e8 = nc.dram_tensor("e8", [128, 4, N_CORE], F8, kind="ExternalInput")
    d_wmax = nc.dram_tensor("wmax", [128, QT, FV_NW], F16, kind="ExternalOutput")

    chunks = [(i * FT_BC, FT_BC) for i in range(N_CORE // FT_BC)]
    rem = N_CORE - (N_CORE // FT_BC) * FT_BC
    if rem:
        chunks.append((N_CORE - rem, rem))

    def mm(ps, x_sb, e_sb, cw, q, lo, hi):
        """fp8 DR matmuls for chunk cols [lo, hi) into ps[:, 0:hi-lo]."""
        for s0 in range(lo, hi, 512):
            sw = min(512, hi - s0)
            for g in range(2):
                lhsT = x_sb[:, g * 2 * B:(g + 1) * 2 * B].rearrange(
                    "p (j b) -> p j b", j=2)[:, :, q * 128:(q + 1) * 128]
                rhs = e_sb[:, g * 2 * cw:(g + 1) * 2 * cw].rearrange(
                    "p (j n) -> p j n", j=2)[:, :, s0:s0 + sw]
                nc.tensor.matmul(ps[:, s0 - lo:s0 - lo + sw], lhsT, rhs,
                                 perf_mode=DR, start=(g == 0), stop=(g == 1))

    with tile.TileContext(nc) as tc:
        with (
            tc.tile_pool(name="xpool", bufs=1) as xpool,
            tc.tile_pool(name="epool", bufs=3) as epool,
            tc.tile_pool(name="psd", bufs=2, space="PSUM") as psd_pool,
            tc.tile_pool(name="psp", bufs=2, space="PSUM") as psp_pool,
            tc.tile_pool(name="s1p", bufs=3) as s1_pool,
            tc.tile_pool(name="wm", bufs=2) as wm_pool,
        ):
            x_sb = xpool.tile([128, 4 * B], F8, tag="x8")
            nc.sync.dma_start(
                x_sb[:, :].rearrange("p (k b) -> p k b", k=4), d_x8[:, :, :])

            wout = 0
            for ci, (c0, cw) in enumerate(chunks):
                half = cw // 2          # cols per engine region
                hq = half // 2          # fold1 pair count per region
                w4 = half // 4          # width-4 window count (DVE region)
                nwc = w4 + half // 2    # window cols this chunk
                e_sb = epool.tile([128, 4 * cw], F8, tag="e8")
                nc.sync.dma_start(
                    e_sb[:, :].rearrange("p (k n) -> p k n", k=4),
                    d_e8[:, :, c0:c0 + cw])
                wm_sb = wm_pool.tile([128, QT * nwc], F16, tag="wm")
                for q in range(QT):
                    # DVE region: chunk cols [0, half)
                    ps_d = psd_pool.tile([128, half], F32, tag="psd")
                    mm(ps_d, x_sb, e_sb, cw, q, 0, half)
                    s1 = s1_pool.tile([128, hq], F16, tag="s1")
                    nc.vector.tensor_tensor(
                        s1[:, :], ps_d[:, :hq], ps_d[:, hq:], op=Max)
                    o = q * nwc
                    nc.vector.tensor_tensor(
                        wm_sb[:, o:o + w4], s1[:, :w4], s1[:, w4:], op=Max)
                    # Pool region: chunk cols [half, cw) -> width-2 windows
                    ps_p = psp_pool.tile([128, half], F32, tag="psp")
                    mm(ps_p, x_sb, e_sb, cw, q, half, cw)
                    nc.gpsimd.tensor_tensor(
                        wm_sb[:, o + w4:o + nwc], ps_p[:, :hq], ps_p[:, hq:],
                        op=Max)
                nc.sync.dma_start(
                    d_wmax[:, :, wout:wout + nwc],
                    wm_sb[:, :].rearrange("p (q w) -> p q w", q=QT))
                wout += nwc

    nc.compile()
    return nc


# f8m variant: hardware-legal consumption of the sim matrix. Real TRN2
# constraints (walrus verifier): an instruction reads at most ONE operand
# from PSUM; gpsimd (Pool) cannot run TensorTensor at all; DVE pool_max is
# rejected. Legal fast path: Act copies half the candidates PSUM->fp16 SBUF
# (0.83 ns/elem) while DVE retires a pair per cycle via
# max(psum_half, staged_half) (one PSUM operand). 1-bank PSUM tiles at
# depth 4 hide the mm->act->t_t chain; stg/wm are per-chunk tiles so WAR
# deps land on ancient instructions.
FM_NW = 6 * 1024 + 256       # width-2 window cols per core (6400)


def _fm_chunks():
    """(c0, cw) list in PROCESSING order: the 512-col remainder first."""
    ch = [(i * FT_BC, FT_BC) for i in range(N_CORE // FT_BC)]
    rem = N_CORE - (N_CORE // FT_BC) * FT_BC
    return ([(N_CORE - rem, rem)] if rem else []) + ch


def _build_f8m(nc):
    Max = mybir.AluOpType.max
    F8 = mybir.dt.float8e4
    DR = mybir.MatmulPerfMode.DoubleRow
    Copy = mybir.ActivationFunctionType.Copy

    d_x8 = nc.dram_tensor("x8", [128, 4, B], F8, kind="ExternalInput")
    d_e8 = nc.dram_tensor("e8", [128, 4, N_CORE], F8, kind="ExternalInput")
    d_wmax = nc.dram_tensor("wmax", [128, QT, FM_NW], F16, kind="ExternalOutput")

    # Big chunks only in the main loop; the 512-col remainder's 16 Act-heavy
    # iterations are interleaved 1-per-8 among the main iterations so both
    # engines stay loaded (as its own phase it would serialize: PE is paced
    # by the psa ring, so a trailing Act-bound phase can't overlap anything)
    chunks = [(i * FT_BC, FT_BC) for i in range(N_CORE // FT_BC)]
    rem_c0 = (N_CORE // FT_BC) * FT_BC
    rem_cw = N_CORE - rem_c0

    def mm(ps, x_sb, e_sb, cw, q, lo, hi):
        for s0 in range(lo, hi, 512):
            sw = min(512, hi - s0)
            for g in range(2):
                lhsT = x_sb[:, g * 2 * B:(g + 1) * 2 * B].rearrange(
                    "p (j b) -> p j b", j=2)[:, :, q * 128:(q + 1) * 128]
                rhs = e_sb[:, g * 2 * cw:(g + 1) * 2 * cw].rearrange(
                    "p (j n) -> p j n", j=2)[:, :, s0:s0 + sw]
                nc.tensor.matmul(ps[:, s0 - lo:s0 - lo + sw], lhsT, rhs,
                                 perf_mode=DR, start=(g == 0), stop=(g == 1))

    with tile.TileContext(nc) as tc:
        with (
            tc.tile_pool(name="xpool", bufs=1) as xpool,
            tc.tile_pool(name="epool", bufs=3) as epool,
            tc.tile_pool(name="psd", bufs=4, space="PSUM") as psd_pool,
            tc.tile_pool(name="psa", bufs=4, space="PSUM") as psa_pool,
            tc.tile_pool(name="stg", bufs=2) as stg_pool,
            tc.tile_pool(name="stg2", bufs=2) as stg2_pool,
            tc.tile_pool(name="wm", bufs=2) as wm_pool,
            tc.tile_pool(name="erem", bufs=1) as erem_pool,
            tc.tile_pool(name="wmrem", bufs=1) as wmrem_pool,
        ):
            x_sb = xpool.tile([128, 4 * B], F8, tag="x8")
            x_view = x_sb[:, :].rearrange("p (k b) -> p k b", k=4)
            e_rem = erem_pool.tile([128, 4 * rem_cw], F8, tag="erem")
            wm_rem = wmrem_pool.tile([128, QT * (rem_cw // 2)], F16, tag="wmr")
            rem_st = [0]     # remainder iterations emitted so far

            def emit_rem():
                qr = rem_st[0]
                rpw = rem_cw // 2
                ps_a = psa_pool.tile([128, rem_cw], F32, tag="psa")
                mm(ps_a, x_sb, e_rem, rem_cw, qr, 0, rem_cw)
                st2 = stg2_pool.tile([128, rem_cw], F16, tag="stg2")
                nc.scalar.activation(st2[:, :], ps_a[:, :], Copy)
                nc.vector.tensor_tensor(
                    wm_rem[:, qr * rpw:(qr + 1) * rpw],
                    st2[:, :rpw], st2[:, rpw:], op=Max)
                rem_st[0] += 1
                if rem_st[0] == QT:   # all done: remainder wm -> cols [0,rpw)
                    nc.sync.dma_start(
                        d_wmax[:, :, 0:rpw],
                        wm_rem[:, :].rearrange("p (q w) -> p q w", q=QT))

            wout = rem_cw // 2
            mit = 0          # global main-iteration counter
            for ci, (c0, cw) in enumerate(chunks):
                nhalf = cw // 1024 if cw >= 1024 else 1
                hw_ = cw // nhalf        # cols per iteration (1024 or 512)
                pw = hw_ // 2            # pair count per iteration
                nwc = QT * nhalf * pw    # wm cols this chunk
                e_sb = epool.tile([128, 4 * cw], F8, tag="e8")
                e_view = e_sb[:, :].rearrange("p (k n) -> p k n", k=4)
                # chunk 0: piece-wise loads (first e8 piece, then x8 by
                # q-group) so the first matmuls start ~3us earlier
                if ci == 0:
                    # load order: x q-group 0 + e8 lower half (unblocks half
                    # 0 for q0-3 together, keeping PE in-order), then the
                    # remainder shard + later x groups, e8 upper half last
                    nc.sync.dma_start(x_view[:, :, 0:512], d_x8[:, :, 0:512])
                    nc.sync.dma_start(e_view[:, :, 0:hw_],
                                      d_e8[:, :, c0:c0 + hw_])
                    nc.sync.dma_start(
                        e_rem[:, :].rearrange("p (k n) -> p k n", k=4),
                        d_e8[:, :, rem_c0:rem_c0 + rem_cw])
                    for xi in range(1, 4):
                        nc.sync.dma_start(
                            x_view[:, :, xi * 512:(xi + 1) * 512],
                            d_x8[:, :, xi * 512:(xi + 1) * 512])
                    nc.sync.dma_start(e_view[:, :, hw_:cw],
                                      d_e8[:, :, c0 + hw_:c0 + cw])
                else:
                    nc.sync.dma_start(e_view[:, :, :], d_e8[:, :, c0:c0 + cw])
                wm_sb = wm_pool.tile([128, nwc], F16, tag="wm")
                stg_sb = stg_pool.tile([128, QT * nhalf * pw], F16, tag="stg")
                if ci == len(chunks) - 1:
                    ng = 8
                elif ci == len(chunks) - 2:
                    ng = 4
                else:
                    ng = 2
                qg = QT // ng
                # (h, q) order: all 16 queries on half 0 before half 1, so
                # chunk 0's later e8 pieces aren't needed until ~iteration 16
                for h in range(nhalf):
                    lo = h * hw_
                    hb = h * QT * pw     # this half's stg/wm column base
                    for q in range(QT):
                        st = stg_sb[:, hb + q * pw:hb + (q + 1) * pw]
                        out = wm_sb[:, hb + q * pw:hb + (q + 1) * pw]
                        ps_d = psd_pool.tile([128, pw], F32, tag="psd")
                        mm(ps_d, x_sb, e_sb, cw, q, lo, lo + pw)
                        ps_a = psa_pool.tile([128, pw], F32, tag="psa")
                        mm(ps_a, x_sb, e_sb, cw, q, lo + pw, lo + hw_)
                        nc.scalar.activation(st, ps_a[:, :], Copy)
                        nc.vector.tensor_tensor(out, ps_d[:, :], st, op=Max)
                        mit += 1
                        if mit % 8 == 5 and rem_st[0] < QT and mit > 8:
                            emit_rem()
                        # ship finished q-groups mid-chunk so the output
                        # transfer overlaps compute; finer groups on the
                        # last chunks to shrink the drain tail
                        if (q + 1) % qg == 0:
                            g0 = (q + 1) // qg - 1
                            nc.sync.dma_start(
                                d_wmax[:, g0 * qg:(g0 + 1) * qg,
                                       wout + h * pw:wout + (h + 1) * pw],
                                wm_sb[:, hb + g0 * qg * pw:hb + (g0 + 1) * qg * pw]
                                .rearrange("p (q w) -> p q w", q=qg))
                wout += nhalf * pw

    nc.compile()
    return nc


def _fm_members():
    """[FM_NW, 4] member map for f8m: all windows width-2 {c, c+pw}, in the
    device's chunk PROCESSING order (remainder first)."""
    M = np.full((FM_NW, 4), N_EMB, np.int64)
    w0 = 0
    for c0, cw in _fm_chunks():
        nhalf = cw // 1024 if cw >= 1024 else 1
        pw = cw // nhalf // 2
        for h in range(nhalf):
            j = np.arange(pw)[:, None]
            M[w0:w0 + pw, :2] = c0 + h * 2 * pw + j + np.arange(2)[None, :] * pw
            w0 += pw
    assert w0 == FM_NW
    return M


def _prep_f8w(xn, e, inv):
    """in_maps for the f8w variant: fp8e4m3 transposed normalized shards,
    scaled by F8_SCALE to stay clear of the fp8 subnormal range."""
    import ml_dtypes
    f8 = ml_dtypes.float8_e4m3
    x8 = _to_f8(np.ascontiguousarray(xn.T) * np.float32(F8_SCALE))
    in_maps = []
    for i in range(CORES):
        lo_r, hi_r = i * N_CORE, (i + 1) * N_CORE
        n_real = max(0, min(hi_r, N_EMB) - lo_r)
        e8 = np.zeros((D, N_CORE), dtype=f8)
        if n_real > 0:
            sl = e[lo_r:lo_r + n_real] * (inv[lo_r:lo_r + n_real]
                                          * np.float32(F8_SCALE))[:, None]
            e8[:, :n_real] = _to_f8(sl.T)
        in_maps.append({"x8": x8, "e8": e8})
    return in_maps


def _prep_f8t(xn, e, inv):
    """f8t in_maps: fp8 shards in [p, slot, cols] layout (slot k = rows
    128k..128k+127 of the [D, *] transposed operand)."""
    import ml_dtypes
    f8 = ml_dtypes.float8_e4m3
    x8 = _to_f8(np.ascontiguousarray(xn.T) * np.float32(F8_SCALE))
    x8 = np.ascontiguousarray(x8.reshape(4, 128, B).transpose(1, 0, 2))
    in_maps = []
    for i in range(CORES):
        lo_r, hi_r = i * N_CORE, (i + 1) * N_CORE
        n_real = max(0, min(hi_r, N_EMB) - lo_r)
        e8 = np.zeros((D, N_CORE), dtype=f8)
        if n_real > 0:
            sl = e[lo_r:lo_r + n_real] * (inv[lo_r:lo_r + n_real]
                                          * np.float32(F8_SCALE))[:, None]
            e8[:, :n_real] = _to_f8(sl.T)
        e8 = np.ascontiguousarray(e8.reshape(4, 128, N_CORE).transpose(1, 0, 2))
        in_maps.append({"x8": x8, "e8": e8})
    return in_maps


def _f8t_members():
    """[FT_NW, 4] member map: core-relative candidate ids of each window."""
    M = np.empty((FT_NW, 4), np.int64)
    w0 = 0
    c0 = 0
    while c0 < N_CORE:
        cw = min(FT_BC, N_CORE - c0)
        quar = cw // 4
        M[w0:w0 + quar] = (c0 + np.arange(quar)[:, None]
                           + np.arange(4)[None, :] * quar)
        w0 += quar
        c0 += cw
    assert w0 == FT_NW
    return M


def _fv_members():
    """[FV_NW, 4] member map for f8v; width-2 windows pad with N_EMB (which
    the merge masks to -inf)."""
    M = np.full((FV_NW, 4), N_EMB, np.int64)
    w0 = 0
    c0 = 0
    while c0 < N_CORE:
        cw = min(FT_BC, N_CORE - c0)
        half, hq, w4 = cw // 2, cw // 4, cw // 8
        j = np.arange(w4)[:, None]
        M[w0:w0 + w4] = c0 + j + np.arange(4)[None, :] * w4
        c = np.arange(hq)[:, None]
        M[w0 + w4:w0 + w4 + hq, :2] = c0 + half + c + np.arange(2)[None, :] * hq
        w0 += w4 + hq
        c0 += cw
    assert w0 == FV_NW
    return M


def _merge_fv(results, labels, xn, e, inv, margin, nw, members):
    """Margin-select windows from per-window maxes, exact rescore of each
    kept window's members, exact top-10 + mode. Works for any window->member
    map `members` [nw, 4] (pad slots with ids >= N_EMB)."""
    tot = CORES * nw
    wv = np.empty((B, tot), np.float32)                      # [B, 8*nw]
    for i, r in enumerate(results):
        src = np.asarray(r["wmax"]).astype(np.float32)       # [128, QT, nw]
        wv[:, i * nw:(i + 1) * nw] = src.transpose(1, 0, 2).reshape(B, nw)
    w10 = np.partition(wv, tot - K_NEIGH, axis=1)[:, tot - K_NEIGH]
    keep = wv >= (w10[:, None] - margin)                     # [B, 8*nw]

    rows_idx, wins = np.nonzero(keep)        # wins: global window ids
    counts = keep.sum(axis=1)
    starts = np.concatenate(([0], np.cumsum(counts[:-1])))
    slots = np.arange(len(wins)) - starts[rows_idx]          # index within row
    smax = int(counts.max())

    e = np.asarray(e, dtype=np.float32)
    xn32 = np.ascontiguousarray(xn, dtype=np.float32)

    # exact rescore, vectorized over (kept window, member) pairs
    mem4 = (wins // nw)[:, None] * N_CORE + members[wins % nw]   # [K, 4]
    valid = mem4 < N_EMB
    pr = np.broadcast_to(rows_idx[:, None], mem4.shape)[valid]
    ps_ = np.broadcast_to(slots[:, None], mem4.shape)[valid]
    pk = np.broadcast_to(np.arange(4)[None, :], mem4.shape)[valid]
    pc = mem4[valid]

    sims = np.full((B, smax, 4), -np.inf, dtype=np.float32)
    wfull = np.zeros((B, smax), dtype=np.int64)
    wfull[rows_idx, slots] = wins
    CH = 1 << 19
    for o in range(0, len(pc), CH):
        r, c = pr[o:o + CH], pc[o:o + CH]
        s = np.einsum("ij,ij->i", xn32[r], e[c], optimize=True) * inv[c]
        sims[r, ps_[o:o + CH], pk[o:o + CH]] = s

    cores_f = wfull // nw
    cand = np.minimum(cores_f[:, :, None] * N_CORE + members[wfull % nw],
                      N_EMB).reshape(B, -1)
    sims = sims.reshape(B, -1)
    u = sims.view(np.uint32)
    mono = np.where(u & 0x80000000, ~u, u | 0x80000000).astype(np.uint64)
    combo = ((np.uint64(0xFFFFFFFF) - mono) << np.uint64(17)) | \
        cand.astype(np.uint64)
    combo[sims == -np.inf] = np.uint64(0xFFFFFFFFFFFFFFFF)
    ordr = np.argsort(combo, axis=1, kind="stable")[:, :K_NEIGH]
    neighbors = np.take_along_axis(cand, ordr, axis=1)
    return _mode_pred(neighbors, labels)


def _merge_f8t(results, labels, xn, e, inv, margin):
    """Margin-select windows from per-window maxes, exact rescore of the
    4 members of each kept window, exact top-10 + mode."""
    # device output [p, q, w] -> rows q*128+p
    wv = np.concatenate(
        [np.asarray(r["wmax"]).transpose(1, 0, 2).reshape(B, FT_NW)
         for r in results], axis=1).astype(np.float32)       # [B, 8*FT_NW]
    nw = wv.shape[1]
    w10 = np.partition(wv, nw - K_NEIGH, axis=1)[:, nw - K_NEIGH]
    keep = wv >= (w10[:, None] - margin)                     # [B, 8*FT_NW]

    rows_idx, wins = np.nonzero(keep)        # wins: global window ids
    slots = (np.cumsum(keep, axis=1) - 1)[rows_idx, wins]
    smax = int(keep.sum(axis=1).max())

    M = _f8t_members()                       # [FT_NW, 4]
    e = np.asarray(e, dtype=np.float32)
    xn32 = np.ascontiguousarray(xn, dtype=np.float32)
    order = np.argsort(wins, kind="stable")
    rows_s, slots_s, wins_s = rows_idx[order], slots[order], wins[order]
    uniq, starts = np.unique(wins_s, return_index=True)
    bounds = np.append(starts, len(wins_s))

    sims = np.full((B, smax, 4), -np.inf, dtype=np.float32)
    wfull = np.zeros((B, smax), dtype=np.int64)
    wfull[rows_idx, slots] = wins
    for ui in range(len(uniq)):
        w = int(uniq[ui])
        core, wloc = divmod(w, FT_NW)
        mem = core * N_CORE + M[wloc]                        # [4] global ids
        valid = mem < N_EMB
        if not valid.any():
            continue
        mv = mem[valid]
        s0, s1 = bounds[ui], bounds[ui + 1]
        en_w = e[mv] * inv[mv][:, None]
        sblk = xn32[rows_s[s0:s1]] @ en_w.T                  # [nrows, <=4]
        sims[rows_s[s0:s1][:, None], slots_s[s0:s1][:, None],
             np.nonzero(valid)[0][None, :]] = sblk

    cores_f = wfull // FT_NW
    cand = (cores_f[:, :, None] * N_CORE + M[wfull % FT_NW]).reshape(B, -1)
    sims = sims.reshape(B, -1)
    u = sims.view(np.uint32)
    mono = np.where(u & 0x80000000, ~u, u | 0x80000000).astype(np.uint64)
    combo = ((np.uint64(0xFFFFFFFF) - mono) << np.uint64(17)) | \
        cand.astype(np.uint64)
    combo[sims == -np.inf] = np.uint64(0xFFFFFFFFFFFFFFFF)
    ordr = np.argsort(combo, axis=1, kind="stable")[:, :K_NEIGH]
    neighbors = np.take_along_axis(cand, ordr, axis=1)
    return _mode_pred(neighbors, labels)


def _get_nc(variant=None):
    variant = variant or MM_DTYPE
    if variant not in _CACHE:
        _CACHE[variant] = _build(variant)
    return _CACHE[variant]


def _normalize(x, embeddings):
    x = np.asarray(x, dtype=np.float32)
    e = np.asarray(embeddings, dtype=np.float32)
    xn = x / np.maximum(np.linalg.norm(x, axis=1, keepdims=True), EPS)
    inv = (1.0 / np.maximum(np.linalg.norm(e, axis=1), EPS)).astype(np.float32)
    return xn, e, inv


def _prep_f16w(xn, e, inv):
    """in_maps for the f16w variant: fp16 transposed normalized shards."""
    xh = np.ascontiguousarray(xn.T).astype(np.float16)
    in_maps = []
    for i in range(CORES):
        lo_r, hi_r = i * N_CORE, (i + 1) * N_CORE
        n_real = max(0, min(hi_r, N_EMB) - lo_r)
        eh = np.zeros((D, N_CORE), dtype=np.float16)
        if n_real > 0:
            sl = e[lo_r:lo_r + n_real] * inv[lo_r:lo_r + n_real][:, None]
            eh[:, :n_real] = sl.T.astype(np.float16)
        in_maps.append({"xh": xh, "eh": eh})
    return in_maps


def _prep_inputs(x, embeddings, variant):
    """Host prep: normalize embeddings, pad, transpose, shard; returns in_maps.

    Works per-core-shard to keep intermediates cache-sized."""
    if variant == "f16w":
        xn, e, inv = _normalize(x, embeddings)
        return _prep_f16w(xn, e, inv)
    if variant in ("f8t", "f8v", "f8m"):
        xn, e, inv = _normalize(x, embeddings)
        return _prep_f8t(xn, e, inv)
    if variant in ("f8w", "f8d", "f8e"):
        xn, e, inv = _normalize(x, embeddings)
        return _prep_f8w(xn, e, inv)
    x = np.asarray(x, dtype=np.float32)
    e = np.asarray(embeddings, dtype=np.float32)
    inv = (1.0 / np.maximum(np.linalg.norm(e, axis=1), EPS)).astype(np.float32)
    xt = np.ascontiguousarray(x.T)               # [D, B]

    in_maps = []
    for i in range(CORES):
        lo_r, hi_r = i * N_CORE, (i + 1) * N_CORE
        n_real = max(0, min(hi_r, N_EMB) - lo_r)
        ent = np.zeros((D, N_CORE), dtype=np.float32)
        if n_real > 0:
            sl = e[lo_r:lo_r + n_real]
            ent[:, :n_real] = sl.T * inv[lo_r:lo_r + n_real][None, :]
        if variant == "f16x3":
            ehi = ent.astype(np.float16)
            elo = (ent - ehi).astype(np.float16)
            in_maps.append({"ehi": ehi, "elo": elo})
        else:
            in_maps.append({"ent": ent})

    if variant == "f16x3":
        xhi = xt.astype(np.float16)
        xlo = (xt - xhi).astype(np.float16)
        for m in in_maps:
            m["xhi"] = xhi
            m["xlo"] = xlo
    else:
        for m in in_maps:
            m["xt"] = xt
    return in_maps


def _merge(results, labels):
    """Host merge: exact global top-10 from per-core per-chunk top-8 pools,
    then the reference's mode computation."""
    vals = np.concatenate([r["vals"] for r in results], axis=1)   # [B, 8*NOUT]
    idx8 = np.concatenate([r["idx"] for r in results], axis=1).astype(np.int64)

    col_base = (np.arange(NOUT, dtype=np.int64) // 8) * CHUNK      # chunk offset
    core_base = np.repeat(np.arange(CORES, dtype=np.int64) * N_CORE, NOUT)
    g = idx8 + np.tile(col_base, CORES)[None, :] + core_base[None, :]

    # padding rows (g >= N_EMB) are zero embeddings: exclude
    u = vals.view(np.uint32)
    key = np.where(u & 0x80000000, ~u, u | 0x80000000).astype(np.uint64)
    combo = ((np.uint64(0xFFFFFFFF) - key) << np.uint64(17)) | g.astype(np.uint64)
    combo[g >= N_EMB] = np.uint64(0xFFFFFFFFFFFFFFFF)
    order = np.argsort(combo, axis=1, kind="stable")[:, :K_NEIGH]
    neighbors = np.take_along_axis(g, order, axis=1)               # [B, 10]

    labels = np.asarray(labels)
    nl = labels[neighbors].astype(np.int64)                        # [B, 10]
    eq = nl[:, :, None] == nl[:, None, :]
    counts = eq.sum(-1)
    mkey = counts * (NUM_CLASSES + 1) + (NUM_CLASSES - nl)
    mi = np.argmax(mkey, axis=1)
    pred = np.take_along_axis(nl, mi[:, None], axis=1)[:, 0]
    return pred.astype(labels.dtype)


class _Runner:
    """Caches the shard_map-jitted executable across calls (mirrors
    bass2jax.run_bass_via_pjrt's multi-core path, which re-traces per call)."""

    def __init__(self, variant):
        import jax
        import concourse.mybir as mb
        from concourse import bass2jax
        from jax.experimental.shard_map import shard_map
        from jax.sharding import Mesh, PartitionSpec

        bass2jax.install_neuronx_cc_hook()
        self.jax = jax
        nc = _get_nc(variant)
        partition_name = (nc.partition_id_tensor.name
                          if nc.partition_id_tensor else None)
        in_names, out_names, out_avals, zeros = [], [], [], []
        for alloc in nc.m.functions[0].allocations:
            if not isinstance(alloc, mb.MemoryLocationSet):
                continue
            name = alloc.memorylocations[0].name
            if alloc.kind == "ExternalInput":
                if name != partition_name:
                    in_names.append(name)
            elif alloc.kind == "ExternalOutput":
                shape = tuple(alloc.tensor_shape)
                dtype = mb.dt.np(alloc.dtype)
                out_avals.append(jax.core.ShapedArray(shape, dtype))
                out_names.append(name)
                zeros.append(np.zeros((CORES * shape[0],) + shape[1:], dtype))
        self.in_names = list(in_names)
        self.out_names = out_names
        self.out_avals = out_avals
        self.zeros = zeros
        n_params = len(in_names)
        all_names = in_names + out_names
        if partition_name is not None:
            all_names = all_names + [partition_name]
        donate = tuple(range(n_params, n_params + len(out_names)))

        def _body(*args):
            operands = list(args)
            if partition_name is not None:
                operands.append(bass2jax.partition_id_tensor())
            outs = bass2jax._bass_exec_p.bind(
                *operands,
                out_avals=tuple(out_avals),
                in_names=tuple(all_names),
                out_names=tuple(out_names),
                lowering_input_output_aliases=(),
                sim_require_finite=True,
                sim_require_nnan=True,
                nc=nc,
            )
            return tuple(outs)

        devices = jax.devices()[:CORES]
        self.mesh = Mesh(np.asarray(devices), ("core",))
        self.pspec = PartitionSpec("core")
        in_specs = (self.pspec,) * (n_params + len(out_names))
        out_specs = (self.pspec,) * len(out_names)
        self.sharded = jax.jit(
            shard_map(_body, mesh=self.mesh, in_specs=in_specs,
                      out_specs=out_specs, check_rep=False),
            donate_argnums=donate, keep_unused=True,
        )

    def concat_inputs(self, in_maps):
        return [
            np.concatenate([np.asarray(m[name]) for m in in_maps], axis=0)
            for name in self.in_names
        ]

    def device_put(self, concat_in):
        from jax.sharding import NamedSharding
        sh = NamedSharding(self.mesh, self.pspec)
        return [self.jax.device_put(a, sh) for a in concat_in]

    def execute(self, concat_in):
        zeros = [np.zeros_like(z) for z in self.zeros]
        out_arrs = self.sharded(*concat_in, *zeros)
        return out_arrs

    def run(self, in_maps):
        out_arrs = self.execute(self.concat_inputs(in_maps))
        return [
            {
                name: np.asarray(out_arrs[i]).reshape(
                    CORES, *self.out_avals[i].shape)[c]
                for i, name in enumerate(self.out_names)
            }
            for c in range(CORES)
        ]


_RUNNERS = {}


def _get_runner(variant=None):
    variant = variant or MM_DTYPE
    if variant not in _RUNNERS:
        _RUNNERS[variant] = _Runner(variant)
    return _RUNNERS[variant]


def _mode_pred(neighbors, labels):
    """Reference's torch.mode semantics on gathered neighbor labels."""
    labels = np.asarray(labels)
    nl = labels[neighbors].astype(np.int64)                        # [B, 10]
    eq = nl[:, :, None] == nl[:, None, :]
    counts = eq.sum(-1)
    mkey = counts * (NUM_CLASSES + 1) + (NUM_CLASSES - nl)
    mi = np.argmax(mkey, axis=1)
    pred = np.take_along_axis(nl, mi[:, None], axis=1)[:, 0]
    return pred.astype(labels.dtype)


def _merge_f16w(results, labels, xn, e, inv, margin=MARGIN):
    """Select windows >= (10th-best window max) - margin, rescore those
    candidates exactly in fp64, exact global top-10, then mode."""
    wv = np.stack([r["wvals"] for r in results], axis=1)      # [B, 8, 32]
    wi = np.stack([r["widx"] for r in results], axis=1).astype(np.int64)
    wi[:, :, NSEL:] += HALF_A   # half-B indices are relative to its slice
    gw = wi + (np.arange(CORES, dtype=np.int64) * WPC)[None, :, None]
    wv = wv.reshape(B, CORES * 2 * NSEL)
    gw = gw.reshape(B, CORES * 2 * NSEL)

    w10 = np.partition(wv, wv.shape[1] - K_NEIGH, axis=1)[:, wv.shape[1] - K_NEIGH]
    keep = wv >= (w10[:, None] - margin)
    smax = int(keep.sum(axis=1).max())

    # top-smax windows per row by value; mask out ones below the cutoff
    order = np.argsort(-wv, axis=1, kind="stable")[:, :smax]
    sel_g = np.take_along_axis(gw, order, axis=1)              # [B, smax]
    sel_keep = np.take_along_axis(keep, order, axis=1)

    # rescore grouped by window: each window's embeddings are one contiguous
    # 32-row slice, shared by every query that selected it (~6400 windows
    # total vs ~170k (row, window) pairs -> tiny gathers, BLAS-sized GEMMs)
    e = np.asarray(e, dtype=np.float32)
    xn32 = np.ascontiguousarray(xn, dtype=np.float32)
    rows_idx, slots = np.nonzero(sel_keep)
    wins = sel_g[rows_idx, slots]
    order = np.argsort(wins, kind="stable")
    rows_idx, slots, wins = rows_idx[order], slots[order], wins[order]
    uniq, starts = np.unique(wins, return_index=True)
    bounds = np.append(starts, len(wins))

    sims = np.full((B, smax, WWIN), -np.inf, dtype=np.float32)
    for ui in range(len(uniq)):
        w = int(uniq[ui])
        c0, c1 = w * WWIN, min(w * WWIN + WWIN, N_EMB)
        if c1 <= c0:
            continue
        s0, s1 = bounds[ui], bounds[ui + 1]
        en_w = e[c0:c1] * inv[c0:c1][:, None]                  # [<=32, D]
        sblk = xn32[rows_idx[s0:s1]] @ en_w.T                  # [nrows, <=32]
        sims[rows_idx[s0:s1], slots[s0:s1], :c1 - c0] = sblk

    cand = (sel_g[:, :, None] * WWIN +
            np.arange(WWIN, dtype=np.int64)[None, None, :]).reshape(B, -1)
    sims = sims.reshape(B, -1)

    # exact top-10 by (-sim, cand) via an order-preserving uint64 key
    u = sims.view(np.uint32)
    mono = np.where(u & 0x80000000, ~u, u | 0x80000000).astype(np.uint64)
    combo = ((np.uint64(0xFFFFFFFF) - mono) << np.uint64(17)) | \
        cand.astype(np.uint64)
    combo[sims == -np.inf] = np.uint64(0xFFFFFFFFFFFFFFFF)
    ordr = np.argsort(combo, axis=1, kind="stable")[:, :K_NEIGH]
    neighbors = np.take_along_axis(cand, ordr, axis=1)
    return _mode_pred(neighbors, labels)


def _merge_f8d(results, labels, xn, e, inv, margin):
    """Host-side window selection from the full per-window-max arrays, then
    the window-grouped exact rescore."""
    wv = np.concatenate([r["wmax"] for r in results], axis=1)   # [B, 8*WPC]
    nw = wv.shape[1]
    w10 = np.partition(wv, nw - K_NEIGH, axis=1)[:, nw - K_NEIGH]
    keep = wv >= (w10[:, None] - margin)                        # [B, 8*WPC]

    rows_idx, wins = np.nonzero(keep)        # wins are global window ids
    slots = (np.cumsum(keep, axis=1) - 1)[rows_idx, wins]
    smax = int(keep.sum(axis=1).max())

    e = np.asarray(e, dtype=np.float32)
    xn32 = np.ascontiguousarray(xn, dtype=np.float32)
    order = np.argsort(wins, kind="stable")
    rows_s, slots_s, wins_s = rows_idx[order], slots[order], wins[order]
    uniq, starts = np.unique(wins_s, return_index=True)
    bounds = np.append(starts, len(wins_s))

    sims = np.full((B, smax, WWIN), -np.inf, dtype=np.float32)
    wfull = np.zeros((B, smax), dtype=np.int64)
    wfull[rows_idx, slots] = wins
    for ui in range(len(uniq)):
        w = int(uniq[ui])
        c0, c1 = w * WWIN, min(w * WWIN + WWIN, N_EMB)
        if c1 <= c0:
            continue
        s0, s1 = bounds[ui], bounds[ui + 1]
        en_w = e[c0:c1] * inv[c0:c1][:, None]
        sblk = xn32[rows_s[s0:s1]] @ en_w.T
        sims[rows_s[s0:s1], slots_s[s0:s1], :c1 - c0] = sblk

    cand = (wfull[:, :, None] * WWIN +
            np.arange(WWIN, dtype=np.int64)[None, None, :]).reshape(B, -1)
    sims = sims.reshape(B, -1)
    u = sims.view(np.uint32)
    mono = np.where(u & 0x80000000, ~u, u | 0x80000000).astype(np.uint64)
    combo = ((np.uint64(0xFFFFFFFF) - mono) << np.uint64(17)) | \
        cand.astype(np.uint64)
    combo[sims == -np.inf] = np.uint64(0xFFFFFFFFFFFFFFFF)
    ordr = np.argsort(combo, axis=1, kind="stable")[:, :K_NEIGH]
    neighbors = np.take_along_axis(cand, ordr, axis=1)
    return _mode_pred(neighbors, labels)


def run_on_hw(x, embeddings, variant=None):
    runner = _get_runner(variant)
    in_maps = _prep_inputs(x, embeddings, variant or MM_DTYPE)
    return runner.run(in_maps)


def kernel(x, embeddings, labels):
    variant = MM_DTYPE
    if variant == "f16w":
        xn, e, inv = _normalize(x, embeddings)
        runner = _get_runner(variant)
        results = runner.run(_prep_f16w(xn, e, inv))
        return _merge_f16w(results, labels, xn, e, inv)
    if variant == "f8w":
        xn, e, inv = _normalize(x, embeddings)
        runner = _get_runner(variant)
        results = runner.run(_prep_f8w(xn, e, inv))
        return _merge_f16w(results, labels, xn, e, inv,
                           margin=MARGIN_F8 * F8_SCALE * F8_SCALE)
    if variant == "f8t":
        xn, e, inv = _normalize(x, embeddings)
        runner = _get_runner(variant)
        results = runner.run(_prep_f8t(xn, e, inv))
        # margin: fp8 sim error (scaled) + 2x fp16 rounding of the maxes
        return _merge_f8t(results, labels, xn, e, inv,
                          margin=MARGIN_F8 * F8_SCALE * F8_SCALE + 0.5)
    if variant == "f8v":
        xn, e, inv = _normalize(x, embeddings)
        runner = _get_runner(variant)
        results = runner.run(_prep_f8t(xn, e, inv))
        return _merge_fv(results, labels, xn, e, inv,
                         MARGIN_F8 * F8_SCALE * F8_SCALE + 0.5,
                         FV_NW, _fv_members())
    if variant == "f8m":
        xn, e, inv = _normalize(x, embeddings)
        runner = _get_runner(variant)
        results = runner.run(_prep_f8t(xn, e, inv))
        return _merge_fv(results, labels, xn, e, inv,
                         MARGIN_F8 * F8_SCALE * F8_SCALE + 0.5,
                         FM_NW, _fm_members())
    if variant in ("f8d", "f8e"):
        xn, e, inv = _normalize(x, embeddings)
        runner = _get_runner(variant)
        results = runner.run(_prep_f8w(xn, e, inv))
        return _merge_f8d(results, labels, xn, e, inv,
                          margin=MARGIN_F8 * F8_SCALE * F8_SCALE)
    results = run_on_hw(x, embeddings)
    return _merge(results, labels)

